# revision 1
# baseline (speedup 1.0000x reference)
"""Trainium2 Bass kernel for nn_EnhancedGATGCN (GAT -> GCN -> pool -> MLP, + protein conv branch).

Self-contained: host-side sharding prep + 8-core SPMD Bass/Tile device program.

Sharding strategy:
  - Edges (incl. self loops) sorted by dst, partitioned into 8 contiguous dst
    ranges of 2500 nodes; scatter-add is core-local via one-hot matmuls on the
    TensorEngine with PSUM accumulation per 128-dst window.
  - Node feature tables (h for GAT, dinv-scaled h2 for GCN) are computed
    node-sharded, AllGathered as bf16 tables in HBM, and per-edge messages are
    fetched with SWDGE dma_gather (f32 attention logits bit-packed into the
    bf16 rows). GAT softmax denominator rides the scatter matmul as extra rhs
    columns; a_d[dst] is expanded per edge with a transposed-mask matmul; GCN
    sym-norm is factored into per-node dinv scales.
  - Dense tail (conv/MLPs) is data-parallel over the 200-graph batch (25/core)
    and scheduled to overlap the AllGathers / gather-bound stretches.
"""
import os
import sys

import numpy as np

sys.path.insert(0, "/opt/trn_rl_repo")

import ml_dtypes

import concourse.bacc as bacc
import concourse.bass as bass
import concourse.mybir as mybir
import concourse.tile as tile
from concourse.bass_utils import run_bass_kernel_spmd
from concourse.masks import make_identity

F32 = mybir.dt.float32
BF16 = mybir.dt.bfloat16
I16 = mybir.dt.int16
I32 = mybir.dt.int32
AF = mybir.ActivationFunctionType
OP = mybir.AluOpType
BF = ml_dtypes.bfloat16

N, E, B, H, F = 20000, 400000, 200, 10, 78
HID = H * F  # 780
SEQ, VOC, EMB, NF, KS = 1000, 26, 128, 32, 8
CONV_OUT = SEQ - KS + 1  # 993

NCORES = 8
NPC = N // NCORES  # 2500
NPAD = 2560
NWIN = NPAD // 128  # 20
RBF = 896  # bf16 cols per table row; bytes = 1792 (%256==0)
# row: [0:780 h bf16 | 780:800 a_s 10xf32 | 800:896 zeros (820:830 exd scratch on msg tiles)]
GSLOT = 64
MY_G = 32
BPC = B // NCORES  # 25
TOK = BPC * SEQ
TOKPAD = 25600  # 5 groups x 5120 (each: 5 graphs x 1000 tok + 120 pad)
KPAD = 32 * 1024


# ---------------------------------------------------------------- host prep


def _wrap16(idx, epc):
    a = np.zeros((128, epc // 16), np.int16)
    w = idx.reshape(epc // 16, 16).T
    a[:, :] = np.tile(w, (8, 1))
    return a


def host_prep(inputs):
    x = np.asarray(inputs["x"], np.float32)
    edge_index = np.asarray(inputs["edge_index"], np.int64)
    batch = np.asarray(inputs["batch"], np.int64)
    target = np.asarray(inputs["target"], np.int64)

    loops = np.arange(N, dtype=np.int64)
    src = np.concatenate([edge_index[0], loops])
    dst = np.concatenate([edge_index[1], loops])
    order = np.argsort(dst, kind="stable")
    src, dst = src[order], dst[order]

    core_of = dst // NPC
    dst_local = dst - core_of * NPC
    win = dst_local // 128
    maxw = 0
    per_core_edges = []
    for c in range(NCORES):
        m = core_of == c
        s_c, dl_c, w_c = src[m], dst_local[m], win[m]
        per_core_edges.append((s_c, dl_c, w_c))
        maxw = max(maxw, int(np.bincount(w_c, minlength=NWIN).max()))
    tpw = -(-maxw // 128)
    tpw = -(-tpw // 4) * 4
    ntile = NWIN * tpw
    epc = ntile * 128
    nchunk = ntile // 16

    def remap(n):
        return (n // NPC) * NPAD + (n % NPC)

    cores = []
    for c in range(NCORES):
        s_c, dl_c, w_c = per_core_edges[c]
        es = np.zeros(epc, np.int64)
        ew = np.full(epc, -1000.0, np.float32)
        for w in range(NWIN):
            m = w_c == w
            k = int(m.sum())
            o = w * tpw * 128
            es[o : o + k] = s_c[m]
            ew[o : o + k] = (dl_c[m] - w * 128).astype(np.float32)
        cores.append(dict(es=remap(es), ew=ew))

    gat_W = np.asarray(inputs["gat_W"], np.float32)
    wpack0 = np.zeros((78, 1024), np.float32)
    wpack0[:, :HID] = gat_W
    gat_WT_pad = np.zeros((896, 78), np.float32)
    gat_WT_pad[:HID] = gat_W.T
    wasbd = np.zeros((896, 20), np.float32)
    a_src = np.asarray(inputs["gat_a_src"], np.float32)
    a_dst = np.asarray(inputs["gat_a_dst"], np.float32)
    for h in range(H):
        wasbd[h * F : (h + 1) * F, h] = a_src[h]
        wasbd[h * F : (h + 1) * F, 10 + h] = a_dst[h]

    gcn_W_pad = np.zeros((896, HID), np.float32)
    gcn_W_pad[:HID] = np.asarray(inputs["gcn_W"], np.float32)
    fcg1_W_pad = np.zeros((896, 1500), np.float32)
    fcg1_W_pad[:HID] = np.asarray(inputs["fcg1_W"], np.float32)
    fcg2_W_pad = np.zeros((1536, 128), np.float32)
    fcg2_W_pad[:1500] = np.asarray(inputs["fcg2_W"], np.float32)

    fxt_W = np.asarray(inputs["fxt_W"], np.float32)
    fxt_Wp = np.zeros((KPAD, 128), np.float32)
    fxt_Wp.reshape(NF, 1024, 128)[:, :CONV_OUT] = fxt_W.reshape(NF, CONV_OUT, 128)

    cW = np.asarray(inputs["cW"], np.float32)
    cwt = np.ascontiguousarray(cW.transpose(2, 1, 0))  # [8, 128, 32]

    gbase = np.array([batch[c * NPC] for c in range(NCORES)], np.int64)
    span = np.array(
        [batch[min(c * NPC + NPC, N) - 1] - gbase[c] + 1 for c in range(NCORES)]
    )
    assert span.max() <= GSLOT, span.max()
    Cc_all = []
    for c in range(NCORES):
        Cmat = np.zeros((NCORES * GSLOT, MY_G), np.float32)
        for r in range(NCORES):
            for slot in range(GSLOT):
                g = gbase[r] + slot
                col = g - c * BPC
                if 0 <= col < BPC and g < B:
                    Cmat[r * GSLOT + slot, col] = 1.0
        Cc_all.append(Cmat)

    meta = dict(tpw=tpw, ntile=ntile, epc=epc, nchunk=nchunk)

    per_core = []
    for c in range(NCORES):
        ed_ = cores[c]
        bw = np.full(NPAD, -1000.0, np.float32)
        bw[:NPC] = (batch[c * NPC : (c + 1) * NPC] - gbase[c]).astype(np.float32)
        batchw = bw.reshape(NWIN, 128).T.copy()

        tgt = np.zeros(TOKPAD, np.int64)
        tg = target[c * BPC : (c + 1) * BPC].reshape(5, 5 * SEQ)
        tgt.reshape(5, 5120)[:, : 5 * SEQ] = tg

        dstw = ed_["ew"].reshape(ntile, 128).T.copy()

        xTc = np.zeros((78, NPAD), np.float32)
        xTc[:, :NPC] = x[c * NPC : (c + 1) * NPC].T

        d = {
            "xTc": xTc,
            "wpack0": wpack0,
            "gat_WT": gat_WT_pad,
            "wasbd": wasbd,
            "src16": _wrap16(ed_["es"], epc),
            "tgt16": _wrap16(tgt, TOKPAD),
            "dstw": dstw,
            "batchw": batchw,
            "Cc": Cc_all[c],
            "gat_b": np.asarray(inputs["gat_b"], np.float32).reshape(1, HID),
            "gcn_Wp": gcn_W_pad,
            "gcn_b": np.asarray(inputs["gcn_b"], np.float32).reshape(1, HID),
            "fcg1_Wp": fcg1_W_pad,
            "fcg1_b": np.asarray(inputs["fcg1_b"], np.float32).reshape(1, 1500),
            "fcg2_Wp": fcg2_W_pad,
            "fcg2_b": np.asarray(inputs["fcg2_b"], np.float32).reshape(1, 128),
            "emb_bf": np.asarray(inputs["emb"], np.float32).astype(BF),
            "cwt_bf": cwt.astype(BF),
            "cb": np.asarray(inputs["cb"], np.float32).reshape(NF, 1),
            "fxt_Wp": fxt_Wp,
            "fxt_b": np.asarray(inputs["fxt_b"], np.float32).reshape(1, 128),
            "f1_W": np.asarray(inputs["f1_W"], np.float32),
            "f1_b": np.asarray(inputs["f1_b"], np.float32).reshape(1, 1024),
            "f2_W": np.asarray(inputs["f2_W"], np.float32),
            "f2_b": np.asarray(inputs["f2_b"], np.float32).reshape(1, 512),
            "f3_W": np.asarray(inputs["f3_W"], np.float32),
            "f3_b": np.asarray(inputs["f3_b"], np.float32).reshape(1, 256),
            "f4_W": np.asarray(inputs["f4_W"], np.float32),
            "f4_b": np.asarray(inputs["f4_b"], np.float32).reshape(1, 128),
            "o_W": np.asarray(inputs["o_W"], np.float32),
            "o_b": np.asarray(inputs["o_b"], np.float32).reshape(1, 1),
        }
        per_core.append(d)
    return per_core, meta


# ---------------------------------------------------------------- device build

_CACHE = {}


def build_bass(meta):
    PH = int(os.environ.get("KPHASE", "9"))
    key = (meta["tpw"], PH)
    if key in _CACHE:
        return _CACHE[key]

    tpw, ntile, epc, nchunk = meta["tpw"], meta["ntile"], meta["epc"], meta["nchunk"]

    nc = bacc.Bacc(
        "TRN2",
        target_bir_lowering=False,
        debug=False,
        num_devices=NCORES,
        num_swdge_queues=2,
    )

    def inp(name, shape, dt=F32):
        return nc.dram_tensor(name, list(shape), dt, kind="ExternalInput")

    xTc = inp("xTc", (78, NPAD))
    wpack0 = inp("wpack0", (78, 1024))
    gat_WT = inp("gat_WT", (896, 78))
    wasbd = inp("wasbd", (896, 20))
    src16 = inp("src16", (128, epc // 16), I16)
    tgt16 = inp("tgt16", (128, TOKPAD // 16), I16)
    dstw = inp("dstw", (128, ntile))
    batchw = inp("batchw", (128, NWIN))
    Cc = inp("Cc", (NCORES * GSLOT, MY_G))
    gat_b = inp("gat_b", (1, HID))
    gcn_Wp = inp("gcn_Wp", (896, HID))
    gcn_b = inp("gcn_b", (1, HID))
    fcg1_Wp = inp("fcg1_Wp", (896, 1500))
    fcg1_b = inp("fcg1_b", (1, 1500))
    fcg2_Wp = inp("fcg2_Wp", (1536, 128))
    fcg2_b = inp("fcg2_b", (1, 128))
    emb_bf = inp("emb_bf", (VOC, EMB), BF16)
    cwt_bf = inp("cwt_bf", (KS, EMB, NF), BF16)
    cb = inp("cb", (NF, 1))
    fxt_Wp = inp("fxt_Wp", (KPAD, 128))
    fxt_b = inp("fxt_b", (1, 128))
    f1_W = inp("f1_W", (256, 1024))
    f1_b = inp("f1_b", (1, 1024))
    f2_W = inp("f2_W", (1024, 512))
    f2_b = inp("f2_b", (1, 512))
    f3_W = inp("f3_W", (512, 256))
    f3_b = inp("f3_b", (1, 256))
    f4_W = inp("f4_W", (256, 128))
    f4_b = inp("f4_b", (1, 128))
    o_W = inp("o_W", (128, 1))
    o_b = inp("o_b", (1, 1))
    out_d = nc.dram_tensor("out", [MY_G, 1], F32, kind="ExternalOutput")
    KDEBUG = bool(int(os.environ.get("KDEBUG", "0")))
    if KDEBUG:
        out_x1 = nc.dram_tensor("out_x1", [NPAD, HID], F32, kind="ExternalOutput")
        out_h = nc.dram_tensor("out_h", [256, RBF], BF16, kind="ExternalOutput")
        out_adw = nc.dram_tensor("out_adw", [128, NWIN * 10], BF16, kind="ExternalOutput")
        out_xt = nc.dram_tensor("out_xt", [MY_G, 128], F32, kind="ExternalOutput")

    hin = nc.dram_tensor("hin", [NPAD, RBF], BF16)
    htabG = nc.dram_tensor("htabG", [NCORES * NPAD, RBF], BF16, addr_space="Shared")
    x1d = nc.dram_tensor("x1d", [NPAD, HID], F32)
    agin = nc.dram_tensor("agin", [NPAD, RBF], BF16)
    htab2G = nc.dram_tensor("htab2G", [NCORES * NPAD, RBF], BF16, addr_space="Shared")
    poolin = nc.dram_tensor("poolin", [GSLOT, HID], F32)
    poolall = nc.dram_tensor("poolall", [NCORES * GSLOT, HID], F32, addr_space="Shared")

    RG = [list(range(NCORES))]

    with tile.TileContext(nc) as tc:
        import contextlib

        ctx = contextlib.ExitStack()
        with ctx:
            pers = ctx.enter_context(tc.tile_pool(name="pers", bufs=1))

            # consts
            iota_i = pers.tile([128, 128], I32)
            nc.gpsimd.iota(iota_i[:], pattern=[[1, 128]], base=0, channel_multiplier=0)
            iota_f = pers.tile([128, 128], F32)
            nc.vector.tensor_copy(iota_f[:], iota_i[:])
            ident = pers.tile([128, 128], F32)
            make_identity(nc, ident[:])
            ident_bf = pers.tile([128, 128], BF16)
            nc.vector.tensor_copy(ident_bf[:], ident[:])
            ones1 = pers.tile([1, 128], F32)
            nc.gpsimd.memset(ones1[:], 1.0)

            bias_tiles = {}
            with tc.tile_pool(name="psB", bufs=1, space="PSUM") as psB:

                def bcast_bias(dram, width, name):
                    t = pers.tile([128, width], F32, tag=f"bc_{name}")
                    row = pers.tile([1, width], F32, tag=f"br_{name}")
                    nc.sync.dma_start(row[:], dram[0:1, :])
                    for n0 in range(0, width, 512):
                        nn = min(512, width - n0)
                        ps = psB.tile([128, 512], F32, space="PSUM", tag="bcps")
                        nc.tensor.matmul(
                            ps[:, :nn], lhsT=ones1[:], rhs=row[:, n0 : n0 + nn],
                            start=True, stop=True,
                        )
                        nc.any.tensor_copy(t[:, n0 : n0 + nn], ps[:, :nn])
                    return t

                gatb_bc = bcast_bias(gat_b, HID, "gatb")
                gcnb_bc = bcast_bias(gcn_b, HID, "gcnb")
                fcg1b_bc = bcast_bias(fcg1_b, 1500, "fcg1b")
                fcg2b_bc = bcast_bias(fcg2_b, 128, "fcg2b")
                fxtb_bc = bcast_bias(fxt_b, 128, "fxtb")
                f1b_bc = bcast_bias(f1_b, 1024, "f1b")
                f2b_bc = bcast_bias(f2_b, 512, "f2b")
                f3b_bc = bcast_bias(f3_b, 256, "f3b")
                f4b_bc = bcast_bias(f4_b, 128, "f4b")
                ob_bc = bcast_bias(o_b, 1, "ob")

            # residents
            dstw_t = pers.tile([128, ntile], F32)
            nc.sync.dma_start(dstw_t[:], dstw[:, :])
            batchw_t = pers.tile([128, NWIN], F32)
            nc.sync.dma_start(batchw_t[:], batchw[:, :])
            src_t = pers.tile([128, epc // 16], I16)
            nc.sync.dma_start(src_t[:], src16[:, :])
            tgt_t = pers.tile([128, TOKPAD // 16], I16)
            nc.sync.dma_start(tgt_t[:], tgt16[:, :])

            dinv_all = pers.tile([128, NWIN], F32)
            deg_all = pers.tile([128, NWIN], F32)
            adw_all = pers.tile([128, NWIN, 10], BF16)
            xt_sb = pers.tile([MY_G, 128], F32)
            nc.gpsimd.memset(xt_sb[:], 0.0)

            # shared edge-phase machinery -----------------------------------
            def edge_phase(table, gat, x_epilogue, agg_bufs):
                with (
                    tc.tile_pool(name="msgp", bufs=3) as msgp,
                    tc.tile_pool(name="smallp", bufs=2) as smallp,
                    tc.tile_pool(name="maskp", bufs=6) as maskp,
                    tc.tile_pool(name="epip", bufs=2) as epip,
                    tc.tile_pool(name="psA", bufs=agg_bufs, space="PSUM") as psA,
                    tc.tile_pool(name="psS", bufs=2, space="PSUM") as psS,
                ):
                    aggp = None
                    for c in range(nchunk):
                        isl = slice(c * 128, (c + 1) * 128)
                        msg = msgp.tile([128, 16, RBF], BF16, tag="msg")
                        nc.gpsimd.dma_gather(
                            msg[:],
                            table.ap()[:, 0:RBF],
                            src_t[:, isl],
                            num_idxs=2048,
                            num_idxs_reg=2048,
                            elem_size=RBF,
                            elem_step=RBF,
                            single_packet=False,
                        )
                        maskall = maskp.tile([128, 16, 128], BF16, tag="maskall")
                        if gat:
                            nc.gpsimd.memset(msg[:, :, 830:831], 1.0)
                            sall = smallp.tile([128, 16, 10], F32, tag="sall")
                        # pass 1 (per 4-tile group): masks, a_d expansion,
                        # leaky+exp, message scaling
                        for q4 in range(4):
                            jsl = slice(q4 * 4, q4 * 4 + 4)
                            for i in range(4):
                                g = c * 16 + q4 * 4 + i
                                nc.vector.tensor_tensor(
                                    maskall[:, q4 * 4 + i, :],
                                    dstw_t[:, g : g + 1].to_broadcast([128, 128]),
                                    iota_f[:],
                                    op=OP.is_equal,
                                )
                            if not gat:
                                continue
                            trT = psS.tile([128, 512], BF16, space="PSUM", tag="trT")
                            for i in range(4):
                                nc.tensor.transpose(
                                    trT[:, i * 128 : (i + 1) * 128],
                                    maskall[:, q4 * 4 + i, :],
                                    ident_bf[:],
                                )
                            maskT = maskp.tile([128, 4, 128], BF16, tag="maskT")
                            nc.scalar.copy(maskT[:], trT[:])
                            adx = psS.tile([128, 512], F32, space="PSUM", tag="adx")
                            for i in range(4):
                                nc.tensor.matmul(
                                    adx[:, i * 10 : i * 10 + 10],
                                    lhsT=maskT[:, i, :],
                                    rhs=adw_all[:, (c * 16 + q4 * 4 + i) // tpw, :],
                                    start=True,
                                    stop=True,
                                )
                            nc.vector.tensor_tensor(
                                sall[:, jsl, :],
                                msg[:, jsl, 780:800].bitcast(F32),
                                adx[:, 0:40].rearrange("p (a b) -> p a b", b=10),
                                op=OP.add,
                            )
                            s2 = smallp.tile([128, 4, 10], F32, tag="s2")
                            nc.vector.tensor_scalar_mul(s2[:], sall[:, jsl, :], 0.2)
                            nc.vector.tensor_tensor(
                                sall[:, jsl, :], sall[:, jsl, :], s2[:], op=OP.max
                            )
                            nc.scalar.activation(
                                msg[:, jsl, 820:830], sall[:, jsl, :], AF.Exp
                            )
                            nc.vector.tensor_tensor(
                                msg[:, jsl, 0:HID].rearrange(
                                    "p c (h f) -> p c h f", h=H
                                ),
                                msg[:, jsl, 0:HID].rearrange(
                                    "p c (h f) -> p c h f", h=H
                                ),
                                msg[:, jsl, 820:830, None].to_broadcast([128, 4, H, F]),
                                op=OP.mult,
                            )
                        # pass 2: scatter matmuls
                        for j in range(16):
                            g = c * 16 + j
                            w = g // tpw
                            first = g % tpw == 0
                            last = g % tpw == tpw - 1
                            if first:
                                aggp = psA.tile(
                                    [128, 1024], F32, space="PSUM", tag="aggp"
                                )
                            n_hi = 831 if gat else HID
                            for n0, nn in ((0, 512), (512, n_hi - 512)):
                                nc.tensor.matmul(
                                    aggp[:, n0 : n0 + nn],
                                    lhsT=maskall[:, j, :],
                                    rhs=msg[:, j, n0 : n0 + nn],
                                    start=first,
                                    stop=last,
                                )
                            if last:
                                x_epilogue(w, aggp, epip)

                    return

            def gat_epilogue(w, aggp, epip):
                rec = epip.tile([128, 12], F32, tag="rec")
                nc.vector.tensor_scalar_add(rec[:, 0:11], aggp[:, 820:831], 1e-20)
                nc.vector.tensor_copy(deg_all[:, w : w + 1], rec[:, 10:11])
                rcp = epip.tile([128, 10], F32, tag="rcp")
                nc.vector.reciprocal(rcp[:], rec[:, 0:10])
                x1w = epip.tile([128, HID], F32, tag="x1w")
                nc.vector.tensor_tensor(
                    x1w[:].rearrange("p (h f) -> p h f", h=H),
                    aggp[:, 0:HID].rearrange("p (h f) -> p h f", h=H),
                    rcp[:, :, None].to_broadcast([128, H, F]),
                    op=OP.mult,
                )
                nc.vector.tensor_tensor(x1w[:], x1w[:], gatb_bc[:], op=OP.add)
                nc.vector.tensor_scalar_max(x1w[:], x1w[:], 0.0)
                nc.sync.dma_start(x1d.ap()[w * 128 : (w + 1) * 128, :], x1w[:])

            # ---- phase 1: own h rows; AllGather table ----
            if PH >= 1:
              with (
                tc.tile_pool(name="p1", bufs=1) as p1,
                tc.tile_pool(name="p1h", bufs=3) as p1h,
                tc.tile_pool(name="ps1", bufs=1, space="PSUM") as ps1,
              ):
                xT_sb = p1.tile([78, NPAD], F32)
                nc.sync.dma_start(xT_sb[:], xTc[:, :])
                wp_sb = p1.tile([78, 1024], F32)
                nc.sync.dma_start(wp_sb[:], wpack0[:, :])
                gwt_sb = p1.tile([128, 7, 78], F32)
                nc.sync.dma_start(
                    gwt_sb[:], gat_WT.ap().rearrange("(c p) f -> p c f", p=128)
                )
                was_sb = p1.tile([128, 7, 20], F32)
                nc.sync.dma_start(
                    was_sb[:], wasbd.ap().rearrange("(c p) f -> p c f", p=128)
                )
                wcps = ps1.tile([78, 512], F32, space="PSUM", tag="wcps")
                for kc in range(7):
                    nc.tensor.matmul(
                        wcps[:, 0:20],
                        lhsT=gwt_sb[:, kc, :],
                        rhs=was_sb[:, kc, :],
                        start=(kc == 0),
                        stop=(kc == 6),
                    )
                nc.any.tensor_copy(wp_sb[:, HID : HID + 20], wcps[:, 0:20])

                for t in range(NWIN):
                    hp = ps1.tile([128, 1024], F32, space="PSUM", tag="hp")
                    for n0 in (0, 512):
                        nc.tensor.matmul(
                            hp[:, n0 : n0 + 512],
                            lhsT=xT_sb[:, t * 128 : (t + 1) * 128],
                            rhs=wp_sb[:, n0 : n0 + 512],
                            start=True,
                            stop=True,
                        )
                    hrow = p1h.tile([128, RBF], BF16, tag="hrow")
                    nc.vector.tensor_copy(hrow[:, 0:HID], hp[:, 0:HID])
                    nc.vector.tensor_copy(
                        hrow[:, 780:800].bitcast(F32), hp[:, 780:790]
                    )
                    nc.gpsimd.memset(hrow[:, 800:RBF], 0.0)
                    nc.vector.tensor_copy(adw_all[:, t, :], hp[:, 790:800])
                    nc.sync.dma_start(
                        hin.ap()[t * 128 : (t + 1) * 128, :], hrow[:]
                    )
                nc.gpsimd.collective_compute(
                    "AllGather",
                    OP.bypass,
                    replica_groups=RG,
                    ins=[hin.ap().opt()],
                    outs=[htabG.ap().opt()],
                )

            # ---- phase 2: GAT ----
            if PH >= 2:
                edge_phase(htabG, True, gat_epilogue, agg_bufs=2)
                nc.scalar.activation(dinv_all[:], deg_all[:], AF.Sqrt)
                nc.vector.tensor_scalar_add(dinv_all[:], dinv_all[:], 1e-20)
                nc.vector.reciprocal(dinv_all[:], dinv_all[:])

            # ================== protein scope (overlaps phase 3 + AllGathers) =========
            with (
                tc.tile_pool(name="pp", bufs=1) as pp,
                tc.tile_pool(name="ppg", bufs=2) as ppg,
                tc.tile_pool(name="ppw", bufs=2) as ppw,
                tc.tile_pool(name="psC", bufs=2, space="PSUM") as psC,
                tc.tile_pool(name="psTr", bufs=1, space="PSUM") as psTr,
                tc.tile_pool(name="psX", bufs=1, space="PSUM") as psX,
            ):
                # ---- phase 3: h2s + AllGather ----
                if PH >= 3:
                  with (
                    tc.tile_pool(name="p3", bufs=2) as p3,
                    tc.tile_pool(name="p3w", bufs=1) as p3w,
                    tc.tile_pool(name="psT3", bufs=1, space="PSUM") as psT3,
                    tc.tile_pool(name="psH3", bufs=1, space="PSUM") as psH3,
                  ):
                    gcnw_sb = p3w.tile([128, 7, HID], F32)
                    nc.sync.dma_start(
                        gcnw_sb[:], gcn_Wp.ap().rearrange("(c p) f -> p c f", p=128)
                    )
                    for t in range(NWIN):
                        x1t = p3.tile([128, HID], F32, tag="x1t")
                        nc.sync.dma_start(x1t[:], x1d.ap()[t * 128 : (t + 1) * 128, :])
                        x1T = p3.tile([128, 7, 128], F32, tag="x1T")
                        for kc in range(7):
                            sz = 128 if kc < 6 else 12
                            trp = psT3.tile([128, 128], F32, space="PSUM", tag="trp")
                            nc.tensor.transpose(
                                trp[0:sz, :], x1t[:, kc * 128 : kc * 128 + sz], ident[:]
                            )
                            nc.any.tensor_copy(x1T[0:sz, kc, :], trp[0:sz, :])
                        h2ps = psH3.tile([128, 1024], F32, space="PSUM", tag="h2ps")
                        for n0, nn in ((0, 512), (512, 268)):
                            for kc in range(7):
                                sz = 128 if kc < 6 else 12
                                nc.tensor.matmul(
                                    h2ps[:, n0 : n0 + nn],
                                    lhsT=x1T[0:sz, kc, :],
                                    rhs=gcnw_sb[0:sz, kc, n0 : n0 + nn],
                                    start=(kc == 0),
                                    stop=(kc == 6),
                                )
                        h2s = p3.tile([128, RBF], BF16, tag="h2s")
                        nc.vector.tensor_tensor(
                            h2s[:, 0:HID],
                            h2ps[:, 0:HID],
                            dinv_all[:, t : t + 1].to_broadcast([128, HID]),
                            op=OP.mult,
                        )
                        nc.gpsimd.memset(h2s[:, HID:RBF], 0.0)
                        nc.sync.dma_start(agin.ap()[t * 128 : (t + 1) * 128, :], h2s[:])
                    nc.gpsimd.collective_compute(
                        "AllGather",
                        OP.bypass,
                        replica_groups=RG,
                        ins=[agin.ap().opt()],
                        outs=[htab2G.ap().opt()],
                    )

                # ---- protein branch (gap-filler; no deps on graph phases) ----
                if PH >= 4:
                    cwt_sb = pp.tile([128, KS, NF], BF16)
                    nc.sync.dma_start(
                        cwt_sb[:], cwt_bf.ap().rearrange("k p o -> p k o")
                    )
                    cb_sb = pp.tile([NF, 1], F32)
                    nc.sync.dma_start(cb_sb[:], cb.ap()[:, :])
                    cT_all = pp.tile([128, 8, NF, BPC], F32)

                    for grp in range(5):
                        gt = ppg.tile([128, 40, 128], BF16, tag="embg")
                        nc.gpsimd.dma_gather(
                            gt[:],
                            emb_bf.ap()[:, :],
                            tgt_t[:, grp * 320 : (grp + 1) * 320],
                            num_idxs=5120,
                            num_idxs_reg=5120,
                            elem_size=128,
                            elem_step=128,
                            single_packet=False,
                        )
                        et5 = ppg.tile([128, 5120], BF16, tag="et5")
                        for i in range(40):
                            trp = psTr.tile([128, 128], BF16, space="PSUM", tag="trp2")
                            nc.tensor.transpose(trp[:], gt[:, i, :], ident_bf[:])
                            nc.any.tensor_copy(et5[:, i * 128 : (i + 1) * 128], trp[:])
                        for bl in range(5):
                            b = grp * 5 + bl
                            boff = bl * 1000
                            csb = pp.tile([NF, 1024], F32, tag="csb")
                            for p0 in (0, 512):
                                cps = psC.tile([NF, 512], F32, space="PSUM", tag="cps")
                                for k in range(KS):
                                    nc.tensor.matmul(
                                        cps[:, 0:512],
                                        lhsT=cwt_sb[:, k, :],
                                        rhs=et5[:, boff + k + p0 : boff + k + p0 + 512],
                                        start=(k == 0),
                                        stop=(k == KS - 1),
                                    )
                                nc.scalar.activation(
                                    csb[:, p0 : p0 + 512], cps[:, 0:512],
                                    AF.Identity, bias=cb_sb[:, 0:1],
                                )
                            for pc in range(8):
                                trc = psTr.tile(
                                    [128, 128], F32, space="PSUM", tag="trc"
                                )
                                nc.tensor.transpose(
                                    trc[:, 0:NF],
                                    csb[:, pc * 128 : (pc + 1) * 128],
                                    ident[0:NF, 0:NF],
                                )
                                nc.any.tensor_copy(cT_all[:, pc, :, b], trc[:, 0:NF])

                    xtps = psX.tile([MY_G, 128], F32, space="PSUM", tag="xtps")
                    for sc in range(16):
                        wpt = ppw.tile([128, 16, 128], F32, tag="wpt")
                        nc.sync.dma_start(
                            wpt[:],
                            fxt_Wp.ap()[sc * 2048 : (sc + 1) * 2048, :].rearrange(
                                "(c p) j -> p c j", p=128
                            ),
                        )
                        for sub in range(16):
                            q = sc * 16 + sub
                            o, t8 = q // 8, q % 8
                            nc.tensor.matmul(
                                xtps[0:BPC, :],
                                lhsT=cT_all[:, t8, o, :],
                                rhs=wpt[:, sub, :],
                                start=(q == 0),
                                stop=(q == 255),
                            )
                    nc.vector.tensor_tensor(
                        xt_sb[0:BPC, :], xtps[0:BPC, :], fxtb_bc[0:BPC, :], op=OP.add
                    )

            # ================== phase 4: GCN + pooling; phase 5: head ========
            if PH >= 5:
              with tc.tile_pool(name="psP", bufs=1, space="PSUM") as psP:
                poolps = psP.tile([GSLOT, 1024], F32, space="PSUM", tag="poolps")

                def gcn_epilogue(w, aggp, epip):
                    x2w = epip.tile([128, HID], F32, tag="x2w")
                    nc.vector.tensor_tensor(
                        x2w[:],
                        aggp[:, 0:HID],
                        dinv_all[:, w : w + 1].to_broadcast([128, HID]),
                        op=OP.mult,
                    )
                    nc.vector.tensor_tensor(x2w[:], x2w[:], gcnb_bc[:], op=OP.add)
                    nc.vector.tensor_scalar_max(x2w[:], x2w[:], 0.0)
                    ph = epip.tile([128, GSLOT], F32, tag="poolhot")
                    nc.vector.tensor_tensor(
                        ph[:],
                        batchw_t[:, w : w + 1].to_broadcast([128, GSLOT]),
                        iota_f[:, 0:GSLOT],
                        op=OP.is_equal,
                    )
                    for n0, nn in ((0, 512), (512, 268)):
                        nc.tensor.matmul(
                            poolps[:, n0 : n0 + nn],
                            lhsT=ph[:],
                            rhs=x2w[:, n0 : n0 + nn],
                            start=(w == 0),
                            stop=(w == NWIN - 1),
                        )

                edge_phase(htab2G, False, gcn_epilogue, agg_bufs=2)
                poolsb = pers.tile([GSLOT, HID], F32)
                nc.any.tensor_copy(poolsb[:], poolps[:, 0:HID])

              with (
                    tc.tile_pool(name="p5", bufs=1) as p5,
                    tc.tile_pool(name="p5w", bufs=2) as p5w,
                    tc.tile_pool(name="ps5", bufs=2, space="PSUM") as ps5,
                    tc.tile_pool(name="ps5t", bufs=2, space="PSUM") as ps5t,
                ):
                    nc.sync.dma_start(poolin.ap()[:, :], poolsb[:])
                    nc.gpsimd.collective_compute(
                        "AllGather",
                        OP.bypass,
                        replica_groups=RG,
                        ins=[poolin.ap().opt()],
                        outs=[poolall.ap().opt()],
                    )
                    Cc_sb = p5.tile([128, 4, MY_G], F32)
                    nc.sync.dma_start(
                        Cc_sb[:], Cc.ap().rearrange("(c p) g -> p c g", p=128)
                    )
                    pall = p5.tile([128, 4, HID], F32)
                    nc.sync.dma_start(
                        pall[:], poolall.ap().rearrange("(c p) f -> p c f", p=128)
                    )
                    xgps = ps5.tile([MY_G, 1024], F32, space="PSUM", tag="mlp_ps")
                    for kc in range(4):
                        for n0, nn in ((0, 512), (512, 268)):
                            nc.tensor.matmul(
                                xgps[:, n0 : n0 + nn],
                                lhsT=Cc_sb[:, kc, :],
                                rhs=pall[:, kc, n0 : n0 + nn],
                                start=(kc == 0),
                                stop=(kc == 3),
                            )
                    xg = p5.tile([MY_G, HID], F32, tag="act0")
                    nc.any.tensor_copy(xg[:], xgps[:, 0:HID])

                    def dense(x_sb, k_real, w_dram, w_rows, n_out, b_bc, relu, tag):
                        nkc = (k_real + 127) // 128
                        xT_t = p5.tile([128, nkc, MY_G], F32, tag="xT5")
                        for kc in range(nkc):
                            sz = min(128, k_real - kc * 128)
                            trp = ps5t.tile([128, MY_G], F32, space="PSUM", tag="tr5")
                            nc.tensor.transpose(
                                trp[0:sz, :],
                                x_sb[:, kc * 128 : kc * 128 + sz],
                                ident[0:MY_G, 0:MY_G],
                            )
                            nc.any.tensor_copy(xT_t[0:sz, kc, :], trp[0:sz, :])
                        w_sb = p5w.tile([128, w_rows // 128, n_out], F32, tag="w5")
                        nc.sync.dma_start(
                            w_sb[:], w_dram.ap().rearrange("(c p) f -> p c f", p=128)
                        )
                        yps = ps5.tile([MY_G, 1536], F32, space="PSUM", tag="mlp_ps")
                        for n0 in range(0, n_out, 512):
                            nn = min(512, n_out - n0)
                            for kc in range(nkc):
                                sz = min(128, k_real - kc * 128)
                                nc.tensor.matmul(
                                    yps[:, n0 : n0 + nn],
                                    lhsT=xT_t[0:sz, kc, :],
                                    rhs=w_sb[0:sz, kc, n0 : n0 + nn],
                                    start=(kc == 0),
                                    stop=(kc == nkc - 1),
                                )
                        y = p5.tile([MY_G, n_out], F32, tag="y5")
                        nc.vector.tensor_tensor(
                            y[:], yps[:, 0:n_out], b_bc[0:MY_G, 0:n_out], op=OP.add
                        )
                        if relu:
                            nc.vector.tensor_scalar_max(y[:], y[:], 0.0)
                        return y

                    y1 = dense(xg, HID, fcg1_Wp, 896, 1500, fcg1b_bc, True, "fcg1")
                    xgo = dense(y1, 1500, fcg2_Wp, 1536, 128, fcg2b_bc, False, "fcg2")
                    xc = p5.tile([MY_G, 256], F32, tag="xc")
                    nc.any.tensor_copy(xc[:, 0:128], xgo[:])
                    nc.any.tensor_copy(xc[:, 128:256], xt_sb[:])
                    a1 = dense(xc, 256, f1_W, 256, 1024, f1b_bc, True, "f1")
                    a2 = dense(a1, 1024, f2_W, 1024, 512, f2b_bc, True, "f2")
                    a3 = dense(a2, 512, f3_W, 512, 256, f3b_bc, True, "f3")
                    a4 = dense(a3, 256, f4_W, 256, 128, f4b_bc, True, "f4")
                    yo = dense(a4, 128, o_W, 128, 1, ob_bc, False, "o")
                    nc.sync.dma_start(out_d.ap()[:, :], yo[:])
                    if KDEBUG:
                        dbg = p5.tile([128, 2, RBF], BF16, tag="dbg")
                        nc.sync.dma_start(
                            dbg[:], htabG.ap()[0:256, :].rearrange("(c p) f -> p c f", p=128)
                        )
                        nc.sync.dma_start(
                            out_h.ap().rearrange("(c p) f -> p c f", p=128), dbg[:]
                        )
                        dbg2 = p5.tile([128, NWIN, HID], F32, tag="dbg2")
                        nc.sync.dma_start(
                            dbg2[:], x1d.ap().rearrange("(c p) f -> p c f", p=128)
                        )
                        nc.sync.dma_start(
                            out_x1.ap().rearrange("(c p) f -> p c f", p=128), dbg2[:]
                        )
                        nc.sync.dma_start(
                            out_adw.ap(), adw_all[:].rearrange("p a b -> p (a b)")
                        )
                        nc.sync.dma_start(out_xt.ap()[:, :], xt_sb[:])

    nc.compile()
    _CACHE[key] = nc
    return nc


# ---------------------------------------------------------------- entry point


def _ensure_ntff_hook():
    """Install antenv.axon_hooks + register the ctypes NTFF hook if the image
    lacks them (profiling only; failures are non-fatal)."""
    import types

    try:
        import antenv.axon_hooks  # noqa: F401

        if antenv.axon_hooks.get_axon_ntff_profile_hook() is not None:
            return
    except ImportError:
        import antenv

        mod = types.ModuleType("antenv.axon_hooks")
        mod._hook = None

        def set_axon_ntff_profile_hook(h, _m=mod):
            _m._hook = h

        def get_axon_ntff_profile_hook(_m=mod):
            return _m._hook

        mod.set_axon_ntff_profile_hook = set_axon_ntff_profile_hook
        mod.get_axon_ntff_profile_hook = get_axon_ntff_profile_hook
        sys.modules["antenv.axon_hooks"] = mod
        antenv.axon_hooks = mod
    try:
        from antenv.axon_hooks import set_axon_ntff_profile_hook as _set
        from trn_agent_boot.trn_boot import _ntff_profile_via_ctypes

        hook = _ntff_profile_via_ctypes("/opt/axon/libaxon_pjrt.so")
        if hook is not None:
            _set(hook)
    except Exception:
        pass


def _enable_ldw_opt():
    """Turn on walrus's LDWEIGHTS dedup pass (consecutive matmuls sharing a
    stationary operand skip the reload). Opt-in via KLDWOPT=1."""
    import concourse.bass_utils as bu

    if getattr(bu, "_ldw_patched", False):
        return
    orig = bu.run_command

    def patched(argv, **kw):
        argv = [
            "--enable-ldw-opt=true" if a == "--enable-ldw-opt=false" else a
            for a in argv
        ]
        return orig(argv, **kw)

    bu.run_command = patched
    bu._ldw_patched = True


def kernel(**inputs) -> np.ndarray:
    if bool(int(os.environ.get("KLDWOPT", "0"))):
        _enable_ldw_opt()
    per_core, meta = host_prep(inputs)
    nc = build_bass(meta)
    in_maps = [{k: np.ascontiguousarray(v) for k, v in d.items()} for d in per_core]
    trace = bool(int(os.environ.get("KERNEL_TRACE", "0")))
    if trace:
        _ensure_ntff_hook()
    res = run_bass_kernel_spmd(nc, in_maps, core_ids=list(range(NCORES)), trace=trace)
    if trace and res.exec_time_ns is not None:
        print(f"HW exec time: {res.exec_time_ns} ns")
        kernel.last_exec_ns = res.exec_time_ns
    out = np.concatenate([res.results[c]["out"][:BPC] for c in range(NCORES)], 0)
    return out.astype(np.float32)



# revision 3
# speedup vs baseline: 1.1038x; 1.1038x over previous
"""Trainium2 Bass kernel for nn_EnhancedGATGCN (GAT -> GCN -> pool -> MLP, + protein conv branch).

Self-contained: host-side sharding prep + 8-core SPMD Bass/Tile device program.

Sharding strategy:
  - Edges (incl. self loops) sorted by dst, partitioned into 8 contiguous dst
    ranges of 2500 nodes; scatter-add is core-local via one-hot matmuls on the
    TensorEngine with PSUM accumulation per 128-dst window.
  - Node feature tables (h for GAT, dinv-scaled h2 for GCN) are computed
    node-sharded, AllGathered as bf16 tables in HBM, and per-edge messages are
    fetched with SWDGE dma_gather (f32 attention logits bit-packed into the
    bf16 rows). GAT softmax denominator rides the scatter matmul as extra rhs
    columns; a_d[dst] is expanded per edge with a transposed-mask matmul; GCN
    sym-norm is factored into per-node dinv scales.
  - Dense tail (conv/MLPs) is data-parallel over the 200-graph batch (25/core)
    and scheduled to overlap the AllGathers / gather-bound stretches.
"""
import os
import sys

import numpy as np

sys.path.insert(0, "/opt/trn_rl_repo")

import ml_dtypes

import concourse.bacc as bacc
import concourse.bass as bass
import concourse.mybir as mybir
import concourse.tile as tile
from concourse.bass_utils import run_bass_kernel_spmd
from concourse.masks import make_identity

F32 = mybir.dt.float32
BF16 = mybir.dt.bfloat16
I16 = mybir.dt.int16
I32 = mybir.dt.int32
AF = mybir.ActivationFunctionType
OP = mybir.AluOpType
BF = ml_dtypes.bfloat16

N, E, B, H, F = 20000, 400000, 200, 10, 78
HID = H * F  # 780
SEQ, VOC, EMB, NF, KS = 1000, 26, 128, 32, 8
CONV_OUT = SEQ - KS + 1  # 993

NCORES = 8
NPC = N // NCORES  # 2500
NPAD = 2560
NWIN = NPAD // 128  # 20
RBF = 896  # bf16 cols per table row; bytes = 1792 (%256==0)
# row: [0:780 h bf16 | 780:800 a_s 10xf32 | 800:896 zeros (820:830 exd scratch on msg tiles)]
GSLOT = 64
MY_G = 32
BPC = B // NCORES  # 25
TOK = BPC * SEQ
TOKPAD = 25600  # 5 groups x 5120 (each: 5 graphs x 1000 tok + 120 pad)
KPAD = 32 * 1024


# ---------------------------------------------------------------- host prep


def _wrap16(idx, epc):
    a = np.zeros((128, epc // 16), np.int16)
    w = idx.reshape(epc // 16, 16).T
    a[:, :] = np.tile(w, (8, 1))
    return a


def host_prep(inputs):
    x = np.asarray(inputs["x"], np.float32)
    edge_index = np.asarray(inputs["edge_index"], np.int64)
    batch = np.asarray(inputs["batch"], np.int64)
    target = np.asarray(inputs["target"], np.int64)

    loops = np.arange(N, dtype=np.int64)
    src = np.concatenate([edge_index[0], loops])
    dst = np.concatenate([edge_index[1], loops])
    order = np.argsort(dst, kind="stable")
    src, dst = src[order], dst[order]

    core_of = dst // NPC
    dst_local = dst - core_of * NPC
    win = dst_local // 128
    maxw = 0
    per_core_edges = []
    for c in range(NCORES):
        m = core_of == c
        s_c, dl_c, w_c = src[m], dst_local[m], win[m]
        per_core_edges.append((s_c, dl_c, w_c))
        maxw = max(maxw, int(np.bincount(w_c, minlength=NWIN).max()))
    tpw = -(-maxw // 128)
    tpw = -(-tpw // 4) * 4
    ntile = NWIN * tpw
    epc = ntile * 128
    nchunk = ntile // 16

    def remap(n):
        return (n // NPC) * NPAD + (n % NPC)

    cores = []
    for c in range(NCORES):
        s_c, dl_c, w_c = per_core_edges[c]
        es = np.zeros(epc, np.int64)
        ew = np.full(epc, -1000.0, np.float32)
        for w in range(NWIN):
            m = w_c == w
            k = int(m.sum())
            o = w * tpw * 128
            es[o : o + k] = s_c[m]
            ew[o : o + k] = (dl_c[m] - w * 128).astype(np.float32)
        cores.append(dict(es=remap(es), ew=ew))

    gat_W = np.asarray(inputs["gat_W"], np.float32)
    wpack0 = np.zeros((78, 1024), np.float32)
    wpack0[:, :HID] = gat_W
    gat_WT_pad = np.zeros((896, 78), np.float32)
    gat_WT_pad[:HID] = gat_W.T
    wasbd = np.zeros((896, 20), np.float32)
    a_src = np.asarray(inputs["gat_a_src"], np.float32)
    a_dst = np.asarray(inputs["gat_a_dst"], np.float32)
    for h in range(H):
        wasbd[h * F : (h + 1) * F, h] = a_src[h]
        wasbd[h * F : (h + 1) * F, 10 + h] = a_dst[h]

    gcn_W_pad = np.zeros((896, HID), np.float32)
    gcn_W_pad[:HID] = np.asarray(inputs["gcn_W"], np.float32)
    fcg1_W_pad = np.zeros((896, 1500), np.float32)
    fcg1_W_pad[:HID] = np.asarray(inputs["fcg1_W"], np.float32)
    fcg2_W_pad = np.zeros((1536, 128), np.float32)
    fcg2_W_pad[:1500] = np.asarray(inputs["fcg2_W"], np.float32)

    fxt_W = np.asarray(inputs["fxt_W"], np.float32)
    fxt_Wp = np.zeros((KPAD, 128), np.float32)
    fxt_Wp.reshape(NF, 1024, 128)[:, :CONV_OUT] = fxt_W.reshape(NF, CONV_OUT, 128)

    cW = np.asarray(inputs["cW"], np.float32)
    cwt = np.ascontiguousarray(cW.transpose(2, 1, 0))  # [8, 128, 32]

    gbase = np.array([batch[c * NPC] for c in range(NCORES)], np.int64)
    span = np.array(
        [batch[min(c * NPC + NPC, N) - 1] - gbase[c] + 1 for c in range(NCORES)]
    )
    assert span.max() <= GSLOT, span.max()
    Cc_all = []
    for c in range(NCORES):
        Cmat = np.zeros((NCORES * GSLOT, MY_G), np.float32)
        for r in range(NCORES):
            for slot in range(GSLOT):
                g = gbase[r] + slot
                col = g - c * BPC
                if 0 <= col < BPC and g < B:
                    Cmat[r * GSLOT + slot, col] = 1.0
        Cc_all.append(Cmat)

    meta = dict(tpw=tpw, ntile=ntile, epc=epc, nchunk=nchunk)

    per_core = []
    for c in range(NCORES):
        ed_ = cores[c]
        bw = np.full(NPAD, -1000.0, np.float32)
        bw[:NPC] = (batch[c * NPC : (c + 1) * NPC] - gbase[c]).astype(np.float32)
        batchw = bw.reshape(NWIN, 128).T.copy()

        tgt = np.zeros(TOKPAD, np.int64)
        tg = target[c * BPC : (c + 1) * BPC].reshape(5, 5 * SEQ)
        tgt.reshape(5, 5120)[:, : 5 * SEQ] = tg

        dstw = ed_["ew"].reshape(ntile, 128).T.copy()

        xTc = np.zeros((78, NPAD), np.float32)
        xTc[:, :NPC] = x[c * NPC : (c + 1) * NPC].T

        d = {
            "xTc": xTc,
            "wpack0": wpack0,
            "gat_WT": gat_WT_pad,
            "wasbd": wasbd,
            "src16": _wrap16(ed_["es"], epc),
            "tgt16": _wrap16(tgt, TOKPAD),
            "dstw": dstw,
            "batchw": batchw,
            "Cc": Cc_all[c],
            "gat_b": np.asarray(inputs["gat_b"], np.float32).reshape(1, HID),
            "gcn_Wp": gcn_W_pad,
            "gcn_b": np.asarray(inputs["gcn_b"], np.float32).reshape(1, HID),
            "fcg1_Wp": fcg1_W_pad,
            "fcg1_b": np.asarray(inputs["fcg1_b"], np.float32).reshape(1, 1500),
            "fcg2_Wp": fcg2_W_pad,
            "fcg2_b": np.asarray(inputs["fcg2_b"], np.float32).reshape(1, 128),
            "emb_bf": np.asarray(inputs["emb"], np.float32).astype(BF),
            "cwt_bf": cwt.astype(BF),
            "cb": np.asarray(inputs["cb"], np.float32).reshape(NF, 1),
            "fxt_Wp": fxt_Wp,
            "fxt_b": np.asarray(inputs["fxt_b"], np.float32).reshape(1, 128),
            "f1_W": np.asarray(inputs["f1_W"], np.float32),
            "f1_b": np.asarray(inputs["f1_b"], np.float32).reshape(1, 1024),
            "f2_W": np.asarray(inputs["f2_W"], np.float32),
            "f2_b": np.asarray(inputs["f2_b"], np.float32).reshape(1, 512),
            "f3_W": np.asarray(inputs["f3_W"], np.float32),
            "f3_b": np.asarray(inputs["f3_b"], np.float32).reshape(1, 256),
            "f4_W": np.asarray(inputs["f4_W"], np.float32),
            "f4_b": np.asarray(inputs["f4_b"], np.float32).reshape(1, 128),
            "o_W": np.asarray(inputs["o_W"], np.float32),
            "o_b": np.asarray(inputs["o_b"], np.float32).reshape(1, 1),
        }
        per_core.append(d)
    return per_core, meta


# ---------------------------------------------------------------- device build

_CACHE = {}


def build_bass(meta):
    PH = int(os.environ.get("KPHASE", "9"))
    key = (meta["tpw"], PH)
    if key in _CACHE:
        return _CACHE[key]

    tpw, ntile, epc, nchunk = meta["tpw"], meta["ntile"], meta["epc"], meta["nchunk"]

    nc = bacc.Bacc(
        "TRN2",
        target_bir_lowering=False,
        debug=False,
        num_devices=NCORES,
        num_swdge_queues=2,
    )

    def inp(name, shape, dt=F32):
        return nc.dram_tensor(name, list(shape), dt, kind="ExternalInput")

    xTc = inp("xTc", (78, NPAD))
    wpack0 = inp("wpack0", (78, 1024))
    gat_WT = inp("gat_WT", (896, 78))
    wasbd = inp("wasbd", (896, 20))
    src16 = inp("src16", (128, epc // 16), I16)
    tgt16 = inp("tgt16", (128, TOKPAD // 16), I16)
    dstw = inp("dstw", (128, ntile))
    batchw = inp("batchw", (128, NWIN))
    Cc = inp("Cc", (NCORES * GSLOT, MY_G))
    gat_b = inp("gat_b", (1, HID))
    gcn_Wp = inp("gcn_Wp", (896, HID))
    gcn_b = inp("gcn_b", (1, HID))
    fcg1_Wp = inp("fcg1_Wp", (896, 1500))
    fcg1_b = inp("fcg1_b", (1, 1500))
    fcg2_Wp = inp("fcg2_Wp", (1536, 128))
    fcg2_b = inp("fcg2_b", (1, 128))
    emb_bf = inp("emb_bf", (VOC, EMB), BF16)
    cwt_bf = inp("cwt_bf", (KS, EMB, NF), BF16)
    cb = inp("cb", (NF, 1))
    fxt_Wp = inp("fxt_Wp", (KPAD, 128))
    fxt_b = inp("fxt_b", (1, 128))
    f1_W = inp("f1_W", (256, 1024))
    f1_b = inp("f1_b", (1, 1024))
    f2_W = inp("f2_W", (1024, 512))
    f2_b = inp("f2_b", (1, 512))
    f3_W = inp("f3_W", (512, 256))
    f3_b = inp("f3_b", (1, 256))
    f4_W = inp("f4_W", (256, 128))
    f4_b = inp("f4_b", (1, 128))
    o_W = inp("o_W", (128, 1))
    o_b = inp("o_b", (1, 1))
    out_d = nc.dram_tensor("out", [MY_G, 1], F32, kind="ExternalOutput")
    KDEBUG = bool(int(os.environ.get("KDEBUG", "0")))
    if KDEBUG:
        out_x1 = nc.dram_tensor("out_x1", [NPAD, HID], F32, kind="ExternalOutput")
        out_h = nc.dram_tensor("out_h", [256, RBF], BF16, kind="ExternalOutput")
        out_adw = nc.dram_tensor("out_adw", [128, NWIN * 10], BF16, kind="ExternalOutput")
        out_xt = nc.dram_tensor("out_xt", [MY_G, 128], F32, kind="ExternalOutput")

    hin = nc.dram_tensor("hin", [NPAD, RBF], BF16)
    htabG = nc.dram_tensor("htabG", [NCORES * NPAD, RBF], BF16, addr_space="Shared")
    x1d = nc.dram_tensor("x1d", [NPAD, HID], F32)
    agin = nc.dram_tensor("agin", [NPAD, RBF], BF16)
    htab2G = nc.dram_tensor("htab2G", [NCORES * NPAD, RBF], BF16, addr_space="Shared")
    poolin = nc.dram_tensor("poolin", [GSLOT, HID], F32)
    poolall = nc.dram_tensor("poolall", [NCORES * GSLOT, HID], F32, addr_space="Shared")

    RG = [list(range(NCORES))]

    with tile.TileContext(nc) as tc:
        import contextlib

        ctx = contextlib.ExitStack()
        with ctx:
            pers = ctx.enter_context(tc.tile_pool(name="pers", bufs=1))

            # consts
            iota_i = pers.tile([128, 128], I32)
            nc.gpsimd.iota(iota_i[:], pattern=[[1, 128]], base=0, channel_multiplier=0)
            iota_f = pers.tile([128, 128], F32)
            nc.vector.tensor_copy(iota_f[:], iota_i[:])
            ident = pers.tile([128, 128], F32)
            make_identity(nc, ident[:])
            ident_bf = pers.tile([128, 128], BF16)
            nc.vector.tensor_copy(ident_bf[:], ident[:])
            ones1 = pers.tile([1, 128], F32)
            nc.gpsimd.memset(ones1[:], 1.0)

            bias_tiles = {}
            with tc.tile_pool(name="psB", bufs=1, space="PSUM") as psB:

                def bcast_bias(dram, width, name):
                    t = pers.tile([128, width], F32, tag=f"bc_{name}")
                    row = pers.tile([1, width], F32, tag=f"br_{name}")
                    nc.sync.dma_start(row[:], dram[0:1, :])
                    for n0 in range(0, width, 512):
                        nn = min(512, width - n0)
                        ps = psB.tile([128, 512], F32, space="PSUM", tag="bcps")
                        nc.tensor.matmul(
                            ps[:, :nn], lhsT=ones1[:], rhs=row[:, n0 : n0 + nn],
                            start=True, stop=True,
                        )
                        nc.any.tensor_copy(t[:, n0 : n0 + nn], ps[:, :nn])
                    return t

                gatb_bc = bcast_bias(gat_b, HID, "gatb")
                gcnb_bc = bcast_bias(gcn_b, HID, "gcnb")
                fcg1b_bc = bcast_bias(fcg1_b, 1500, "fcg1b")
                fcg2b_bc = bcast_bias(fcg2_b, 128, "fcg2b")
                fxtb_bc = bcast_bias(fxt_b, 128, "fxtb")
                f1b_bc = bcast_bias(f1_b, 1024, "f1b")
                f2b_bc = bcast_bias(f2_b, 512, "f2b")
                f3b_bc = bcast_bias(f3_b, 256, "f3b")
                f4b_bc = bcast_bias(f4_b, 128, "f4b")
                ob_bc = bcast_bias(o_b, 1, "ob")

            # residents
            dstw_t = pers.tile([128, ntile], F32)
            nc.sync.dma_start(dstw_t[:], dstw[:, :])
            batchw_t = pers.tile([128, NWIN], F32)
            nc.sync.dma_start(batchw_t[:], batchw[:, :])
            src_t = pers.tile([128, epc // 16], I16)
            nc.sync.dma_start(src_t[:], src16[:, :])
            tgt_t = pers.tile([128, TOKPAD // 16], I16)
            nc.sync.dma_start(tgt_t[:], tgt16[:, :])

            dinv_all = pers.tile([128, NWIN], F32)
            deg_all = pers.tile([128, NWIN], F32)
            adw_all = pers.tile([128, NWIN, 10], BF16)
            xt_sb = pers.tile([MY_G, 128], F32)
            nc.gpsimd.memset(xt_sb[:], 0.0)

            # shared edge-phase machinery -----------------------------------
            def edge_phase(table, gat, x_epilogue, agg_bufs):
                with (
                    tc.tile_pool(name="msgp", bufs=3) as msgp,
                    tc.tile_pool(name="smallp", bufs=2) as smallp,
                    tc.tile_pool(name="maskp", bufs=6) as maskp,
                    tc.tile_pool(name="epip", bufs=2) as epip,
                    tc.tile_pool(name="psA", bufs=agg_bufs, space="PSUM") as psA,
                    tc.tile_pool(name="psS", bufs=2, space="PSUM") as psS,
                ):
                    aggp = None
                    for c in range(nchunk):
                        isl = slice(c * 128, (c + 1) * 128)
                        msg = msgp.tile([128, 16, RBF], BF16, tag="msg")
                        nc.gpsimd.dma_gather(
                            msg[:],
                            table.ap()[:, 0:RBF],
                            src_t[:, isl],
                            num_idxs=2048,
                            num_idxs_reg=2048,
                            elem_size=RBF,
                            elem_step=RBF,
                            single_packet=bool(int(os.environ.get("KSP", "0"))),
                            queue_num=(c % 2) if bool(int(os.environ.get("KQ2", "0"))) else 0,
                        )
                        maskall = maskp.tile([128, 16, 128], BF16, tag="maskall")
                        if gat:
                            nc.gpsimd.memset(msg[:, :, 830:831], 1.0)
                            sall = smallp.tile([128, 16, 10], F32, tag="sall")
                        # pass 1 (per 4-tile group): masks, a_d expansion,
                        # leaky+exp, message scaling
                        for q4 in range(4):
                            jsl = slice(q4 * 4, q4 * 4 + 4)
                            for i in range(4):
                                g = c * 16 + q4 * 4 + i
                                nc.vector.tensor_tensor(
                                    maskall[:, q4 * 4 + i, :],
                                    dstw_t[:, g : g + 1].to_broadcast([128, 128]),
                                    iota_f[:],
                                    op=OP.is_equal,
                                )
                            if not gat:
                                continue
                            trT = psS.tile([128, 512], BF16, space="PSUM", tag="trT")
                            for i in range(4):
                                nc.tensor.transpose(
                                    trT[:, i * 128 : (i + 1) * 128],
                                    maskall[:, q4 * 4 + i, :],
                                    ident_bf[:],
                                )
                            maskT = maskp.tile([128, 4, 128], BF16, tag="maskT")
                            nc.scalar.copy(maskT[:], trT[:])
                            adx = psS.tile([128, 512], F32, space="PSUM", tag="adx")
                            for i in range(4):
                                nc.tensor.matmul(
                                    adx[:, i * 10 : i * 10 + 10],
                                    lhsT=maskT[:, i, :],
                                    rhs=adw_all[:, (c * 16 + q4 * 4 + i) // tpw, :],
                                    start=True,
                                    stop=True,
                                )
                            nc.vector.tensor_tensor(
                                sall[:, jsl, :],
                                msg[:, jsl, 780:800].bitcast(F32),
                                adx[:, 0:40].rearrange("p (a b) -> p a b", b=10),
                                op=OP.add,
                            )
                            s2 = smallp.tile([128, 4, 10], F32, tag="s2")
                            nc.vector.tensor_scalar_mul(s2[:], sall[:, jsl, :], 0.2)
                            nc.vector.tensor_tensor(
                                sall[:, jsl, :], sall[:, jsl, :], s2[:], op=OP.max
                            )
                            nc.scalar.activation(
                                msg[:, jsl, 820:830], sall[:, jsl, :], AF.Exp
                            )
                            nc.vector.tensor_tensor(
                                msg[:, jsl, 0:HID].rearrange(
                                    "p c (h f) -> p c h f", h=H
                                ),
                                msg[:, jsl, 0:HID].rearrange(
                                    "p c (h f) -> p c h f", h=H
                                ),
                                msg[:, jsl, 820:830, None].to_broadcast([128, 4, H, F]),
                                op=OP.mult,
                            )
                        # pass 2: scatter matmuls
                        for j in range(16):
                            g = c * 16 + j
                            w = g // tpw
                            first = g % tpw == 0
                            last = g % tpw == tpw - 1
                            if first:
                                aggp = psA.tile(
                                    [128, 1024], F32, space="PSUM", tag="aggp"
                                )
                            n_hi = 831 if gat else HID
                            for n0, nn in ((0, 512), (512, n_hi - 512)):
                                nc.tensor.matmul(
                                    aggp[:, n0 : n0 + nn],
                                    lhsT=maskall[:, j, :],
                                    rhs=msg[:, j, n0 : n0 + nn],
                                    start=first,
                                    stop=last,
                                )
                            if last:
                                x_epilogue(w, aggp, epip)

                    return

            def gat_epilogue(w, aggp, epip):
                rec = epip.tile([128, 12], F32, tag="rec")
                nc.vector.tensor_scalar_add(rec[:, 0:11], aggp[:, 820:831], 1e-20)
                nc.vector.tensor_copy(deg_all[:, w : w + 1], rec[:, 10:11])
                rcp = epip.tile([128, 10], F32, tag="rcp")
                nc.vector.reciprocal(rcp[:], rec[:, 0:10])
                x1w = epip.tile([128, HID], F32, tag="x1w")
                nc.vector.tensor_tensor(
                    x1w[:].rearrange("p (h f) -> p h f", h=H),
                    aggp[:, 0:HID].rearrange("p (h f) -> p h f", h=H),
                    rcp[:, :, None].to_broadcast([128, H, F]),
                    op=OP.mult,
                )
                nc.vector.tensor_tensor(x1w[:], x1w[:], gatb_bc[:], op=OP.add)
                nc.vector.tensor_scalar_max(x1w[:], x1w[:], 0.0)
                nc.sync.dma_start(x1d.ap()[w * 128 : (w + 1) * 128, :], x1w[:])

            # ---- phase 1: own h rows; AllGather table ----
            if PH >= 1:
              with (
                tc.tile_pool(name="p1", bufs=1) as p1,
                tc.tile_pool(name="p1h", bufs=3) as p1h,
                tc.tile_pool(name="ps1", bufs=1, space="PSUM") as ps1,
              ):
                xT_sb = p1.tile([78, NPAD], F32)
                nc.sync.dma_start(xT_sb[:], xTc[:, :])
                wp_sb = p1.tile([78, 1024], F32)
                nc.sync.dma_start(wp_sb[:], wpack0[:, :])
                gwt_sb = p1.tile([128, 7, 78], F32)
                nc.sync.dma_start(
                    gwt_sb[:], gat_WT.ap().rearrange("(c p) f -> p c f", p=128)
                )
                was_sb = p1.tile([128, 7, 20], F32)
                nc.sync.dma_start(
                    was_sb[:], wasbd.ap().rearrange("(c p) f -> p c f", p=128)
                )
                wcps = ps1.tile([78, 512], F32, space="PSUM", tag="wcps")
                for kc in range(7):
                    nc.tensor.matmul(
                        wcps[:, 0:20],
                        lhsT=gwt_sb[:, kc, :],
                        rhs=was_sb[:, kc, :],
                        start=(kc == 0),
                        stop=(kc == 6),
                    )
                nc.any.tensor_copy(wp_sb[:, HID : HID + 20], wcps[:, 0:20])

                for t in range(NWIN):
                    hp = ps1.tile([128, 1024], F32, space="PSUM", tag="hp")
                    for n0 in (0, 512):
                        nc.tensor.matmul(
                            hp[:, n0 : n0 + 512],
                            lhsT=xT_sb[:, t * 128 : (t + 1) * 128],
                            rhs=wp_sb[:, n0 : n0 + 512],
                            start=True,
                            stop=True,
                        )
                    hrow = p1h.tile([128, RBF], BF16, tag="hrow")
                    nc.vector.tensor_copy(hrow[:, 0:HID], hp[:, 0:HID])
                    nc.vector.tensor_copy(
                        hrow[:, 780:800].bitcast(F32), hp[:, 780:790]
                    )
                    nc.gpsimd.memset(hrow[:, 800:RBF], 0.0)
                    nc.vector.tensor_copy(adw_all[:, t, :], hp[:, 790:800])
                    nc.sync.dma_start(
                        hin.ap()[t * 128 : (t + 1) * 128, :], hrow[:]
                    )
                nc.gpsimd.collective_compute(
                    "AllGather",
                    OP.bypass,
                    replica_groups=RG,
                    ins=[hin.ap().opt()],
                    outs=[htabG.ap().opt()],
                )

            # ---- phase 2: GAT ----
            if PH >= 2:
                edge_phase(htabG, True, gat_epilogue, agg_bufs=2)
                nc.scalar.activation(dinv_all[:], deg_all[:], AF.Sqrt)
                nc.vector.tensor_scalar_add(dinv_all[:], dinv_all[:], 1e-20)
                nc.vector.reciprocal(dinv_all[:], dinv_all[:])

            # ================== protein scope (overlaps phase 3 + AllGathers) =========
            with (
                tc.tile_pool(name="pp", bufs=1) as pp,
                tc.tile_pool(name="ppg", bufs=2) as ppg,
                tc.tile_pool(name="ppw", bufs=2) as ppw,
                tc.tile_pool(name="psC", bufs=2, space="PSUM") as psC,
                tc.tile_pool(name="psTr", bufs=1, space="PSUM") as psTr,
                tc.tile_pool(name="psX", bufs=1, space="PSUM") as psX,
            ):
                # ---- phase 3: h2s + AllGather ----
                if PH >= 3:
                  with (
                    tc.tile_pool(name="p3", bufs=2) as p3,
                    tc.tile_pool(name="p3w", bufs=1) as p3w,
                    tc.tile_pool(name="psT3", bufs=1, space="PSUM") as psT3,
                    tc.tile_pool(name="psH3", bufs=1, space="PSUM") as psH3,
                  ):
                    gcnw_sb = p3w.tile([128, 7, HID], F32)
                    nc.sync.dma_start(
                        gcnw_sb[:], gcn_Wp.ap().rearrange("(c p) f -> p c f", p=128)
                    )
                    for t in range(NWIN):
                        x1t = p3.tile([128, HID], F32, tag="x1t")
                        nc.sync.dma_start(x1t[:], x1d.ap()[t * 128 : (t + 1) * 128, :])
                        x1T = p3.tile([128, 7, 128], F32, tag="x1T")
                        for kc in range(7):
                            sz = 128 if kc < 6 else 12
                            trp = psT3.tile([128, 128], F32, space="PSUM", tag="trp")
                            nc.tensor.transpose(
                                trp[0:sz, :], x1t[:, kc * 128 : kc * 128 + sz], ident[:]
                            )
                            nc.any.tensor_copy(x1T[0:sz, kc, :], trp[0:sz, :])
                        h2ps = psH3.tile([128, 1024], F32, space="PSUM", tag="h2ps")
                        for n0, nn in ((0, 512), (512, 268)):
                            for kc in range(7):
                                sz = 128 if kc < 6 else 12
                                nc.tensor.matmul(
                                    h2ps[:, n0 : n0 + nn],
                                    lhsT=x1T[0:sz, kc, :],
                                    rhs=gcnw_sb[0:sz, kc, n0 : n0 + nn],
                                    start=(kc == 0),
                                    stop=(kc == 6),
                                )
                        h2s = p3.tile([128, RBF], BF16, tag="h2s")
                        nc.vector.tensor_tensor(
                            h2s[:, 0:HID],
                            h2ps[:, 0:HID],
                            dinv_all[:, t : t + 1].to_broadcast([128, HID]),
                            op=OP.mult,
                        )
                        nc.gpsimd.memset(h2s[:, HID:RBF], 0.0)
                        nc.sync.dma_start(agin.ap()[t * 128 : (t + 1) * 128, :], h2s[:])
                    nc.gpsimd.collective_compute(
                        "AllGather",
                        OP.bypass,
                        replica_groups=RG,
                        ins=[agin.ap().opt()],
                        outs=[htab2G.ap().opt()],
                    )

                # ---- protein branch (gap-filler; no deps on graph phases) ----
                if PH >= 4:
                    cwt_sb = pp.tile([128, KS, NF], BF16)
                    nc.sync.dma_start(
                        cwt_sb[:], cwt_bf.ap().rearrange("k p o -> p k o")
                    )
                    cb_sb = pp.tile([NF, 1], F32)
                    nc.sync.dma_start(cb_sb[:], cb.ap()[:, :])
                    cT_all = pp.tile([128, 8, NF, BPC], F32)

                    for grp in range(5):
                        gt = ppg.tile([128, 40, 128], BF16, tag="embg")
                        nc.gpsimd.dma_gather(
                            gt[:],
                            emb_bf.ap()[:, :],
                            tgt_t[:, grp * 320 : (grp + 1) * 320],
                            num_idxs=5120,
                            num_idxs_reg=5120,
                            elem_size=128,
                            elem_step=128,
                            single_packet=bool(int(os.environ.get("KSP", "0"))),
                            queue_num=1 if bool(int(os.environ.get("KQ2", "0"))) else 0,
                        )
                        et5 = ppg.tile([128, 5120], BF16, tag="et5")
                        for i in range(40):
                            trp = psTr.tile([128, 128], BF16, space="PSUM", tag="trp2")
                            nc.tensor.transpose(trp[:], gt[:, i, :], ident_bf[:])
                            nc.any.tensor_copy(et5[:, i * 128 : (i + 1) * 128], trp[:])
                        for bl in range(5):
                            b = grp * 5 + bl
                            boff = bl * 1000
                            csb = pp.tile([NF, 1024], F32, tag="csb")
                            for p0 in (0, 512):
                                cps = psC.tile([NF, 512], F32, space="PSUM", tag="cps")
                                for k in range(KS):
                                    nc.tensor.matmul(
                                        cps[:, 0:512],
                                        lhsT=cwt_sb[:, k, :],
                                        rhs=et5[:, boff + k + p0 : boff + k + p0 + 512],
                                        start=(k == 0),
                                        stop=(k == KS - 1),
                                    )
                                nc.scalar.activation(
                                    csb[:, p0 : p0 + 512], cps[:, 0:512],
                                    AF.Identity, bias=cb_sb[:, 0:1],
                                )
                            for pc in range(8):
                                trc = psTr.tile(
                                    [128, 128], F32, space="PSUM", tag="trc"
                                )
                                nc.tensor.transpose(
                                    trc[:, 0:NF],
                                    csb[:, pc * 128 : (pc + 1) * 128],
                                    ident[0:NF, 0:NF],
                                )
                                nc.any.tensor_copy(cT_all[:, pc, :, b], trc[:, 0:NF])

                    xtps = psX.tile([MY_G, 128], F32, space="PSUM", tag="xtps")
                    for sc in range(16):
                        wpt = ppw.tile([128, 16, 128], F32, tag="wpt")
                        nc.sync.dma_start(
                            wpt[:],
                            fxt_Wp.ap()[sc * 2048 : (sc + 1) * 2048, :].rearrange(
                                "(c p) j -> p c j", p=128
                            ),
                        )
                        for sub in range(16):
                            q = sc * 16 + sub
                            o, t8 = q // 8, q % 8
                            nc.tensor.matmul(
                                xtps[0:BPC, :],
                                lhsT=cT_all[:, t8, o, :],
                                rhs=wpt[:, sub, :],
                                start=(q == 0),
                                stop=(q == 255),
                            )
                    nc.vector.tensor_tensor(
                        xt_sb[0:BPC, :], xtps[0:BPC, :], fxtb_bc[0:BPC, :], op=OP.add
                    )

            # ================== phase 4: GCN + pooling; phase 5: head ========
            if PH >= 5:
              with tc.tile_pool(name="psP", bufs=1, space="PSUM") as psP:
                poolps = psP.tile([GSLOT, 1024], F32, space="PSUM", tag="poolps")

                def gcn_epilogue(w, aggp, epip):
                    x2w = epip.tile([128, HID], F32, tag="x2w")
                    nc.vector.tensor_tensor(
                        x2w[:],
                        aggp[:, 0:HID],
                        dinv_all[:, w : w + 1].to_broadcast([128, HID]),
                        op=OP.mult,
                    )
                    nc.vector.tensor_tensor(x2w[:], x2w[:], gcnb_bc[:], op=OP.add)
                    nc.vector.tensor_scalar_max(x2w[:], x2w[:], 0.0)
                    ph = epip.tile([128, GSLOT], F32, tag="poolhot")
                    nc.vector.tensor_tensor(
                        ph[:],
                        batchw_t[:, w : w + 1].to_broadcast([128, GSLOT]),
                        iota_f[:, 0:GSLOT],
                        op=OP.is_equal,
                    )
                    for n0, nn in ((0, 512), (512, 268)):
                        nc.tensor.matmul(
                            poolps[:, n0 : n0 + nn],
                            lhsT=ph[:],
                            rhs=x2w[:, n0 : n0 + nn],
                            start=(w == 0),
                            stop=(w == NWIN - 1),
                        )

                edge_phase(htab2G, False, gcn_epilogue, agg_bufs=2)
                poolsb = pers.tile([GSLOT, HID], F32)
                nc.any.tensor_copy(poolsb[:], poolps[:, 0:HID])

              with (
                    tc.tile_pool(name="p5", bufs=1) as p5,
                    tc.tile_pool(name="p5w", bufs=2) as p5w,
                    tc.tile_pool(name="ps5", bufs=2, space="PSUM") as ps5,
                    tc.tile_pool(name="ps5t", bufs=2, space="PSUM") as ps5t,
                ):
                    nc.sync.dma_start(poolin.ap()[:, :], poolsb[:])
                    nc.gpsimd.collective_compute(
                        "AllGather",
                        OP.bypass,
                        replica_groups=RG,
                        ins=[poolin.ap().opt()],
                        outs=[poolall.ap().opt()],
                    )
                    Cc_sb = p5.tile([128, 4, MY_G], F32)
                    nc.sync.dma_start(
                        Cc_sb[:], Cc.ap().rearrange("(c p) g -> p c g", p=128)
                    )
                    pall = p5.tile([128, 4, HID], F32)
                    nc.sync.dma_start(
                        pall[:], poolall.ap().rearrange("(c p) f -> p c f", p=128)
                    )
                    xgps = ps5.tile([MY_G, 1024], F32, space="PSUM", tag="mlp_ps")
                    for kc in range(4):
                        for n0, nn in ((0, 512), (512, 268)):
                            nc.tensor.matmul(
                                xgps[:, n0 : n0 + nn],
                                lhsT=Cc_sb[:, kc, :],
                                rhs=pall[:, kc, n0 : n0 + nn],
                                start=(kc == 0),
                                stop=(kc == 3),
                            )
                    xg = p5.tile([MY_G, HID], F32, tag="act0")
                    nc.any.tensor_copy(xg[:], xgps[:, 0:HID])

                    def dense(x_sb, k_real, w_dram, w_rows, n_out, b_bc, relu, tag):
                        nkc = (k_real + 127) // 128
                        xT_t = p5.tile([128, nkc, MY_G], F32, tag="xT5")
                        for kc in range(nkc):
                            sz = min(128, k_real - kc * 128)
                            trp = ps5t.tile([128, MY_G], F32, space="PSUM", tag="tr5")
                            nc.tensor.transpose(
                                trp[0:sz, :],
                                x_sb[:, kc * 128 : kc * 128 + sz],
                                ident[0:MY_G, 0:MY_G],
                            )
                            nc.any.tensor_copy(xT_t[0:sz, kc, :], trp[0:sz, :])
                        w_sb = p5w.tile([128, w_rows // 128, n_out], F32, tag="w5")
                        nc.sync.dma_start(
                            w_sb[:], w_dram.ap().rearrange("(c p) f -> p c f", p=128)
                        )
                        yps = ps5.tile([MY_G, 1536], F32, space="PSUM", tag="mlp_ps")
                        for n0 in range(0, n_out, 512):
                            nn = min(512, n_out - n0)
                            for kc in range(nkc):
                                sz = min(128, k_real - kc * 128)
                                nc.tensor.matmul(
                                    yps[:, n0 : n0 + nn],
                                    lhsT=xT_t[0:sz, kc, :],
                                    rhs=w_sb[0:sz, kc, n0 : n0 + nn],
                                    start=(kc == 0),
                                    stop=(kc == nkc - 1),
                                )
                        y = p5.tile([MY_G, n_out], F32, tag="y5")
                        nc.vector.tensor_tensor(
                            y[:], yps[:, 0:n_out], b_bc[0:MY_G, 0:n_out], op=OP.add
                        )
                        if relu:
                            nc.vector.tensor_scalar_max(y[:], y[:], 0.0)
                        return y

                    y1 = dense(xg, HID, fcg1_Wp, 896, 1500, fcg1b_bc, True, "fcg1")
                    xgo = dense(y1, 1500, fcg2_Wp, 1536, 128, fcg2b_bc, False, "fcg2")
                    xc = p5.tile([MY_G, 256], F32, tag="xc")
                    nc.any.tensor_copy(xc[:, 0:128], xgo[:])
                    nc.any.tensor_copy(xc[:, 128:256], xt_sb[:])
                    a1 = dense(xc, 256, f1_W, 256, 1024, f1b_bc, True, "f1")
                    a2 = dense(a1, 1024, f2_W, 1024, 512, f2b_bc, True, "f2")
                    a3 = dense(a2, 512, f3_W, 512, 256, f3b_bc, True, "f3")
                    a4 = dense(a3, 256, f4_W, 256, 128, f4b_bc, True, "f4")
                    yo = dense(a4, 128, o_W, 128, 1, ob_bc, False, "o")
                    nc.sync.dma_start(out_d.ap()[:, :], yo[:])
                    if KDEBUG:
                        dbg = p5.tile([128, 2, RBF], BF16, tag="dbg")
                        nc.sync.dma_start(
                            dbg[:], htabG.ap()[0:256, :].rearrange("(c p) f -> p c f", p=128)
                        )
                        nc.sync.dma_start(
                            out_h.ap().rearrange("(c p) f -> p c f", p=128), dbg[:]
                        )
                        dbg2 = p5.tile([128, NWIN, HID], F32, tag="dbg2")
                        nc.sync.dma_start(
                            dbg2[:], x1d.ap().rearrange("(c p) f -> p c f", p=128)
                        )
                        nc.sync.dma_start(
                            out_x1.ap().rearrange("(c p) f -> p c f", p=128), dbg2[:]
                        )
                        nc.sync.dma_start(
                            out_adw.ap(), adw_all[:].rearrange("p a b -> p (a b)")
                        )
                        nc.sync.dma_start(out_xt.ap()[:, :], xt_sb[:])

    nc.compile()
    _CACHE[key] = nc
    return nc


# ---------------------------------------------------------------- entry point


def _ensure_ntff_hook():
    """Install antenv.axon_hooks + register the ctypes NTFF hook if the image
    lacks them (profiling only; failures are non-fatal)."""
    import types

    try:
        import antenv.axon_hooks  # noqa: F401

        if antenv.axon_hooks.get_axon_ntff_profile_hook() is not None:
            return
    except ImportError:
        import antenv

        mod = types.ModuleType("antenv.axon_hooks")
        mod._hook = None

        def set_axon_ntff_profile_hook(h, _m=mod):
            _m._hook = h

        def get_axon_ntff_profile_hook(_m=mod):
            return _m._hook

        mod.set_axon_ntff_profile_hook = set_axon_ntff_profile_hook
        mod.get_axon_ntff_profile_hook = get_axon_ntff_profile_hook
        sys.modules["antenv.axon_hooks"] = mod
        antenv.axon_hooks = mod
    try:
        from antenv.axon_hooks import set_axon_ntff_profile_hook as _set
        from trn_agent_boot.trn_boot import _ntff_profile_via_ctypes

        hook = _ntff_profile_via_ctypes("/opt/axon/libaxon_pjrt.so")
        if hook is not None:
            _set(hook)
    except Exception:
        pass


def _enable_ldw_opt():
    """Turn on walrus's LDWEIGHTS dedup pass (consecutive matmuls sharing a
    stationary operand skip the reload). Opt-in via KLDWOPT=1."""
    import concourse.bass_utils as bu

    if getattr(bu, "_ldw_patched", False):
        return
    orig = bu.run_command

    def patched(argv, **kw):
        argv = [
            "--enable-ldw-opt=true" if a == "--enable-ldw-opt=false" else a
            for a in argv
        ]
        return orig(argv, **kw)

    bu.run_command = patched
    bu._ldw_patched = True


def kernel(**inputs) -> np.ndarray:
    if bool(int(os.environ.get("KLDWOPT", "0"))):
        _enable_ldw_opt()
    per_core, meta = host_prep(inputs)
    nc = build_bass(meta)
    in_maps = [{k: np.ascontiguousarray(v) for k, v in d.items()} for d in per_core]
    trace = bool(int(os.environ.get("KERNEL_TRACE", "0")))
    if trace:
        _ensure_ntff_hook()
    res = run_bass_kernel_spmd(nc, in_maps, core_ids=list(range(NCORES)), trace=trace)
    if trace and res.exec_time_ns is not None:
        print(f"HW exec time: {res.exec_time_ns} ns")
        kernel.last_exec_ns = res.exec_time_ns
    out = np.concatenate([res.results[c]["out"][:BPC] for c in range(NCORES)], 0)
    return out.astype(np.float32)



# revision 27
# speedup vs baseline: 1.2274x; 1.1120x over previous
"""Trainium2 Bass kernel for nn_EnhancedGATGCN (GAT -> GCN -> pool -> MLP, + protein conv branch).

Self-contained: host-side sharding prep + 8-core SPMD Bass/Tile device program.

v2 design (all sizes bf16 unless noted):
  - Edges sorted by dst, 8-way dst-sharded; scatter-add via one-hot mask matmuls
    into per-128-dst-window PSUM. tpw exact (no round-to-4). Row tables gathered
    via SWDGE dma_gather alternating across queues; nothing else runs on the
    Pool engine during edge phases (drain-rate paced).
  - GCN aggregates dinv*x1 rows directly (aggregate-then-project): the x1@W
    projection happens per dst window in the GCN epilogue; phase 3 eliminated.
  - Protein branch: conv1d computed as extended-one-hot matmuls (no embedding
    gather): OHx[26*t+v, q] = (tok[s+q+t]==v), C_blk = OHx.T @ Vx with
    Vx[26t+v,:] = emb[v] @ cW[:,:,t].T host-precomputed; 121 valid positions
    per 128-token block. conv bias folded into fxt bias on host. Runs during
    AllGather #1; the fxt matmul runs during AllGather #2.
  - Dense tail kept transposed ([feat, graph]) so no per-layer transposes;
    per-partition column biases.
"""
import os
import sys

import numpy as np

sys.path.insert(0, "/opt/trn_rl_repo")

import ml_dtypes

import concourse.bacc as bacc
import concourse.bass as bass
import concourse.mybir as mybir
import concourse.tile as tile
from concourse.bass_utils import run_bass_kernel_spmd
from concourse.masks import make_identity

F32 = mybir.dt.float32
BF16 = mybir.dt.bfloat16
I16 = mybir.dt.int16
I32 = mybir.dt.int32
AF = mybir.ActivationFunctionType
OP = mybir.AluOpType
BF = ml_dtypes.bfloat16

N, E, B, H, F = 20000, 400000, 200, 10, 78
HID = H * F  # 780
SEQ, VOC, EMB, NF, KS = 1000, 26, 128, 26 * 0 + 32, 8
CONV_OUT = SEQ - KS + 1  # 993

NCORES = 8
NPC = N // NCORES  # 2500
NPAD = 2560
NWIN = NPAD // 128  # 20
RBF = 896  # bf16 cols per table row; 1792 B/row (%256==0)
# GAT msg row: [0:780 h | 780:800 a_s 10xf32-packed | 800:810 exd | 810 ones | 811:896 junk]
GSLOT = 64
BPC = B // NCORES  # 25
NBLK = 9  # conv position blocks per graph (121 valid pos each)
TOKB = 136  # tokens shipped per block (121 + 7 tap overlap + pad)
TOKP = NBLK * TOKB  # 1224
NQ = int(os.environ.get("KNQ", "2"))  # swdge queues


# ---------------------------------------------------------------- host prep


def _wrap16(idx, epc):
    a = np.zeros((128, epc // 16), np.int16)
    w = idx.reshape(epc // 16, 16).T
    a[:, :] = np.tile(w, (8, 1))
    return a


def host_prep(inputs):
    x = np.asarray(inputs["x"], np.float32)
    edge_index = np.asarray(inputs["edge_index"], np.int64)
    batch = np.asarray(inputs["batch"], np.int64)
    target = np.asarray(inputs["target"], np.int64)

    loops = np.arange(N, dtype=np.int64)
    src = np.concatenate([edge_index[0], loops])
    dst = np.concatenate([edge_index[1], loops])
    order = np.argsort(dst, kind="stable")
    src, dst = src[order], dst[order]

    core_of = dst // NPC
    dst_local = dst - core_of * NPC
    win = dst_local // 128
    maxw = 0
    per_core_edges = []
    for c in range(NCORES):
        m = core_of == c
        s_c, dl_c, w_c = src[m], dst_local[m], win[m]
        per_core_edges.append((s_c, dl_c, w_c))
        maxw = max(maxw, int(np.bincount(w_c, minlength=NWIN).max()))
    tpw = -(-maxw // 128)
    ntile = NWIN * tpw
    epc = ntile * 128
    nchunk = -(-ntile // 16)

    def remap(n):
        return (n // NPC) * NPAD + (n % NPC)

    cores = []
    for c in range(NCORES):
        s_c, dl_c, w_c = per_core_edges[c]
        es = np.zeros(epc, np.int64)
        ew = np.full(epc, -1000.0, np.float32)
        for w in range(NWIN):
            m = w_c == w
            k = int(m.sum())
            o = w * tpw * 128
            es[o : o + k] = s_c[m]
            ew[o : o + k] = (dl_c[m] - w * 128).astype(np.float32)
        cores.append(dict(es=remap(es), ew=ew))

    # GAT weight pack: [0:780 W | 780:790 W@a_src per head | 790:800 W@a_dst]
    gat_W = np.asarray(inputs["gat_W"], np.float32)
    a_src = np.asarray(inputs["gat_a_src"], np.float32)
    a_dst = np.asarray(inputs["gat_a_dst"], np.float32)
    wpack = np.zeros((78, 1024), np.float32)
    wpack[:, :HID] = gat_W
    for h in range(H):
        wpack[:, HID + h] = gat_W[:, h * F : (h + 1) * F] @ a_src[h]
        wpack[:, HID + 10 + h] = gat_W[:, h * F : (h + 1) * F] @ a_dst[h]

    gcn_W_pad = np.zeros((896, 784), np.float32)
    gcn_W_pad[:HID, :HID] = np.asarray(inputs["gcn_W"], np.float32)

    # protein: Vx tables + reordered fxt weights (+ conv bias folded into fxt_b)
    emb = np.asarray(inputs["emb"], np.float32)
    cW = np.asarray(inputs["cW"], np.float32)  # [NF, EMB, KS]
    cb = np.asarray(inputs["cb"], np.float32)
    # tap t occupies 32-partition-aligned row group (t%4)*32; rows 26-31 zero
    Vx = np.zeros((KS * 32, NF), np.float32)
    for t in range(KS):
        Vx[t * 32 : t * 32 + VOC] = emb @ cW[:, :, t].T
    fxt_W = np.asarray(inputs["fxt_W"], np.float32)  # [NF*993, 128]
    fxt_b = np.asarray(inputs["fxt_b"], np.float32)
    fxt_b2 = fxt_b + cb @ fxt_W.reshape(NF, CONV_OUT, 128).sum(axis=1)
    fxtW = np.zeros((NF, NBLK, 128, 128), np.float32)
    for blk in range(NBLK):
        s = blk * 121
        n = min(121, CONV_OUT - s)
        fxtW[:, blk, :n] = fxt_W.reshape(NF, CONV_OUT, 128)[:, s : s + n]
    fxtW = fxtW.reshape(NF * NBLK * 128, 128)

    # head weights (transposed-chain layout, bf16) + column biases
    def colbias(b, n):
        nc_ = -(-n // 128)
        col = np.zeros((nc_, 128), np.float32)
        col.reshape(-1)[: len(b)] = b
        return col.T.copy()

    fcg1_W = np.zeros((896, 1536), np.float32)
    fcg1_W[:HID, :1500] = np.asarray(inputs["fcg1_W"], np.float32)
    fcg2_W = np.zeros((1536, 128), np.float32)
    fcg2_W[:1500] = np.asarray(inputs["fcg2_W"], np.float32)

    # graph slot bookkeeping
    gbase = np.array([batch[c * NPC] for c in range(NCORES)], np.int64)
    span = np.array(
        [batch[min(c * NPC + NPC, N) - 1] - gbase[c] + 1 for c in range(NCORES)]
    )
    assert span.max() <= GSLOT, span.max()
    Cc_all = []
    for c in range(NCORES):
        Cmat = np.zeros((NCORES * GSLOT, BPC), np.float32)
        for r in range(NCORES):
            for slot in range(GSLOT):
                g = gbase[r] + slot
                col = g - c * BPC
                if 0 <= col < BPC and g < B:
                    Cmat[r * GSLOT + slot, col] = 1.0
        Cc_all.append(Cmat)

    vmod = np.full((128, 1), -2.0, np.float32)
    for gi in range(4):
        vmod[gi * 32 : gi * 32 + VOC, 0] = np.arange(VOC)

    RT = -(-ntile // 128)
    meta = dict(tpw=tpw, ntile=ntile, epc=epc, nchunk=nchunk, RT=RT)

    per_core = []
    for c in range(NCORES):
        ed_ = cores[c]
        bw = np.full(NPAD, -1000.0, np.float32)
        bw[:NPC] = (batch[c * NPC : (c + 1) * NPC] - gbase[c]).astype(np.float32)
        batchw = bw.reshape(NWIN, 128).T.copy()

        dstw = ed_["ew"].reshape(ntile, 128).T.copy()  # [128, ntile]

        xTc = np.zeros((78, NPAD), np.float32)
        xTc[:, :NPC] = x[c * NPC : (c + 1) * NPC].T

        tokf = np.full((BPC, TOKP), -1.0, np.float32)
        tg = target[c * BPC : (c + 1) * BPC].astype(np.float32)
        for blk in range(NBLK):
            s = blk * 121
            n = min(TOKB - 1, SEQ - s)
            tokf[:, blk * TOKB : blk * TOKB + n] = tg[:, s : s + n]
        tokb = np.broadcast_to(tokf.astype(BF)[None, :, :], (128, BPC, TOKP))

        d = {
            "xTc": xTc,
            "wpack": wpack,
            "src16": _wrap16(ed_["es"], epc),
            "dstw": dstw,
            "batchw": batchw,
            "vmod": vmod.astype(BF),
            "tokb": tokb,
            "Vxa": Vx[:128],
            "Vxb": Vx[128:],
            "fxtW": fxtW,
            "fxtb_col": colbias(fxt_b2, 128),
            "gat_b": np.asarray(inputs["gat_b"], np.float32).reshape(1, HID),
            "gcnW": gcn_W_pad,
            "gcn_b": np.asarray(inputs["gcn_b"], np.float32).reshape(1, HID),
            "fcg1W": fcg1_W,
            "fcg1b_col": colbias(np.asarray(inputs["fcg1_b"], np.float32), 1536),
            "fcg2W": fcg2_W,
            "fcg2b_col": colbias(np.asarray(inputs["fcg2_b"], np.float32), 128),
            "f1W": np.asarray(inputs["f1_W"], np.float32),
            "f1b_col": colbias(np.asarray(inputs["f1_b"], np.float32), 1024),
            "f2W": np.asarray(inputs["f2_W"], np.float32),
            "f2b_col": colbias(np.asarray(inputs["f2_b"], np.float32), 512),
            "f3W": np.asarray(inputs["f3_W"], np.float32),
            "f3b_col": colbias(np.asarray(inputs["f3_b"], np.float32), 256),
            "f4W": np.asarray(inputs["f4_W"], np.float32),
            "f4b_col": colbias(np.asarray(inputs["f4_b"], np.float32), 128),
            "oW": np.asarray(inputs["o_W"], np.float32),
            "o_b": np.asarray(inputs["o_b"], np.float32).reshape(1, 1),
            "Cc": Cc_all[c],
        }
        per_core.append(d)
    return per_core, meta


# ---------------------------------------------------------------- device build

_CACHE = {}


def build_bass(meta):
    key = (meta["tpw"], NQ)
    if key in _CACHE:
        return _CACHE[key]

    tpw, ntile, epc, nchunk, RT = (
        meta["tpw"], meta["ntile"], meta["epc"], meta["nchunk"], meta["RT"],
    )

    nc = bacc.Bacc(
        "TRN2",
        target_bir_lowering=False,
        debug=False,
        num_devices=NCORES,
        num_swdge_queues=NQ,
    )

    def inp(name, shape, dt=F32):
        return nc.dram_tensor(name, list(shape), dt, kind="ExternalInput")

    xTc = inp("xTc", (78, NPAD))
    wpack = inp("wpack", (78, 1024))
    src16 = inp("src16", (128, epc // 16), I16)
    dstw = inp("dstw", (128, ntile))
    batchw = inp("batchw", (128, NWIN))
    vmod = inp("vmod", (128, 1), BF16)
    tokb = inp("tokb", (128, BPC, TOKP), BF16)
    Vxa = inp("Vxa", (128, NF))
    Vxb = inp("Vxb", (128, NF))
    fxtW = inp("fxtW", (NF * NBLK * 128, 128))
    fxtb_col = inp("fxtb_col", (128, 1))
    gat_b = inp("gat_b", (1, HID))
    gcnW = inp("gcnW", (896, 784))
    gcn_b = inp("gcn_b", (1, HID))
    fcg1W = inp("fcg1W", (896, 1536))
    fcg1b_col = inp("fcg1b_col", (128, 12))
    fcg2W = inp("fcg2W", (1536, 128))
    fcg2b_col = inp("fcg2b_col", (128, 1))
    f1W = inp("f1W", (256, 1024))
    f1b_col = inp("f1b_col", (128, 8))
    f2W = inp("f2W", (1024, 512))
    f2b_col = inp("f2b_col", (128, 4))
    f3W = inp("f3W", (512, 256))
    f3b_col = inp("f3b_col", (128, 2))
    f4W = inp("f4W", (256, 128))
    f4b_col = inp("f4b_col", (128, 1))
    oW = inp("oW", (128, 1))
    o_b = inp("o_b", (1, 1))
    Cc = inp("Cc", (NCORES * GSLOT, BPC))

    out_d = nc.dram_tensor("out", [1, BPC], F32, kind="ExternalOutput")
    KDEBUG = bool(int(os.environ.get("KDEBUG", "0")))
    if KDEBUG:
        out_ag = nc.dram_tensor("out_ag", [NPAD, RBF], BF16, kind="ExternalOutput")
        out_pool = nc.dram_tensor("out_pool", [GSLOT, 784], F32, kind="ExternalOutput")
        out_xt = nc.dram_tensor("out_xt", [128, BPC], F32, kind="ExternalOutput")
        out_xg = nc.dram_tensor("out_xg", [128, 7 * BPC], F32, kind="ExternalOutput")

    hin = nc.dram_tensor("hin", [NPAD, RBF], BF16)
    htabG = nc.dram_tensor("htabG", [NCORES * NPAD, RBF], BF16, addr_space="Shared")
    agin = nc.dram_tensor("agin", [NPAD, RBF], BF16)
    htab2G = nc.dram_tensor("htab2G", [NCORES * NPAD, RBF], BF16, addr_space="Shared")
    poolin = nc.dram_tensor("poolin", [GSLOT, 784], F32)
    poolall = nc.dram_tensor("poolall", [NCORES * GSLOT, 784], F32, addr_space="Shared")

    RG = [list(range(NCORES))]

    with tile.TileContext(nc) as tc:
        import contextlib

        ctx = contextlib.ExitStack()
        with ctx:
            pers = ctx.enter_context(tc.tile_pool(name="pers", bufs=1))

            # consts
            iota_i = pers.tile([128, 128], I32)
            nc.gpsimd.iota(iota_i[:], pattern=[[1, 128]], base=0, channel_multiplier=0)
            iota_f = pers.tile([128, 128], F32)
            nc.vector.tensor_copy(iota_f[:], iota_i[:])
            ident_bf = pers.tile([128, 128], BF16)
            identf = pers.tile([128, 128], F32)
            make_identity(nc, identf[:])
            nc.vector.tensor_copy(ident_bf[:], identf[:])
            ones1 = pers.tile([1, 128], F32)
            nc.gpsimd.memset(ones1[:], 1.0)
            onesc = pers.tile([128, 16], BF16)
            nc.gpsimd.memset(onesc[:], 1.0)

            # residents
            dstw_t = pers.tile([128, ntile], F32)
            nc.sync.dma_start(dstw_t[:], dstw[:, :])
            batchw_t = pers.tile([128, NWIN], F32)
            nc.sync.dma_start(batchw_t[:], batchw[:, :])
            src_t = pers.tile([128, epc // 16], I16)
            nc.sync.dma_start(src_t[:], src16[:, :])
            vmod_t = pers.tile([128, 1], BF16)
            nc.sync.dma_start(vmod_t[:], vmod[:, :])
            Vxa_t = pers.tile([128, NF], F32)
            nc.sync.dma_start(Vxa_t[:], Vxa[:, :])
            Vxb_t = pers.tile([128, NF], F32)
            nc.sync.dma_start(Vxb_t[:], Vxb[:, :])
            fxtb_t = pers.tile([128, 1], F32)
            nc.sync.dma_start(fxtb_t[:], fxtb_col[:, :])

            dinv_all = pers.tile([128, NWIN], F32)
            adw_all = pers.tile([128, NWIN, 10], BF16)
            cT = pers.tile([128, NBLK, NF, BPC], F32)
            xtT_sb = pers.tile([128, BPC], F32)

            # broadcast biases (row-replicated tiles)
            bias_tiles = {}
            with tc.tile_pool(name="psB", bufs=1, space="PSUM") as psB:

                def bcast_bias(dram, width, name):
                    t = pers.tile([128, width], F32, tag=f"bc_{name}")
                    row = pers.tile([1, width], F32, tag=f"br_{name}")
                    nc.sync.dma_start(row[:], dram[0:1, :])
                    for n0 in range(0, width, 512):
                        nn = min(512, width - n0)
                        ps = psB.tile([128, 512], F32, space="PSUM", tag="bcps")
                        nc.tensor.matmul(
                            ps[:, :nn], lhsT=ones1[:], rhs=row[:, n0 : n0 + nn],
                            start=True, stop=True,
                        )
                        nc.any.tensor_copy(t[:, n0 : n0 + nn], ps[:, :nn])
                    return t

                gatb_bc = bcast_bias(gat_b, HID, "gatb")
                gcnb_bc = bcast_bias(gcn_b, HID, "gcnb")

            # ---- phase 1: own h rows ----
            with (
                tc.tile_pool(name="p1", bufs=1) as p1,
                tc.tile_pool(name="p1h", bufs=3) as p1h,
                tc.tile_pool(name="ps1", bufs=2, space="PSUM") as ps1,
            ):
                xT_sb = p1.tile([78, NPAD], F32)
                nc.sync.dma_start(xT_sb[:], xTc[:, :])
                wp_sb = p1.tile([78, 1024], F32)
                nc.sync.dma_start(wp_sb[:], wpack[:, :])
                for t in range(NWIN):
                    hp = ps1.tile([128, 1024], F32, space="PSUM", tag="hp")
                    for n0 in (0, 512):
                        nc.tensor.matmul(
                            hp[:, n0 : n0 + 512],
                            lhsT=xT_sb[:, t * 128 : (t + 1) * 128],
                            rhs=wp_sb[:, n0 : n0 + 512],
                            start=True,
                            stop=True,
                        )
                    hrow = p1h.tile([128, 800], BF16, tag="hrow")
                    nc.vector.tensor_copy(hrow[:, 0:HID], hp[:, 0:HID])
                    nc.vector.tensor_copy(
                        hrow[:, 780:800].bitcast(F32), hp[:, 780:790]
                    )
                    nc.vector.tensor_copy(adw_all[:, t, :], hp[:, 790:800])
                    nc.sync.dma_start(
                        hin.ap()[t * 128 : (t + 1) * 128, 0:800], hrow[:]
                    )
                nc.gpsimd.collective_compute(
                    "AllGather",
                    OP.bypass,
                    replica_groups=RG,
                    ins=[hin.ap().opt()],
                    outs=[htabG.ap().opt()],
                )

            # ---- protein conv (runs during AllGather #1; no graph deps) ----
            with (
                tc.tile_pool(name="ppo", bufs=2) as ppo,
                tc.tile_pool(name="ppt", bufs=2) as ppt,
                tc.tile_pool(name="psPC", bufs=1, space="PSUM") as psPC,
                tc.tile_pool(name="psCq", bufs=2, space="PSUM") as psCq,
            ):
                for g in range(BPC):
                    tokrep = ppt.tile([128, TOKP], BF16, tag="tokrep")
                    nc.sync.dma_start(tokrep[:], tokb.ap()[:, g, :])
                    OHa = ppo.tile([128, NBLK, 128], F32, tag="OHa")
                    OHb = ppo.tile([128, NBLK, 128], F32, tag="OHb")
                    tokr = tokrep.rearrange("p (b q) -> p b q", q=TOKB)
                    for t in range(KS):
                        p0 = (t % 4) * 32
                        oh = (OHa if t < 4 else OHb)[p0 : p0 + 32, :, :]
                        nc.vector.tensor_tensor(
                            oh,
                            tokr[p0 : p0 + 32, :, t : t + 128],
                            vmod_t[p0 : p0 + 32, :, None].to_broadcast(
                                [32, NBLK, 128]
                            ),
                            op=OP.is_equal,
                        )
                    Cq = psCq.tile([128, NBLK, NF], F32, space="PSUM", tag="Cq")
                    for blk in range(NBLK):
                        nc.tensor.matmul(
                            Cq[:, blk, :], lhsT=OHa[:, blk, :], rhs=Vxa_t[:],
                            start=True, stop=False,
                        )
                        nc.tensor.matmul(
                            Cq[:, blk, :], lhsT=OHb[:, blk, :], rhs=Vxb_t[:],
                            start=False, stop=True,
                        )
                    nc.scalar.copy(cT[:, :, :, g], Cq[:, :, :])

            # ---- phase 2: GAT edge phase ----
            def edge_phase(table, gat, epilogue):
                with (
                    tc.tile_pool(name="msgp", bufs=3) as msgp,
                    tc.tile_pool(name="maskp", bufs=3) as maskp,
                    tc.tile_pool(name="mtp", bufs=2) as mtp,
                    tc.tile_pool(name="smallp", bufs=2) as smallp,
                    tc.tile_pool(name="epip", bufs=2) as epip,
                    tc.tile_pool(name="psA", bufs=2, space="PSUM") as psA,
                    tc.tile_pool(name="psS", bufs=2, space="PSUM") as psS,
                    tc.tile_pool(name="psD", bufs=2, space="PSUM") as psD,
                ):
                    aggp = None
                    for c in range(nchunk):
                        T = min(16, ntile - c * 16)
                        msg = msgp.tile([128, 16, RBF], BF16, tag="msg")
                        nc.gpsimd.dma_gather(
                            msg[:, 0:T, :],
                            table.ap()[:, 0:RBF],
                            src_t[:, c * 128 : c * 128 + T * 8],
                            num_idxs=T * 128,
                            num_idxs_reg=T * 128,
                            elem_size=RBF,
                            elem_step=RBF,
                            single_packet=False,
                            queue_num=c % NQ,
                        )
                        maskall = maskp.tile([128, 16, 128], BF16, tag="maskall")
                        if gat:
                            nc.scalar.copy(msg[:, 0:T, 810:811], onesc[:, 0:T, None])
                            sall = smallp.tile([128, 16, 10], F32, tag="sall")
                        for q4 in range(-(-T // 4)):
                            q4n = min(4, T - q4 * 4)
                            jsl = slice(q4 * 4, q4 * 4 + q4n)
                            g4 = c * 16 + q4 * 4
                            nc.vector.tensor_tensor(
                                maskall[:, jsl, :],
                                dstw_t[:, g4 : g4 + q4n, None].to_broadcast(
                                    [128, q4n, 128]
                                ),
                                iota_f[:, None, :].to_broadcast([128, q4n, 128]),
                                op=OP.is_equal,
                            )
                            if not gat:
                                continue
                            trT = psD.tile([128, 512], BF16, space="PSUM", tag="trT")
                            for i in range(q4n):
                                nc.tensor.transpose(
                                    trT[:, i * 128 : (i + 1) * 128],
                                    maskall[:, q4 * 4 + i, :],
                                    ident_bf[:],
                                )
                            maskT = mtp.tile([128, 4, 128], BF16, tag="maskT")
                            nc.scalar.copy(
                                maskT[:, 0:q4n, :],
                                trT[:, 0 : q4n * 128].rearrange(
                                    "p (a b) -> p a b", b=128
                                ),
                            )
                            adx = psS.tile([128, 4, 16], F32, space="PSUM", tag="adx")
                            for i in range(q4n):
                                nc.tensor.matmul(
                                    adx[:, i, 0:10],
                                    lhsT=maskT[:, i, :],
                                    rhs=adw_all[:, (g4 + i) // tpw, :],
                                    start=True,
                                    stop=True,
                                )
                            nc.vector.tensor_tensor(
                                sall[:, jsl, :],
                                msg[:, jsl, 780:800].bitcast(F32),
                                adx[:, 0:q4n, 0:10],
                                op=OP.add,
                            )
                            s2 = smallp.tile([128, 4, 10], F32, tag="s2")
                            nc.vector.tensor_scalar_mul(
                                s2[:, 0:q4n, :], sall[:, jsl, :], 0.2
                            )
                            nc.vector.tensor_tensor(
                                sall[:, jsl, :], sall[:, jsl, :], s2[:, 0:q4n, :],
                                op=OP.max,
                            )
                            nc.scalar.activation(
                                msg[:, jsl, 800:810], sall[:, jsl, :], AF.Exp
                            )
                            nc.vector.tensor_tensor(
                                msg[:, jsl, 0:HID].rearrange(
                                    "p c (h f) -> p c h f", h=H
                                ),
                                msg[:, jsl, 0:HID].rearrange(
                                    "p c (h f) -> p c h f", h=H
                                ),
                                msg[:, jsl, 800:810, None].to_broadcast(
                                    [128, q4n, H, F]
                                ),
                                op=OP.mult,
                            )
                        n_hi = 811 if gat else HID
                        for j in range(T):
                            g = c * 16 + j
                            w = g // tpw
                            first = g % tpw == 0
                            last = g % tpw == tpw - 1
                            if first:
                                aggp = psA.tile(
                                    [128, 1024], F32, space="PSUM", tag="aggp"
                                )
                            for n0, nn in ((0, 512), (512, n_hi - 512)):
                                nc.tensor.matmul(
                                    aggp[:, n0 : n0 + nn],
                                    lhsT=maskall[:, j, :],
                                    rhs=msg[:, j, n0 : n0 + nn],
                                    start=first,
                                    stop=last,
                                )
                            if last:
                                epilogue(w, aggp, epip)

            def gat_epilogue(w, aggp, epip):
                rec = epip.tile([128, 12], F32, tag="rec")
                nc.vector.tensor_scalar_add(rec[:, 0:11], aggp[:, 800:811], 1e-20)
                rcp = epip.tile([128, 12], F32, tag="rcp")
                nc.vector.reciprocal(rcp[:, 0:10], rec[:, 0:10])
                nc.scalar.activation(rcp[:, 10:11], rec[:, 10:11], AF.Sqrt)
                nc.vector.reciprocal(dinv_all[:, w : w + 1], rcp[:, 10:11])
                x1w = epip.tile([128, HID], F32, tag="x1w")
                nc.vector.tensor_tensor(
                    x1w[:].rearrange("p (h f) -> p h f", h=H),
                    aggp[:, 0:HID].rearrange("p (h f) -> p h f", h=H),
                    rcp[:, 0:10, None].to_broadcast([128, H, F]),
                    op=OP.mult,
                )
                nc.vector.tensor_tensor(x1w[:], x1w[:], gatb_bc[:], op=OP.add)
                nc.vector.tensor_scalar_max(x1w[:], x1w[:], 0.0)
                agrow = epip.tile([128, HID], BF16, tag="agrow")
                nc.vector.tensor_tensor(
                    agrow[:],
                    x1w[:],
                    dinv_all[:, w : w + 1].to_broadcast([128, HID]),
                    op=OP.mult,
                )
                nc.sync.dma_start(agin.ap()[w * 128 : (w + 1) * 128, 0:HID], agrow[:])

            edge_phase(htabG, True, gat_epilogue)

            nc.gpsimd.collective_compute(
                "AllGather",
                OP.bypass,
                replica_groups=RG,
                ins=[agin.ap().opt()],
                outs=[htab2G.ap().opt()],
            )

            # ---- fxt matmul (runs during AllGather #2) ----
            with (
                tc.tile_pool(name="fxp", bufs=2) as fxp,
                tc.tile_pool(name="fxw", bufs=2) as fxw,
                tc.tile_pool(name="psX", bufs=1, space="PSUM") as psX,
                tc.tile_pool(name="psXT", bufs=1, space="PSUM") as psXT,
            ):
                xt_ps = psX.tile([BPC, 128], F32, space="PSUM", tag="xtps")
                NR = NF * NBLK  # 288
                for sc in range(NR // 16):
                    wpt = fxw.tile([128, 16, 128], F32, tag="wpt")
                    nc.sync.dma_start(
                        wpt[:],
                        fxtW.ap()[sc * 2048 : (sc + 1) * 2048, :].rearrange(
                            "(c p) j -> p c j", p=128
                        ),
                    )
                    for sub in range(16):
                        r = sc * 16 + sub
                        ch, blk = r // NBLK, r % NBLK
                        nc.tensor.matmul(
                            xt_ps[:, :],
                            lhsT=cT[:, blk, ch, :],
                            rhs=wpt[:, sub, :],
                            start=(r == 0),
                            stop=(r == NR - 1),
                        )
                xt_sb = fxp.tile([BPC, 128], F32, tag="xtsb")
                nc.vector.tensor_copy(xt_sb[:], xt_ps[:])
                xtT_ps = psXT.tile([128, BPC], F32, space="PSUM", tag="xtT")
                nc.tensor.transpose(xtT_ps[:, :], xt_sb[:, :], identf[0:BPC, 0:BPC])
                nc.scalar.activation(
                    xtT_sb[:], xtT_ps[:], AF.Identity, bias=fxtb_t[:, 0:1]
                )
                # preload gcn weights while AG2 is still in flight
                gcnw_sb = pers.tile([128, 7, 784], F32)
                nc.sync.dma_start(
                    gcnw_sb[:], gcnW.ap().rearrange("(c p) f -> p c f", p=128)
                )

            # ---- phase 4: GCN edge phase (aggregate x1*dinv, project, pool) ----
            with (
                tc.tile_pool(name="psP", bufs=1, space="PSUM") as psP,
                tc.tile_pool(name="psTr", bufs=1, space="PSUM") as psTr,
                tc.tile_pool(name="psH", bufs=1, space="PSUM") as psH,
            ):
                poolps = psP.tile([GSLOT, 784], F32, space="PSUM", tag="poolps")

                def gcn_epilogue(w, aggp, epip):
                    aggs = epip.tile([128, HID], F32, tag="aggs")
                    nc.scalar.copy(aggs[:], aggp[:, 0:HID])
                    aT = epip.tile([128, 7, 128], F32, tag="aT")
                    for kc in range(7):
                        sz = 128 if kc < 6 else 12
                        trp = psTr.tile([128, 128], F32, space="PSUM", tag="trp")
                        nc.tensor.transpose(
                            trp[0:sz, :], aggs[:, kc * 128 : kc * 128 + sz],
                            identf[:],
                        )
                        nc.scalar.copy(aT[0:sz, kc, :], trp[0:sz, :])
                    x2w = epip.tile([128, HID], F32, tag="x2w")
                    for n0, nn in ((0, 512), (512, 268)):
                        h2ps = psH.tile([128, 512], F32, space="PSUM", tag="h2ps")
                        for kc in range(7):
                            sz = 128 if kc < 6 else 12
                            nc.tensor.matmul(
                                h2ps[:, 0:nn],
                                lhsT=aT[0:sz, kc, :],
                                rhs=gcnw_sb[0:sz, kc, n0 : n0 + nn],
                                start=(kc == 0),
                                stop=(kc == 6),
                            )
                        x2f = epip.tile([128, 512], F32, tag="x2f")
                        nc.vector.tensor_tensor(
                            x2f[:, 0:nn],
                            h2ps[:, 0:nn],
                            dinv_all[:, w : w + 1].to_broadcast([128, nn]),
                            op=OP.mult,
                        )
                        nc.vector.tensor_tensor(
                            x2f[:, 0:nn], x2f[:, 0:nn], gcnb_bc[:, n0 : n0 + nn],
                            op=OP.add,
                        )
                        nc.vector.tensor_scalar_max(
                            x2w[:, n0 : n0 + nn], x2f[:, 0:nn], 0.0
                        )
                    ph = epip.tile([128, GSLOT], F32, tag="poolhot")
                    nc.vector.tensor_tensor(
                        ph[:],
                        batchw_t[:, w : w + 1].to_broadcast([128, GSLOT]),
                        iota_f[:, 0:GSLOT],
                        op=OP.is_equal,
                    )
                    for n0, nn in ((0, 512), (512, 268)):
                        nc.tensor.matmul(
                            poolps[:, n0 : n0 + nn],
                            lhsT=ph[:],
                            rhs=x2w[:, n0 : n0 + nn],
                            start=(w == 0),
                            stop=(w == NWIN - 1),
                        )

                edge_phase(htab2G, False, gcn_epilogue)
                poolsb = pers.tile([GSLOT, 784], F32)
                nc.any.tensor_copy(poolsb[:, 0:HID], poolps[:, 0:HID])
                nc.gpsimd.memset(poolsb[:, HID:784], 0.0)

            # ---- pool AllGather + transposed dense tail ----
            with (
                tc.tile_pool(name="p5", bufs=1) as p5,
                tc.tile_pool(name="p5w", bufs=2) as p5w,
                tc.tile_pool(name="ps5", bufs=2, space="PSUM") as ps5,
            ):
                nc.sync.dma_start(poolin.ap()[:, :], poolsb[:])
                nc.gpsimd.collective_compute(
                    "AllGather",
                    OP.bypass,
                    replica_groups=RG,
                    ins=[poolin.ap().opt()],
                    outs=[poolall.ap().opt()],
                )
                Cc_sb = p5.tile([128, 4, BPC], F32)
                nc.sync.dma_start(
                    Cc_sb[:], Cc.ap().rearrange("(c p) g -> p c g", p=128)
                )
                # preload head weights (overlaps AG3)
                w1 = p5.tile([128, 7, 1536], F32)
                nc.sync.dma_start(
                    w1[:], fcg1W.ap().rearrange("(c p) f -> p c f", p=128)
                )
                w2 = p5.tile([128, 12, 128], F32)
                nc.sync.dma_start(
                    w2[:], fcg2W.ap().rearrange("(c p) f -> p c f", p=128)
                )
                wf1 = p5.tile([128, 2, 1024], F32)
                nc.sync.dma_start(
                    wf1[:], f1W.ap().rearrange("(c p) f -> p c f", p=128)
                )
                wf2 = p5.tile([128, 8, 512], F32)
                nc.sync.dma_start(
                    wf2[:], f2W.ap().rearrange("(c p) f -> p c f", p=128)
                )
                wf3 = p5.tile([128, 4, 256], F32)
                nc.sync.dma_start(
                    wf3[:], f3W.ap().rearrange("(c p) f -> p c f", p=128)
                )
                wf4 = p5.tile([128, 2, 128], F32)
                nc.sync.dma_start(
                    wf4[:], f4W.ap().rearrange("(c p) f -> p c f", p=128)
                )
                wo = p5.tile([128, 1], F32)
                nc.sync.dma_start(wo[:], oW.ap()[:, :])
                ob_sb = p5.tile([1, 1], F32)
                nc.sync.dma_start(ob_sb[:], o_b.ap()[:, :])
                bcols = {}
                for nm, drm, w_ in (
                    ("fcg1", fcg1b_col, 12), ("fcg2", fcg2b_col, 1),
                    ("f1", f1b_col, 8), ("f2", f2b_col, 4), ("f3", f3b_col, 2),
                    ("f4", f4b_col, 1),
                ):
                    bt = p5.tile([128, w_], F32, tag=f"bc_{nm}")
                    nc.sync.dma_start(bt[:], drm.ap()[:, :])
                    bcols[nm] = bt

                pall = p5.tile([128, 4, 784], F32)
                nc.sync.dma_start(
                    pall[:], poolall.ap().rearrange("(c p) f -> p c f", p=128)
                )
                # xgT[f, g] = sum_slots pall[slot, f] * Cc[slot, g]
                xgT = p5.tile([128, 7, BPC], F32)
                for fc in range(7):
                    sz = 128 if fc < 6 else 12
                    xg_ps = ps5.tile([128, BPC], F32, space="PSUM", tag="mmps")
                    for sc in range(4):
                        nc.tensor.matmul(
                            xg_ps[0:sz, :],
                            lhsT=pall[:, sc, fc * 128 : fc * 128 + sz],
                            rhs=Cc_sb[:, sc, :],
                            start=(sc == 0),
                            stop=(sc == 3),
                        )
                    nc.scalar.copy(xgT[0:sz, fc, :], xg_ps[0:sz, :])

                def dense_T(xT_t, kcs, szs, w_sb, ncs, bname, relu, tag):
                    """yT[n, g] = act(W.T @ x + b): returns [128, ncs, BPC] bf16."""
                    yT = p5.tile([128, ncs, BPC], F32, tag=tag)
                    for n_c in range(ncs):
                        yps = ps5.tile([128, BPC], F32, space="PSUM", tag="mmps")
                        for kc in range(kcs):
                            sz = szs[kc]
                            nc.tensor.matmul(
                                yps[:, :],
                                lhsT=w_sb[0:sz, kc, n_c * 128 : (n_c + 1) * 128],
                                rhs=xT_t[0:sz, kc, :],
                                start=(kc == 0),
                                stop=(kc == kcs - 1),
                            )
                        nc.scalar.activation(
                            yT[:, n_c, :],
                            yps[:, :],
                            AF.Relu if relu else AF.Identity,
                            bias=bcols[bname][:, n_c : n_c + 1],
                        )
                    return yT

                y1 = dense_T(xgT, 7, [128] * 6 + [12], w1, 12, "fcg1", True, "y1")
                xgo = dense_T(y1, 12, [128] * 12, w2, 1, "fcg2", False, "xgo")
                xc = p5.tile([128, 2, BPC], F32, tag="xc")
                nc.any.tensor_copy(xc[:, 0, :], xgo[:, 0, :])
                nc.any.tensor_copy(xc[:, 1, :], xtT_sb[:])
                a1 = dense_T(xc, 2, [128, 128], wf1, 8, "f1", True, "a1")
                a2 = dense_T(a1, 8, [128] * 8, wf2, 4, "f2", True, "a2")
                a3 = dense_T(a2, 4, [128] * 4, wf3, 2, "f3", True, "a3")
                a4 = dense_T(a3, 2, [128, 128], wf4, 1, "f4", True, "a4")
                yo_ps = ps5.tile([1, BPC], F32, space="PSUM", tag="yops")
                nc.tensor.matmul(
                    yo_ps[:, :], lhsT=wo[:, 0:1], rhs=a4[:, 0, :],
                    start=True, stop=True,
                )
                yo = p5.tile([1, BPC], F32, tag="yo")
                nc.scalar.activation(
                    yo[:], yo_ps[:], AF.Identity, bias=ob_sb[:, 0:1]
                )
                nc.sync.dma_start(out_d.ap()[:, :], yo[:])
                if KDEBUG:
                    dbg = p5.tile([128, NWIN, RBF], BF16, tag="dbg")
                    nc.sync.dma_start(
                        dbg[:], agin.ap().rearrange("(c p) f -> p c f", p=128)
                    )
                    nc.sync.dma_start(
                        out_ag.ap().rearrange("(c p) f -> p c f", p=128), dbg[:]
                    )
                    psb2 = p5.tile([GSLOT, 784], F32, tag="psb2")
                    nc.any.tensor_copy(psb2[:], poolsb[:])
                    nc.sync.dma_start(out_pool.ap()[:, :], psb2[:])
                    xt2 = p5.tile([128, BPC], F32, tag="xt2")
                    nc.any.tensor_copy(xt2[:], xtT_sb[:])
                    nc.sync.dma_start(out_xt.ap()[:, :], xt2[:])
                    xg2 = p5.tile([128, 7, BPC], F32, tag="xg2")
                    nc.any.tensor_copy(xg2[:], xgT[:])
                    nc.sync.dma_start(
                        out_xg.ap().rearrange("p (a b) -> p a b", b=BPC), xg2[:]
                    )

    nc.compile()
    _CACHE[key] = nc
    return nc


# ---------------------------------------------------------------- entry point


def _ensure_ntff_hook():
    """Install antenv.axon_hooks + register the ctypes NTFF hook if the image
    lacks them (profiling only; failures are non-fatal)."""
    import types

    try:
        import antenv.axon_hooks  # noqa: F401

        if antenv.axon_hooks.get_axon_ntff_profile_hook() is not None:
            return
    except ImportError:
        import antenv

        mod = types.ModuleType("antenv.axon_hooks")
        mod._hook = None

        def set_axon_ntff_profile_hook(h, _m=mod):
            _m._hook = h

        def get_axon_ntff_profile_hook(_m=mod):
            return _m._hook

        mod.set_axon_ntff_profile_hook = set_axon_ntff_profile_hook
        mod.get_axon_ntff_profile_hook = get_axon_ntff_profile_hook
        sys.modules["antenv.axon_hooks"] = mod
        antenv.axon_hooks = mod
    try:
        from antenv.axon_hooks import set_axon_ntff_profile_hook as _set
        from trn_agent_boot.trn_boot import _ntff_profile_via_ctypes

        hook = _ntff_profile_via_ctypes("/opt/axon/libaxon_pjrt.so")
        if hook is not None:
            _set(hook)
    except Exception:
        pass


def kernel(**inputs) -> np.ndarray:
    per_core, meta = host_prep(inputs)
    nc = build_bass(meta)
    in_maps = [{k: np.ascontiguousarray(v) for k, v in d.items()} for d in per_core]
    trace = bool(int(os.environ.get("KERNEL_TRACE", "0")))
    if trace:
        _ensure_ntff_hook()
    res = run_bass_kernel_spmd(nc, in_maps, core_ids=list(range(NCORES)), trace=trace)
    if trace and res.exec_time_ns is not None:
        print(f"HW exec time: {res.exec_time_ns} ns")
        kernel.last_exec_ns = res.exec_time_ns
    out = np.concatenate(
        [res.results[c]["out"][0, :BPC, None] for c in range(NCORES)], 0
    )
    return out.astype(np.float32)


# revision 29
# speedup vs baseline: 1.3286x; 1.0824x over previous
"""Trainium2 Bass kernel for nn_EnhancedGATGCN (GAT -> GCN -> pool -> MLP, + protein conv branch).

Self-contained: host-side sharding prep + 8-core SPMD Bass/Tile device program.

v2 design (all sizes bf16 unless noted):
  - Edges sorted by dst, 8-way dst-sharded; scatter-add via one-hot mask matmuls
    into per-128-dst-window PSUM. tpw exact (no round-to-4). Row tables gathered
    via SWDGE dma_gather alternating across queues; nothing else runs on the
    Pool engine during edge phases (drain-rate paced).
  - GCN aggregates dinv*x1 rows directly (aggregate-then-project): the x1@W
    projection happens per dst window in the GCN epilogue; phase 3 eliminated.
  - Protein branch: conv1d computed as extended-one-hot matmuls (no embedding
    gather): OHx[26*t+v, q] = (tok[s+q+t]==v), C_blk = OHx.T @ Vx with
    Vx[26t+v,:] = emb[v] @ cW[:,:,t].T host-precomputed; 121 valid positions
    per 128-token block. conv bias folded into fxt bias on host. Runs during
    AllGather #1; the fxt matmul runs during AllGather #2.
  - Dense tail kept transposed ([feat, graph]) so no per-layer transposes;
    per-partition column biases.
"""
import os
import sys

import numpy as np

sys.path.insert(0, "/opt/trn_rl_repo")

import ml_dtypes

import concourse.bacc as bacc
import concourse.bass as bass
import concourse.mybir as mybir
import concourse.tile as tile
from concourse.bass_utils import run_bass_kernel_spmd
from concourse.masks import make_identity

F32 = mybir.dt.float32
BF16 = mybir.dt.bfloat16
I16 = mybir.dt.int16
I32 = mybir.dt.int32
F16 = mybir.dt.float16
AF = mybir.ActivationFunctionType
OP = mybir.AluOpType
BF = ml_dtypes.bfloat16

N, E, B, H, F = 20000, 400000, 200, 10, 78
HID = H * F  # 780
SEQ, VOC, EMB, NF, KS = 1000, 26, 128, 26 * 0 + 32, 8
CONV_OUT = SEQ - KS + 1  # 993

NCORES = 8
NPC = N // NCORES  # 2500
NPAD = 2560
NWIN = NPAD // 128  # 20
RBF = 896  # bf16 cols per table row; 1792 B/row (%256==0)
# GAT msg row: [0:780 h | 780:800 a_s 10xf32-packed | 800:810 exd | 810 ones | 811:896 junk]
GSLOT = 64
BPC = B // NCORES  # 25
NBLK = 9  # conv position blocks per graph (121 valid pos each)
TOKB = 136  # tokens shipped per block (121 + 7 tap overlap + pad)
TOKP = NBLK * TOKB  # 1224
NQ = int(os.environ.get("KNQ", "2"))  # swdge queues


# ---------------------------------------------------------------- host prep


def _wrap16(idx, epc):
    a = np.zeros((128, epc // 16), np.int16)
    w = idx.reshape(epc // 16, 16).T
    a[:, :] = np.tile(w, (8, 1))
    return a


def host_prep(inputs):
    x = np.asarray(inputs["x"], np.float32)
    edge_index = np.asarray(inputs["edge_index"], np.int64)
    batch = np.asarray(inputs["batch"], np.int64)
    target = np.asarray(inputs["target"], np.int64)

    loops = np.arange(N, dtype=np.int64)
    src = np.concatenate([edge_index[0], loops])
    dst = np.concatenate([edge_index[1], loops])
    order = np.argsort(dst, kind="stable")
    src, dst = src[order], dst[order]

    core_of = dst // NPC
    dst_local = dst - core_of * NPC
    win = dst_local // 128
    maxw = 0
    per_core_edges = []
    for c in range(NCORES):
        m = core_of == c
        s_c, dl_c, w_c = src[m], dst_local[m], win[m]
        per_core_edges.append((s_c, dl_c, w_c))
        maxw = max(maxw, int(np.bincount(w_c, minlength=NWIN).max()))
    tpw = -(-maxw // 128)
    ntile = NWIN * tpw
    epc = ntile * 128
    nchunk = -(-ntile // 16)

    def remap(n):
        return (n // NPC) * NPAD + (n % NPC)

    cores = []
    for c in range(NCORES):
        s_c, dl_c, w_c = per_core_edges[c]
        es = np.zeros(epc, np.int64)
        ew = np.full(epc, -1000.0, np.float32)
        for w in range(NWIN):
            m = w_c == w
            k = int(m.sum())
            o = w * tpw * 128
            es[o : o + k] = s_c[m]
            ew[o : o + k] = (dl_c[m] - w * 128).astype(np.float32)
        cores.append(dict(es=remap(es), ew=ew))

    # GAT weight pack: [0:780 W | 780:790 W@a_src per head | 790:800 W@a_dst]
    gat_W = np.asarray(inputs["gat_W"], np.float32)
    a_src = np.asarray(inputs["gat_a_src"], np.float32)
    a_dst = np.asarray(inputs["gat_a_dst"], np.float32)
    # fh-interleaved feature layout: col f*H+h <- head-major col h*F+f
    perm = np.arange(HID).reshape(78, 10)
    perm = (perm % 10) * F + (perm // 10) % F  # perm[f*10+h] = h*78+f
    perm = np.array([(c % 10) * F + c // 10 for c in range(HID)])
    wpack = np.zeros((78, 1024), np.float32)
    wpack[:, :HID] = gat_W[:, perm]
    for h in range(H):
        wpack[:, HID + h] = gat_W[:, h * F : (h + 1) * F] @ a_src[h]
        wpack[:, HID + 10 + h] = gat_W[:, h * F : (h + 1) * F] @ a_dst[h]

    gcn_W_pad = np.zeros((896, 784), np.float32)
    gcn_W_pad[:HID, :HID] = np.asarray(inputs["gcn_W"], np.float32)[perm, :]

    # protein: Vx tables + reordered fxt weights (+ conv bias folded into fxt_b)
    emb = np.asarray(inputs["emb"], np.float32)
    cW = np.asarray(inputs["cW"], np.float32)  # [NF, EMB, KS]
    cb = np.asarray(inputs["cb"], np.float32)
    # tap t occupies 32-partition-aligned row group (t%4)*32; rows 26-31 zero
    Vx = np.zeros((KS * 32, NF), np.float32)
    for t in range(KS):
        Vx[t * 32 : t * 32 + VOC] = emb @ cW[:, :, t].T
    fxt_W = np.asarray(inputs["fxt_W"], np.float32)  # [NF*993, 128]
    fxt_b = np.asarray(inputs["fxt_b"], np.float32)
    fxt_b2 = fxt_b + cb @ fxt_W.reshape(NF, CONV_OUT, 128).sum(axis=1)
    fxtW = np.zeros((NF, NBLK, 128, 128), np.float32)
    for blk in range(NBLK):
        s = blk * 121
        n = min(121, CONV_OUT - s)
        fxtW[:, blk, :n] = fxt_W.reshape(NF, CONV_OUT, 128)[:, s : s + n]
    fxtW = fxtW.reshape(NF * NBLK * 128, 128)

    # head weights (transposed-chain layout, bf16) + column biases
    def colbias(b, n):
        nc_ = -(-n // 128)
        col = np.zeros((nc_, 128), np.float32)
        col.reshape(-1)[: len(b)] = b
        return col.T.copy()

    fcg1_W = np.zeros((896, 1536), np.float32)
    fcg1_W[:HID, :1500] = np.asarray(inputs["fcg1_W"], np.float32)
    fcg2_W = np.zeros((1536, 128), np.float32)
    fcg2_W[:1500] = np.asarray(inputs["fcg2_W"], np.float32)

    # graph slot bookkeeping
    gbase = np.array([batch[c * NPC] for c in range(NCORES)], np.int64)
    span = np.array(
        [batch[min(c * NPC + NPC, N) - 1] - gbase[c] + 1 for c in range(NCORES)]
    )
    assert span.max() <= GSLOT, span.max()
    Cc_all = []
    for c in range(NCORES):
        Cmat = np.zeros((NCORES * GSLOT, BPC), np.float32)
        for r in range(NCORES):
            for slot in range(GSLOT):
                g = gbase[r] + slot
                col = g - c * BPC
                if 0 <= col < BPC and g < B:
                    Cmat[r * GSLOT + slot, col] = 1.0
        Cc_all.append(Cmat)

    vmod = np.full((128, 1), -2.0, np.float16)
    for gi in range(4):
        vmod[gi * 32 : gi * 32 + VOC, 0] = np.arange(VOC)

    RT = -(-ntile // 128)
    meta = dict(tpw=tpw, ntile=ntile, epc=epc, nchunk=nchunk, RT=RT)

    per_core = []
    for c in range(NCORES):
        ed_ = cores[c]
        bw = np.full(NPAD, -1000.0, np.float32)
        bw[:NPC] = (batch[c * NPC : (c + 1) * NPC] - gbase[c]).astype(np.float32)
        batchw = bw.reshape(NWIN, 128).T.copy()

        dstw = ed_["ew"].reshape(ntile, 128).T.copy()  # [128, ntile]

        xTc = np.zeros((78, NPAD), np.float32)
        xTc[:, :NPC] = x[c * NPC : (c + 1) * NPC].T

        tg = target[c * BPC : (c + 1) * BPC].astype(np.float32)
        tokba = np.full((128, BPC, TOKP), -1.0, np.float16)
        tokbb = np.full((128, BPC, TOKP), -1.0, np.float16)
        for p in range(128):
            for sh, tob in ((p // 32, tokba), (4 + p // 32, tokbb)):
                for blk in range(NBLK):
                    s0 = blk * 121 + sh
                    n = max(0, min(TOKB, SEQ - s0))
                    tob[p, :, blk * TOKB : blk * TOKB + n] = tg[:, s0 : s0 + n]

        d = {
            "xTc": xTc,
            "wpack": wpack,
            "src16": _wrap16(ed_["es"], epc),
            "dstw": dstw,
            "batchw": batchw,
            "vmod": vmod,
            "tokba": tokba, "tokbb": tokbb,
            "Vxa": Vx[:128].astype(np.float16),
            "Vxb": Vx[128:].astype(np.float16),
            "fxtW": fxtW,
            "fxtb_col": colbias(fxt_b2, 128),
            "gat_b": np.asarray(inputs["gat_b"], np.float32)[perm].reshape(1, HID),
            "gcnW": gcn_W_pad,
            "gcn_b": np.asarray(inputs["gcn_b"], np.float32).reshape(1, HID),
            "fcg1W": fcg1_W,
            "fcg1b_col": colbias(np.asarray(inputs["fcg1_b"], np.float32), 1536),
            "fcg2W": fcg2_W,
            "fcg2b_col": colbias(np.asarray(inputs["fcg2_b"], np.float32), 128),
            "f1W": np.asarray(inputs["f1_W"], np.float32),
            "f1b_col": colbias(np.asarray(inputs["f1_b"], np.float32), 1024),
            "f2W": np.asarray(inputs["f2_W"], np.float32),
            "f2b_col": colbias(np.asarray(inputs["f2_b"], np.float32), 512),
            "f3W": np.asarray(inputs["f3_W"], np.float32),
            "f3b_col": colbias(np.asarray(inputs["f3_b"], np.float32), 256),
            "f4W": np.asarray(inputs["f4_W"], np.float32),
            "f4b_col": colbias(np.asarray(inputs["f4_b"], np.float32), 128),
            "oW": np.asarray(inputs["o_W"], np.float32),
            "o_b": np.asarray(inputs["o_b"], np.float32).reshape(1, 1),
            "Cc": Cc_all[c],
        }
        per_core.append(d)
    return per_core, meta


# ---------------------------------------------------------------- device build

_CACHE = {}


def build_bass(meta):
    key = (meta["tpw"], NQ)
    if key in _CACHE:
        return _CACHE[key]

    tpw, ntile, epc, nchunk, RT = (
        meta["tpw"], meta["ntile"], meta["epc"], meta["nchunk"], meta["RT"],
    )

    nc = bacc.Bacc(
        "TRN2",
        target_bir_lowering=False,
        debug=False,
        num_devices=NCORES,
        num_swdge_queues=NQ,
    )

    def inp(name, shape, dt=F32):
        return nc.dram_tensor(name, list(shape), dt, kind="ExternalInput")

    xTc = inp("xTc", (78, NPAD))
    wpack = inp("wpack", (78, 1024))
    src16 = inp("src16", (128, epc // 16), I16)
    dstw = inp("dstw", (128, ntile))
    batchw = inp("batchw", (128, NWIN))
    vmod = inp("vmod", (128, 1), F16)
    tokba = inp("tokba", (128, BPC, TOKP), F16)
    tokbb = inp("tokbb", (128, BPC, TOKP), F16)
    Vxa = inp("Vxa", (128, NF), F16)
    Vxb = inp("Vxb", (128, NF), F16)
    fxtW = inp("fxtW", (NF * NBLK * 128, 128))
    fxtb_col = inp("fxtb_col", (128, 1))
    gat_b = inp("gat_b", (1, HID))
    gcnW = inp("gcnW", (896, 784))
    gcn_b = inp("gcn_b", (1, HID))
    fcg1W = inp("fcg1W", (896, 1536))
    fcg1b_col = inp("fcg1b_col", (128, 12))
    fcg2W = inp("fcg2W", (1536, 128))
    fcg2b_col = inp("fcg2b_col", (128, 1))
    f1W = inp("f1W", (256, 1024))
    f1b_col = inp("f1b_col", (128, 8))
    f2W = inp("f2W", (1024, 512))
    f2b_col = inp("f2b_col", (128, 4))
    f3W = inp("f3W", (512, 256))
    f3b_col = inp("f3b_col", (128, 2))
    f4W = inp("f4W", (256, 128))
    f4b_col = inp("f4b_col", (128, 1))
    oW = inp("oW", (128, 1))
    o_b = inp("o_b", (1, 1))
    Cc = inp("Cc", (NCORES * GSLOT, BPC))

    out_d = nc.dram_tensor("out", [1, BPC], F32, kind="ExternalOutput")
    KDEBUG = bool(int(os.environ.get("KDEBUG", "0")))
    if KDEBUG:
        out_ag = nc.dram_tensor("out_ag", [NPAD, RBF], BF16, kind="ExternalOutput")
        out_pool = nc.dram_tensor("out_pool", [GSLOT, 784], F32, kind="ExternalOutput")
        out_xt = nc.dram_tensor("out_xt", [128, BPC], F32, kind="ExternalOutput")
        out_xg = nc.dram_tensor("out_xg", [128, 7 * BPC], F32, kind="ExternalOutput")

    hin = nc.dram_tensor("hin", [NPAD, RBF], BF16)
    htabG = nc.dram_tensor("htabG", [NCORES * NPAD, RBF], BF16, addr_space="Shared")
    agin = nc.dram_tensor("agin", [NPAD, RBF], BF16)
    htab2G = nc.dram_tensor("htab2G", [NCORES * NPAD, RBF], BF16, addr_space="Shared")
    poolin = nc.dram_tensor("poolin", [GSLOT, 784], F32)
    poolall = nc.dram_tensor("poolall", [NCORES * GSLOT, 784], F32, addr_space="Shared")

    RG = [list(range(NCORES))]

    with tile.TileContext(nc) as tc:
        import contextlib

        ctx = contextlib.ExitStack()
        with ctx:
            pers = ctx.enter_context(tc.tile_pool(name="pers", bufs=1))

            # consts
            iota_i = pers.tile([128, 128], I32)
            nc.gpsimd.iota(iota_i[:], pattern=[[1, 128]], base=0, channel_multiplier=0)
            iota_f = pers.tile([128, 128], F32)
            nc.vector.tensor_copy(iota_f[:], iota_i[:])
            ident_bf = pers.tile([128, 128], BF16)
            identf = pers.tile([128, 128], F32)
            make_identity(nc, identf[:])
            nc.vector.tensor_copy(ident_bf[:], identf[:])
            ones1 = pers.tile([1, 128], F32)
            nc.gpsimd.memset(ones1[:], 1.0)
            onesc = pers.tile([128, 16], BF16)
            nc.gpsimd.memset(onesc[:], 1.0)

            # residents
            dstw_t = pers.tile([128, ntile], F32)
            nc.sync.dma_start(dstw_t[:], dstw[:, :])
            batchw_t = pers.tile([128, NWIN], F32)
            nc.sync.dma_start(batchw_t[:], batchw[:, :])
            src_t = pers.tile([128, epc // 16], I16)
            nc.sync.dma_start(src_t[:], src16[:, :])
            vmod_t = pers.tile([128, 1], F16)
            nc.sync.dma_start(vmod_t[:], vmod[:, :])
            Vxa_t = pers.tile([128, NF], F16)
            nc.sync.dma_start(Vxa_t[:], Vxa[:, :])
            Vxb_t = pers.tile([128, NF], F16)
            nc.sync.dma_start(Vxb_t[:], Vxb[:, :])
            fxtb_t = pers.tile([128, 1], F32)
            nc.sync.dma_start(fxtb_t[:], fxtb_col[:, :])

            dinv_all = pers.tile([128, NWIN], F32)
            adw_all = pers.tile([128, NWIN, 10], BF16)
            cT = pers.tile([128, NBLK, NF, BPC], F32)
            xtT_sb = pers.tile([128, BPC], F32)

            # broadcast biases (row-replicated tiles)
            bias_tiles = {}
            with tc.tile_pool(name="psB", bufs=1, space="PSUM") as psB:

                def bcast_bias(dram, width, name):
                    t = pers.tile([128, width], F32, tag=f"bc_{name}")
                    row = pers.tile([1, width], F32, tag=f"br_{name}")
                    nc.sync.dma_start(row[:], dram[0:1, :])
                    for n0 in range(0, width, 512):
                        nn = min(512, width - n0)
                        ps = psB.tile([128, 512], F32, space="PSUM", tag="bcps")
                        nc.tensor.matmul(
                            ps[:, :nn], lhsT=ones1[:], rhs=row[:, n0 : n0 + nn],
                            start=True, stop=True,
                        )
                        nc.any.tensor_copy(t[:, n0 : n0 + nn], ps[:, :nn])
                    return t

                gatb_bc = bcast_bias(gat_b, HID, "gatb")
                gcnb_bc = bcast_bias(gcn_b, HID, "gcnb")

            # ---- phase 1: own h rows ----
            with (
                tc.tile_pool(name="p1", bufs=1) as p1,
                tc.tile_pool(name="p1h", bufs=3) as p1h,
                tc.tile_pool(name="ps1", bufs=2, space="PSUM") as ps1,
            ):
                xT_sb = p1.tile([78, NPAD], F32)
                nc.sync.dma_start(xT_sb[:], xTc[:, :])
                wp_sb = p1.tile([78, 1024], F32)
                nc.sync.dma_start(wp_sb[:], wpack[:, :])
                for t in range(NWIN):
                    hp = ps1.tile([128, 1024], F32, space="PSUM", tag="hp")
                    for n0 in (0, 512):
                        nc.tensor.matmul(
                            hp[:, n0 : n0 + 512],
                            lhsT=xT_sb[:, t * 128 : (t + 1) * 128],
                            rhs=wp_sb[:, n0 : n0 + 512],
                            start=True,
                            stop=True,
                        )
                    hrow = p1h.tile([128, 800], BF16, tag="hrow")
                    nc.vector.tensor_copy(hrow[:, 0:HID], hp[:, 0:HID])
                    nc.vector.tensor_copy(
                        hrow[:, 780:800].bitcast(F32), hp[:, 780:790]
                    )
                    nc.vector.tensor_copy(adw_all[:, t, :], hp[:, 790:800])
                    nc.sync.dma_start(
                        hin.ap()[t * 128 : (t + 1) * 128, 0:800], hrow[:]
                    )
                nc.gpsimd.collective_compute(
                    "AllGather",
                    OP.bypass,
                    replica_groups=RG,
                    ins=[hin.ap().opt()],
                    outs=[htabG.ap().opt()],
                )

            # ---- protein conv (runs during AllGather #1; no graph deps) ----
            ppo = ctx.enter_context(tc.tile_pool(name="ppo", bufs=2))
            ppt = ctx.enter_context(tc.tile_pool(name="ppt", bufs=3))
            with tc.tile_pool(name="psCq", bufs=2, space="PSUM") as psCq:
                for g in range(BPC):
                    tokrA = ppt.tile([128, TOKP], F16, tag="tokrA")
                    nc.sync.dma_start(tokrA[:], tokba.ap()[:, g, :])
                    tokrB = ppt.tile([128, TOKP], F16, tag="tokrB")
                    nc.sync.dma_start(tokrB[:], tokbb.ap()[:, g, :])
                    OHa = ppo.tile([128, NBLK, 128], F16, tag="OHa")
                    OHb = ppo.tile([128, NBLK, 128], F16, tag="OHb")
                    for tok, OH in ((tokrA, OHa), (tokrB, OHb)):
                        nc.vector.tensor_tensor(
                            OH[:],
                            tok.rearrange("p (b q) -> p b q", q=TOKB)[:, :, 0:128],
                            vmod_t[:, :, None].to_broadcast([128, NBLK, 128]),
                            op=OP.is_equal,
                        )
                    Cq = psCq.tile([128, NBLK, NF], F32, space="PSUM", tag="Cq")
                    for blk in range(NBLK):
                        nc.tensor.matmul(
                            Cq[:, blk, :], lhsT=OHa[:, blk, :], rhs=Vxa_t[:],
                            start=True, stop=False,
                        )
                        nc.tensor.matmul(
                            Cq[:, blk, :], lhsT=OHb[:, blk, :], rhs=Vxb_t[:],
                            start=False, stop=True,
                        )
                    nc.scalar.copy(cT[:, :, :, g], Cq[:, :, :])

            # ---- phase 2: GAT edge phase ----
            def edge_phase(table, gat, epilogue):
                with (
                    tc.tile_pool(name="msgp", bufs=2) as msgp,
                    tc.tile_pool(name="maskp", bufs=3) as maskp,
                    tc.tile_pool(name="mtp", bufs=2) as mtp,
                    tc.tile_pool(name="smallp", bufs=2) as smallp,
                    tc.tile_pool(name="epip", bufs=2) as epip,
                    tc.tile_pool(name="psA", bufs=2, space="PSUM") as psA,
                    tc.tile_pool(name="psS", bufs=2, space="PSUM") as psS,
                    tc.tile_pool(name="psD", bufs=2, space="PSUM") as psD,
                ):
                    aggp = None
                    for c in range(nchunk):
                        T = min(16, ntile - c * 16)
                        msg = msgp.tile([128, 16, RBF], BF16, tag="msg")
                        nc.gpsimd.dma_gather(
                            msg[:, 0:T, :],
                            table.ap()[:, 0:RBF],
                            src_t[:, c * 128 : c * 128 + T * 8],
                            num_idxs=T * 128,
                            num_idxs_reg=T * 128,
                            elem_size=RBF,
                            elem_step=RBF,
                            single_packet=False,
                            queue_num=c % NQ,
                        )
                        maskall = maskp.tile([128, 16, 128], BF16, tag="maskall")
                        if gat:
                            nc.scalar.copy(msg[:, 0:T, 810:811], onesc[:, 0:T, None])
                            sall = smallp.tile([128, 16, 10], F32, tag="sall")
                        for q4 in range(-(-T // 4)):
                            q4n = min(4, T - q4 * 4)
                            jsl = slice(q4 * 4, q4 * 4 + q4n)
                            g4 = c * 16 + q4 * 4
                            nc.vector.tensor_tensor(
                                maskall[:, jsl, :],
                                dstw_t[:, g4 : g4 + q4n, None].to_broadcast(
                                    [128, q4n, 128]
                                ),
                                iota_f[:, None, :].to_broadcast([128, q4n, 128]),
                                op=OP.is_equal,
                            )
                            if not gat:
                                continue
                            trT = psD.tile([128, 512], BF16, space="PSUM", tag="trT")
                            for i in range(q4n):
                                nc.tensor.transpose(
                                    trT[:, i * 128 : (i + 1) * 128],
                                    maskall[:, q4 * 4 + i, :],
                                    ident_bf[:],
                                )
                            maskT = mtp.tile([128, 4, 128], BF16, tag="maskT")
                            nc.scalar.copy(
                                maskT[:, 0:q4n, :],
                                trT[:, 0 : q4n * 128].rearrange(
                                    "p (a b) -> p a b", b=128
                                ),
                            )
                            adx = psS.tile([128, 4, 16], F32, space="PSUM", tag="adx")
                            for i in range(q4n):
                                nc.tensor.matmul(
                                    adx[:, i, 0:10],
                                    lhsT=maskT[:, i, :],
                                    rhs=adw_all[:, (g4 + i) // tpw, :],
                                    start=True,
                                    stop=True,
                                )
                            nc.vector.tensor_tensor(
                                sall[:, jsl, :],
                                msg[:, jsl, 780:800].bitcast(F32),
                                adx[:, 0:q4n, 0:10],
                                op=OP.add,
                            )
                            s2 = smallp.tile([128, 4, 10], F32, tag="s2")
                            nc.vector.tensor_scalar_mul(
                                s2[:, 0:q4n, :], sall[:, jsl, :], 0.2
                            )
                            nc.vector.tensor_tensor(
                                sall[:, jsl, :], sall[:, jsl, :], s2[:, 0:q4n, :],
                                op=OP.max,
                            )
                            nc.scalar.activation(
                                msg[:, jsl, 800:810], sall[:, jsl, :], AF.Exp
                            )
                            nc.vector.tensor_tensor(
                                msg[:, jsl, 0:HID].rearrange(
                                    "p c (f h) -> p c f h", h=H
                                ),
                                msg[:, jsl, 0:HID].rearrange(
                                    "p c (f h) -> p c f h", h=H
                                ),
                                msg[:, jsl, None, 800:810].to_broadcast(
                                    [128, q4n, F, H]
                                ),
                                op=OP.mult,
                            )
                        n_hi = 811 if gat else HID
                        for j in range(T):
                            g = c * 16 + j
                            w = g // tpw
                            first = g % tpw == 0
                            last = g % tpw == tpw - 1
                            if first:
                                aggp = psA.tile(
                                    [128, 1024], F32, space="PSUM", tag="aggp"
                                )
                            for n0, nn in ((0, 512), (512, n_hi - 512)):
                                nc.tensor.matmul(
                                    aggp[:, n0 : n0 + nn],
                                    lhsT=maskall[:, j, :],
                                    rhs=msg[:, j, n0 : n0 + nn],
                                    start=first,
                                    stop=last,
                                )
                            if last:
                                epilogue(w, aggp, epip)

            def gat_epilogue(w, aggp, epip):
                rec = epip.tile([128, 12], F32, tag="rec")
                nc.vector.tensor_scalar_add(rec[:, 0:11], aggp[:, 800:811], 1e-20)
                rcp = epip.tile([128, 12], F32, tag="rcp")
                nc.vector.reciprocal(rcp[:, 0:10], rec[:, 0:10])
                nc.scalar.activation(rcp[:, 10:11], rec[:, 10:11], AF.Sqrt)
                nc.vector.reciprocal(dinv_all[:, w : w + 1], rcp[:, 10:11])
                x1w = epip.tile([128, HID], F32, tag="x1w")
                nc.vector.tensor_tensor(
                    x1w[:].rearrange("p (f h) -> p f h", h=H),
                    aggp[:, 0:HID].rearrange("p (f h) -> p f h", h=H),
                    rcp[:, None, 0:10].to_broadcast([128, F, H]),
                    op=OP.mult,
                )
                nc.vector.tensor_tensor(x1w[:], x1w[:], gatb_bc[:], op=OP.add)
                agrow = epip.tile([128, HID], BF16, tag="agrow")
                nc.scalar.activation(
                    agrow[:], x1w[:], AF.Relu, scale=dinv_all[:, w : w + 1]
                )
                nc.sync.dma_start(agin.ap()[w * 128 : (w + 1) * 128, 0:HID], agrow[:])

            edge_phase(htabG, True, gat_epilogue)

            nc.gpsimd.collective_compute(
                "AllGather",
                OP.bypass,
                replica_groups=RG,
                ins=[agin.ap().opt()],
                outs=[htab2G.ap().opt()],
            )

            # ---- fxt matmul (runs during AllGather #2) ----
            with (
                tc.tile_pool(name="fxp", bufs=2) as fxp,
                tc.tile_pool(name="fxw", bufs=2) as fxw,
                tc.tile_pool(name="psX", bufs=1, space="PSUM") as psX,
                tc.tile_pool(name="psXT", bufs=1, space="PSUM") as psXT,
            ):
                xt_ps = psX.tile([BPC, 128], F32, space="PSUM", tag="xtps")
                NR = NF * NBLK  # 288
                for sc in range(NR // 16):
                    wpt = fxw.tile([128, 16, 128], F32, tag="wpt")
                    nc.sync.dma_start(
                        wpt[:],
                        fxtW.ap()[sc * 2048 : (sc + 1) * 2048, :].rearrange(
                            "(c p) j -> p c j", p=128
                        ),
                    )
                    for sub in range(16):
                        r = sc * 16 + sub
                        ch, blk = r // NBLK, r % NBLK
                        nc.tensor.matmul(
                            xt_ps[:, :],
                            lhsT=cT[:, blk, ch, :],
                            rhs=wpt[:, sub, :],
                            start=(r == 0),
                            stop=(r == NR - 1),
                        )
                xt_sb = fxp.tile([BPC, 128], F32, tag="xtsb")
                nc.vector.tensor_copy(xt_sb[:], xt_ps[:])
                xtT_ps = psXT.tile([128, BPC], F32, space="PSUM", tag="xtT")
                nc.tensor.transpose(xtT_ps[:, :], xt_sb[:, :], identf[0:BPC, 0:BPC])
                nc.scalar.activation(
                    xtT_sb[:], xtT_ps[:], AF.Identity, bias=fxtb_t[:, 0:1]
                )
                # preload gcn weights while AG2 is still in flight
                gcnw_sb = pers.tile([128, 7, 784], F32)
                nc.sync.dma_start(
                    gcnw_sb[:], gcnW.ap().rearrange("(c p) f -> p c f", p=128)
                )

            # ---- phase 4: GCN edge phase (aggregate x1*dinv, project, pool) ----
            with (
                tc.tile_pool(name="psP", bufs=1, space="PSUM") as psP,
                tc.tile_pool(name="psTr", bufs=1, space="PSUM") as psTr,
                tc.tile_pool(name="psH", bufs=1, space="PSUM") as psH,
            ):
                poolps = psP.tile([GSLOT, 784], F32, space="PSUM", tag="poolps")

                def gcn_epilogue(w, aggp, epip):
                    aggs = epip.tile([128, HID], F32, tag="aggs")
                    nc.scalar.copy(aggs[:], aggp[:, 0:HID])
                    aT = epip.tile([128, 7, 128], F32, tag="aT")
                    for kc in range(7):
                        sz = 128 if kc < 6 else 12
                        trp = psTr.tile([128, 128], F32, space="PSUM", tag="trp")
                        nc.tensor.transpose(
                            trp[0:sz, :], aggs[:, kc * 128 : kc * 128 + sz],
                            identf[:],
                        )
                        nc.scalar.copy(aT[0:sz, kc, :], trp[0:sz, :])
                    x2w = epip.tile([128, HID], F32, tag="x2w")
                    for n0, nn in ((0, 512), (512, 268)):
                        h2ps = psH.tile([128, 512], F32, space="PSUM", tag="h2ps")
                        for kc in range(7):
                            sz = 128 if kc < 6 else 12
                            nc.tensor.matmul(
                                h2ps[:, 0:nn],
                                lhsT=aT[0:sz, kc, :],
                                rhs=gcnw_sb[0:sz, kc, n0 : n0 + nn],
                                start=(kc == 0),
                                stop=(kc == 6),
                            )
                        x2f = epip.tile([128, 512], F32, tag="x2f")
                        nc.scalar.activation(
                            x2f[:, 0:nn], h2ps[:, 0:nn], AF.Identity,
                            scale=dinv_all[:, w : w + 1],
                        )
                        nc.vector.tensor_tensor(
                            x2f[:, 0:nn], x2f[:, 0:nn], gcnb_bc[:, n0 : n0 + nn],
                            op=OP.add,
                        )
                        nc.scalar.activation(
                            x2w[:, n0 : n0 + nn], x2f[:, 0:nn], AF.Relu
                        )
                    ph = epip.tile([128, GSLOT], F32, tag="poolhot")
                    nc.vector.tensor_tensor(
                        ph[:],
                        batchw_t[:, w : w + 1].to_broadcast([128, GSLOT]),
                        iota_f[:, 0:GSLOT],
                        op=OP.is_equal,
                    )
                    for n0, nn in ((0, 512), (512, 268)):
                        nc.tensor.matmul(
                            poolps[:, n0 : n0 + nn],
                            lhsT=ph[:],
                            rhs=x2w[:, n0 : n0 + nn],
                            start=(w == 0),
                            stop=(w == NWIN - 1),
                        )

                edge_phase(htab2G, False, gcn_epilogue)
                poolsb = pers.tile([GSLOT, 784], F32)
                nc.any.tensor_copy(poolsb[:, 0:HID], poolps[:, 0:HID])
                nc.gpsimd.memset(poolsb[:, HID:784], 0.0)

            # ---- pool AllGather + transposed dense tail ----
            with (
                tc.tile_pool(name="p5", bufs=1) as p5,
                tc.tile_pool(name="p5w", bufs=2) as p5w,
                tc.tile_pool(name="ps5", bufs=2, space="PSUM") as ps5,
            ):
                nc.sync.dma_start(poolin.ap()[:, :], poolsb[:])
                nc.gpsimd.collective_compute(
                    "AllGather",
                    OP.bypass,
                    replica_groups=RG,
                    ins=[poolin.ap().opt()],
                    outs=[poolall.ap().opt()],
                )
                Cc_sb = p5.tile([128, 4, BPC], F32)
                nc.sync.dma_start(
                    Cc_sb[:], Cc.ap().rearrange("(c p) g -> p c g", p=128)
                )
                # preload head weights (overlaps AG3)
                w1 = p5.tile([128, 7, 1536], F32)
                nc.sync.dma_start(
                    w1[:], fcg1W.ap().rearrange("(c p) f -> p c f", p=128)
                )
                w2 = p5.tile([128, 12, 128], F32)
                nc.sync.dma_start(
                    w2[:], fcg2W.ap().rearrange("(c p) f -> p c f", p=128)
                )
                wf1 = p5.tile([128, 2, 1024], F32)
                nc.sync.dma_start(
                    wf1[:], f1W.ap().rearrange("(c p) f -> p c f", p=128)
                )
                wf2 = p5.tile([128, 8, 512], F32)
                nc.sync.dma_start(
                    wf2[:], f2W.ap().rearrange("(c p) f -> p c f", p=128)
                )
                wf3 = p5.tile([128, 4, 256], F32)
                nc.sync.dma_start(
                    wf3[:], f3W.ap().rearrange("(c p) f -> p c f", p=128)
                )
                wf4 = p5.tile([128, 2, 128], F32)
                nc.sync.dma_start(
                    wf4[:], f4W.ap().rearrange("(c p) f -> p c f", p=128)
                )
                wo = p5.tile([128, 1], F32)
                nc.sync.dma_start(wo[:], oW.ap()[:, :])
                ob_sb = p5.tile([1, 1], F32)
                nc.sync.dma_start(ob_sb[:], o_b.ap()[:, :])
                bcols = {}
                for nm, drm, w_ in (
                    ("fcg1", fcg1b_col, 12), ("fcg2", fcg2b_col, 1),
                    ("f1", f1b_col, 8), ("f2", f2b_col, 4), ("f3", f3b_col, 2),
                    ("f4", f4b_col, 1),
                ):
                    bt = p5.tile([128, w_], F32, tag=f"bc_{nm}")
                    nc.sync.dma_start(bt[:], drm.ap()[:, :])
                    bcols[nm] = bt

                pall = p5.tile([128, 4, 784], F32)
                nc.sync.dma_start(
                    pall[:], poolall.ap().rearrange("(c p) f -> p c f", p=128)
                )
                # xgT[f, g] = sum_slots pall[slot, f] * Cc[slot, g]
                xgT = p5.tile([128, 7, BPC], F32)
                for fc in range(7):
                    sz = 128 if fc < 6 else 12
                    xg_ps = ps5.tile([128, BPC], F32, space="PSUM", tag="mmps")
                    for sc in range(4):
                        nc.tensor.matmul(
                            xg_ps[0:sz, :],
                            lhsT=pall[:, sc, fc * 128 : fc * 128 + sz],
                            rhs=Cc_sb[:, sc, :],
                            start=(sc == 0),
                            stop=(sc == 3),
                        )
                    nc.scalar.copy(xgT[0:sz, fc, :], xg_ps[0:sz, :])

                def dense_T(xT_t, kcs, szs, w_sb, ncs, bname, relu, tag):
                    """yT[n, g] = act(W.T @ x + b): returns [128, ncs, BPC] bf16."""
                    yT = p5.tile([128, ncs, BPC], F32, tag=tag)
                    for n_c in range(ncs):
                        yps = ps5.tile([128, BPC], F32, space="PSUM", tag="mmps")
                        for kc in range(kcs):
                            sz = szs[kc]
                            nc.tensor.matmul(
                                yps[:, :],
                                lhsT=w_sb[0:sz, kc, n_c * 128 : (n_c + 1) * 128],
                                rhs=xT_t[0:sz, kc, :],
                                start=(kc == 0),
                                stop=(kc == kcs - 1),
                            )
                        nc.scalar.activation(
                            yT[:, n_c, :],
                            yps[:, :],
                            AF.Relu if relu else AF.Identity,
                            bias=bcols[bname][:, n_c : n_c + 1],
                        )
                    return yT

                y1 = dense_T(xgT, 7, [128] * 6 + [12], w1, 12, "fcg1", True, "y1")
                xgo = dense_T(y1, 12, [128] * 12, w2, 1, "fcg2", False, "xgo")
                xc = p5.tile([128, 2, BPC], F32, tag="xc")
                nc.any.tensor_copy(xc[:, 0, :], xgo[:, 0, :])
                nc.any.tensor_copy(xc[:, 1, :], xtT_sb[:])
                a1 = dense_T(xc, 2, [128, 128], wf1, 8, "f1", True, "a1")
                a2 = dense_T(a1, 8, [128] * 8, wf2, 4, "f2", True, "a2")
                a3 = dense_T(a2, 4, [128] * 4, wf3, 2, "f3", True, "a3")
                a4 = dense_T(a3, 2, [128, 128], wf4, 1, "f4", True, "a4")
                yo_ps = ps5.tile([1, BPC], F32, space="PSUM", tag="yops")
                nc.tensor.matmul(
                    yo_ps[:, :], lhsT=wo[:, 0:1], rhs=a4[:, 0, :],
                    start=True, stop=True,
                )
                yo = p5.tile([1, BPC], F32, tag="yo")
                nc.scalar.activation(
                    yo[:], yo_ps[:], AF.Identity, bias=ob_sb[:, 0:1]
                )
                nc.sync.dma_start(out_d.ap()[:, :], yo[:])
                if KDEBUG:
                    dbg = p5.tile([128, NWIN, RBF], BF16, tag="dbg")
                    nc.sync.dma_start(
                        dbg[:], agin.ap().rearrange("(c p) f -> p c f", p=128)
                    )
                    nc.sync.dma_start(
                        out_ag.ap().rearrange("(c p) f -> p c f", p=128), dbg[:]
                    )
                    psb2 = p5.tile([GSLOT, 784], F32, tag="psb2")
                    nc.any.tensor_copy(psb2[:], poolsb[:])
                    nc.sync.dma_start(out_pool.ap()[:, :], psb2[:])
                    xt2 = p5.tile([128, BPC], F32, tag="xt2")
                    nc.any.tensor_copy(xt2[:], xtT_sb[:])
                    nc.sync.dma_start(out_xt.ap()[:, :], xt2[:])
                    xg2 = p5.tile([128, 7, BPC], F32, tag="xg2")
                    nc.any.tensor_copy(xg2[:], xgT[:])
                    nc.sync.dma_start(
                        out_xg.ap().rearrange("p (a b) -> p a b", b=BPC), xg2[:]
                    )

    nc.compile()
    _CACHE[key] = nc
    return nc


# ---------------------------------------------------------------- entry point


def _ensure_ntff_hook():
    """Install antenv.axon_hooks + register the ctypes NTFF hook if the image
    lacks them (profiling only; failures are non-fatal)."""
    import types

    try:
        import antenv.axon_hooks  # noqa: F401

        if antenv.axon_hooks.get_axon_ntff_profile_hook() is not None:
            return
    except ImportError:
        import antenv

        mod = types.ModuleType("antenv.axon_hooks")
        mod._hook = None

        def set_axon_ntff_profile_hook(h, _m=mod):
            _m._hook = h

        def get_axon_ntff_profile_hook(_m=mod):
            return _m._hook

        mod.set_axon_ntff_profile_hook = set_axon_ntff_profile_hook
        mod.get_axon_ntff_profile_hook = get_axon_ntff_profile_hook
        sys.modules["antenv.axon_hooks"] = mod
        antenv.axon_hooks = mod
    try:
        from antenv.axon_hooks import set_axon_ntff_profile_hook as _set
        from trn_agent_boot.trn_boot import _ntff_profile_via_ctypes

        hook = _ntff_profile_via_ctypes("/opt/axon/libaxon_pjrt.so")
        if hook is not None:
            _set(hook)
    except Exception:
        pass


def kernel(**inputs) -> np.ndarray:
    per_core, meta = host_prep(inputs)
    nc = build_bass(meta)
    in_maps = [{k: np.ascontiguousarray(v) for k, v in d.items()} for d in per_core]
    trace = bool(int(os.environ.get("KERNEL_TRACE", "0")))
    if trace:
        _ensure_ntff_hook()
    res = run_bass_kernel_spmd(nc, in_maps, core_ids=list(range(NCORES)), trace=trace)
    if trace and res.exec_time_ns is not None:
        print(f"HW exec time: {res.exec_time_ns} ns")
        kernel.last_exec_ns = res.exec_time_ns
    out = np.concatenate(
        [res.results[c]["out"][0, :BPC, None] for c in range(NCORES)], 0
    )
    return out.astype(np.float32)


# revision 31
# speedup vs baseline: 1.5169x; 1.1417x over previous
"""Trainium2 Bass kernel for nn_EnhancedGATGCN (GAT -> GCN -> pool -> MLP, + protein conv branch).

Self-contained: host-side sharding prep + 8-core SPMD Bass/Tile device program.

v2 design (all sizes bf16 unless noted):
  - Edges sorted by dst, 8-way dst-sharded; scatter-add via one-hot mask matmuls
    into per-128-dst-window PSUM. tpw exact (no round-to-4). Row tables gathered
    via SWDGE dma_gather alternating across queues; nothing else runs on the
    Pool engine during edge phases (drain-rate paced).
  - GCN aggregates dinv*x1 rows directly (aggregate-then-project): the x1@W
    projection happens per dst window in the GCN epilogue; phase 3 eliminated.
  - Protein branch: conv1d computed as extended-one-hot matmuls (no embedding
    gather): OHx[26*t+v, q] = (tok[s+q+t]==v), C_blk = OHx.T @ Vx with
    Vx[26t+v,:] = emb[v] @ cW[:,:,t].T host-precomputed; 121 valid positions
    per 128-token block. conv bias folded into fxt bias on host. Runs during
    AllGather #1; the fxt matmul runs during AllGather #2.
  - Dense tail kept transposed ([feat, graph]) so no per-layer transposes;
    per-partition column biases.
"""
import os
import sys

import numpy as np

sys.path.insert(0, "/opt/trn_rl_repo")

import ml_dtypes

import concourse.bacc as bacc
import concourse.bass as bass
import concourse.mybir as mybir
import concourse.tile as tile
from concourse.bass_utils import run_bass_kernel_spmd
from concourse.masks import make_identity

F32 = mybir.dt.float32
BF16 = mybir.dt.bfloat16
I16 = mybir.dt.int16
I32 = mybir.dt.int32
F16 = mybir.dt.float16
AF = mybir.ActivationFunctionType
OP = mybir.AluOpType
BF = ml_dtypes.bfloat16

N, E, B, H, F = 20000, 400000, 200, 10, 78
HID = H * F  # 780
SEQ, VOC, EMB, NF, KS = 1000, 26, 128, 26 * 0 + 32, 8
CONV_OUT = SEQ - KS + 1  # 993

NCORES = 8
NPC = N // NCORES  # 2500
NPAD = 2560
NWIN = NPAD // 128  # 20
RBF = 896  # bf16 cols per table row; 1792 B/row (%256==0)
# GAT msg row: [0:780 h | 780:800 a_s 10xf32-packed | 800:810 exd | 810 ones | 811:896 junk]
GSLOT = 64
BPC = B // NCORES  # 25
NBLK = 9  # conv position blocks per graph (121 valid pos each)
TOKB = 136  # tokens shipped per block (121 + 7 tap overlap + pad)
TOKP = NBLK * TOKB  # 1224
NQ = int(os.environ.get("KNQ", "2"))  # swdge queues


# ---------------------------------------------------------------- host prep


def _wrap16(idx, epc):
    a = np.zeros((128, epc // 16), np.int16)
    w = idx.reshape(epc // 16, 16).T
    a[:, :] = np.tile(w, (8, 1))
    return a


def host_prep(inputs):
    x = np.asarray(inputs["x"], np.float32)
    edge_index = np.asarray(inputs["edge_index"], np.int64)
    batch = np.asarray(inputs["batch"], np.int64)
    target = np.asarray(inputs["target"], np.int64)

    loops = np.arange(N, dtype=np.int64)
    src = np.concatenate([edge_index[0], loops])
    dst = np.concatenate([edge_index[1], loops])
    order = np.argsort(dst, kind="stable")
    src, dst = src[order], dst[order]

    core_of = dst // NPC
    dst_local = dst - core_of * NPC
    win = dst_local // 128
    maxw = 0
    per_core_edges = []
    for c in range(NCORES):
        m = core_of == c
        s_c, dl_c, w_c = src[m], dst_local[m], win[m]
        per_core_edges.append((s_c, dl_c, w_c))
        maxw = max(maxw, int(np.bincount(w_c, minlength=NWIN).max()))
    tpw = -(-maxw // 128)
    ntile = NWIN * tpw
    epc = ntile * 128
    nchunk = -(-ntile // 16)

    def remap(n):
        return (n // NPC) * NPAD + (n % NPC)

    cores = []
    for c in range(NCORES):
        s_c, dl_c, w_c = per_core_edges[c]
        es = np.zeros(epc, np.int64)
        ew = np.full(epc, -1000.0, np.float32)
        for w in range(NWIN):
            m = w_c == w
            k = int(m.sum())
            o = w * tpw * 128
            es[o : o + k] = s_c[m]
            ew[o : o + k] = (dl_c[m] - w * 128).astype(np.float32)
        cores.append(dict(es=remap(es), ew=ew))

    # GAT weight pack: [0:780 W | 780:790 W@a_src per head | 790:800 W@a_dst]
    gat_W = np.asarray(inputs["gat_W"], np.float32)
    a_src = np.asarray(inputs["gat_a_src"], np.float32)
    a_dst = np.asarray(inputs["gat_a_dst"], np.float32)
    # fh-interleaved feature layout: col f*H+h <- head-major col h*F+f
    perm = np.arange(HID).reshape(78, 10)
    perm = (perm % 10) * F + (perm // 10) % F  # perm[f*10+h] = h*78+f
    perm = np.array([(c % 10) * F + c // 10 for c in range(HID)])
    wpack = np.zeros((78, 1024), np.float32)
    wpack[:, :HID] = gat_W[:, perm]
    for h in range(H):
        wpack[:, HID + h] = gat_W[:, h * F : (h + 1) * F] @ a_src[h]
        wpack[:, HID + 10 + h] = gat_W[:, h * F : (h + 1) * F] @ a_dst[h]

    gcn_W_pad = np.zeros((896, 784), np.float32)
    gcn_W_pad[:HID, :HID] = np.asarray(inputs["gcn_W"], np.float32)[perm, :]

    # protein: Vx tables + reordered fxt weights (+ conv bias folded into fxt_b)
    emb = np.asarray(inputs["emb"], np.float32)
    cW = np.asarray(inputs["cW"], np.float32)  # [NF, EMB, KS]
    cb = np.asarray(inputs["cb"], np.float32)
    # tap t occupies 32-partition-aligned row group (t%4)*32; rows 26-31 zero
    Vx = np.zeros((KS * 32, NF), np.float32)
    for t in range(KS):
        Vx[t * 32 : t * 32 + VOC] = emb @ cW[:, :, t].T
    fxt_W = np.asarray(inputs["fxt_W"], np.float32)  # [NF*993, 128]
    fxt_b = np.asarray(inputs["fxt_b"], np.float32)
    fxt_b2 = fxt_b + cb @ fxt_W.reshape(NF, CONV_OUT, 128).sum(axis=1)
    fxtW = np.zeros((NF, NBLK, 128, 128), np.float32)
    for blk in range(NBLK):
        s = blk * 121
        n = min(121, CONV_OUT - s)
        fxtW[:, blk, :n] = fxt_W.reshape(NF, CONV_OUT, 128)[:, s : s + n]
    fxtW = fxtW.reshape(NF * NBLK * 128, 128)

    # head weights (transposed-chain layout, bf16) + column biases
    def colbias(b, n):
        nc_ = -(-n // 128)
        col = np.zeros((nc_, 128), np.float32)
        col.reshape(-1)[: len(b)] = b
        return col.T.copy()

    fcg1_W = np.zeros((896, 1536), np.float32)
    fcg1_W[:HID, :1500] = np.asarray(inputs["fcg1_W"], np.float32)
    fcg2_W = np.zeros((1536, 128), np.float32)
    fcg2_W[:1500] = np.asarray(inputs["fcg2_W"], np.float32)

    # graph slot bookkeeping
    gbase = np.array([batch[c * NPC] for c in range(NCORES)], np.int64)
    span = np.array(
        [batch[min(c * NPC + NPC, N) - 1] - gbase[c] + 1 for c in range(NCORES)]
    )
    assert span.max() <= GSLOT, span.max()
    Cc_all = []
    for c in range(NCORES):
        Cmat = np.zeros((NCORES * GSLOT, BPC), np.float32)
        for r in range(NCORES):
            for slot in range(GSLOT):
                g = gbase[r] + slot
                col = g - c * BPC
                if 0 <= col < BPC and g < B:
                    Cmat[r * GSLOT + slot, col] = 1.0
        Cc_all.append(Cmat)

    vmod = np.full((128, 1), -2.0, np.float16)
    for gi in range(4):
        vmod[gi * 32 : gi * 32 + VOC, 0] = np.arange(VOC)

    RT = -(-ntile // 128)
    meta = dict(tpw=tpw, ntile=ntile, epc=epc, nchunk=nchunk, RT=RT)

    per_core = []
    for c in range(NCORES):
        ed_ = cores[c]
        bw = np.full(NPAD, -1000.0, np.float32)
        bw[:NPC] = (batch[c * NPC : (c + 1) * NPC] - gbase[c]).astype(np.float32)
        batchw = bw.reshape(NWIN, 128).T.copy()

        dstw = ed_["ew"].reshape(ntile, 128).T.copy()  # [128, ntile]

        xTc = np.zeros((78, NPAD), np.float32)
        xTc[:, :NPC] = x[c * NPC : (c + 1) * NPC].T

        tg = target[c * BPC : (c + 1) * BPC].astype(np.float32)
        tokba = np.full((128, BPC, TOKP), -1.0, np.float16)
        tokbb = np.full((128, BPC, TOKP), -1.0, np.float16)
        for p in range(128):
            for sh, tob in ((p // 32, tokba), (4 + p // 32, tokbb)):
                for blk in range(NBLK):
                    s0 = blk * 121 + sh
                    n = max(0, min(TOKB, SEQ - s0))
                    tob[p, :, blk * TOKB : blk * TOKB + n] = tg[:, s0 : s0 + n]

        d = {
            "xTc": xTc.astype(np.float16),
            "wpack": wpack.astype(np.float16),
            "src16": _wrap16(ed_["es"], epc),
            "dstw": dstw,
            "batchw": batchw,
            "vmod": vmod,
            "tokba": tokba, "tokbb": tokbb,
            "Vxa": Vx[:128].astype(np.float16),
            "Vxb": Vx[128:].astype(np.float16),
            "fxtW": fxtW.astype(np.float16),
            "fxtb_col": colbias(fxt_b2, 128),
            "gat_b": np.asarray(inputs["gat_b"], np.float32)[perm].reshape(1, HID),
            "gcnW": gcn_W_pad.astype(np.float16),
            "gcn_b": np.asarray(inputs["gcn_b"], np.float32).reshape(1, HID),
            "fcg1W": fcg1_W.astype(np.float16),
            "fcg1b_col": colbias(np.asarray(inputs["fcg1_b"], np.float32), 1536),
            "fcg2W": fcg2_W.astype(np.float16),
            "fcg2b_col": colbias(np.asarray(inputs["fcg2_b"], np.float32), 128),
            "f1W": np.asarray(inputs["f1_W"], np.float16),
            "f1b_col": colbias(np.asarray(inputs["f1_b"], np.float32), 1024),
            "f2W": np.asarray(inputs["f2_W"], np.float16),
            "f2b_col": colbias(np.asarray(inputs["f2_b"], np.float32), 512),
            "f3W": np.asarray(inputs["f3_W"], np.float16),
            "f3b_col": colbias(np.asarray(inputs["f3_b"], np.float32), 256),
            "f4W": np.asarray(inputs["f4_W"], np.float16),
            "f4b_col": colbias(np.asarray(inputs["f4_b"], np.float32), 128),
            "oW": np.asarray(inputs["o_W"], np.float16),
            "o_b": np.asarray(inputs["o_b"], np.float32).reshape(1, 1),
            "Cc": Cc_all[c].astype(np.float16),
        }
        per_core.append(d)
    return per_core, meta


# ---------------------------------------------------------------- device build

_CACHE = {}


def build_bass(meta):
    key = (meta["tpw"], NQ)
    if key in _CACHE:
        return _CACHE[key]

    tpw, ntile, epc, nchunk, RT = (
        meta["tpw"], meta["ntile"], meta["epc"], meta["nchunk"], meta["RT"],
    )

    nc = bacc.Bacc(
        "TRN2",
        target_bir_lowering=False,
        debug=False,
        num_devices=NCORES,
        num_swdge_queues=NQ,
    )

    def inp(name, shape, dt=F32):
        return nc.dram_tensor(name, list(shape), dt, kind="ExternalInput")

    xTc = inp("xTc", (78, NPAD), F16)
    wpack = inp("wpack", (78, 1024), F16)
    src16 = inp("src16", (128, epc // 16), I16)
    dstw = inp("dstw", (128, ntile))
    batchw = inp("batchw", (128, NWIN))
    vmod = inp("vmod", (128, 1), F16)
    tokba = inp("tokba", (128, BPC, TOKP), F16)
    tokbb = inp("tokbb", (128, BPC, TOKP), F16)
    Vxa = inp("Vxa", (128, NF), F16)
    Vxb = inp("Vxb", (128, NF), F16)
    fxtW = inp("fxtW", (NF * NBLK * 128, 128), F16)
    fxtb_col = inp("fxtb_col", (128, 1))
    gat_b = inp("gat_b", (1, HID))
    gcnW = inp("gcnW", (896, 784), F16)
    gcn_b = inp("gcn_b", (1, HID))
    fcg1W = inp("fcg1W", (896, 1536), F16)
    fcg1b_col = inp("fcg1b_col", (128, 12))
    fcg2W = inp("fcg2W", (1536, 128), F16)
    fcg2b_col = inp("fcg2b_col", (128, 1))
    f1W = inp("f1W", (256, 1024), F16)
    f1b_col = inp("f1b_col", (128, 8))
    f2W = inp("f2W", (1024, 512), F16)
    f2b_col = inp("f2b_col", (128, 4))
    f3W = inp("f3W", (512, 256), F16)
    f3b_col = inp("f3b_col", (128, 2))
    f4W = inp("f4W", (256, 128), F16)
    f4b_col = inp("f4b_col", (128, 1))
    oW = inp("oW", (128, 1), F16)
    o_b = inp("o_b", (1, 1))
    Cc = inp("Cc", (NCORES * GSLOT, BPC), F16)

    out_d = nc.dram_tensor("out", [1, BPC], F32, kind="ExternalOutput")
    KDEBUG = bool(int(os.environ.get("KDEBUG", "0")))
    if KDEBUG:
        out_ag = nc.dram_tensor("out_ag", [NPAD, RBF], F16, kind="ExternalOutput")
        out_pool = nc.dram_tensor("out_pool", [GSLOT, 784], F32, kind="ExternalOutput")
        out_xt = nc.dram_tensor("out_xt", [128, BPC], F32, kind="ExternalOutput")
        out_xg = nc.dram_tensor("out_xg", [128, 7 * BPC], F32, kind="ExternalOutput")

    hin = nc.dram_tensor("hin", [NPAD, RBF], F16)
    htabG = nc.dram_tensor("htabG", [NCORES * NPAD, RBF], F16, addr_space="Shared")
    agin = nc.dram_tensor("agin", [NPAD, RBF], F16)
    htab2G = nc.dram_tensor("htab2G", [NCORES * NPAD, RBF], F16, addr_space="Shared")
    poolin = nc.dram_tensor("poolin", [GSLOT, 784], F16)
    poolall = nc.dram_tensor("poolall", [NCORES * GSLOT, 784], F16, addr_space="Shared")

    RG = [list(range(NCORES))]

    with tile.TileContext(nc) as tc:
        import contextlib

        ctx = contextlib.ExitStack()
        with ctx:
            pers = ctx.enter_context(tc.tile_pool(name="pers", bufs=1))

            # consts
            iota_i = pers.tile([128, 128], I32)
            nc.gpsimd.iota(iota_i[:], pattern=[[1, 128]], base=0, channel_multiplier=0)
            iota_f = pers.tile([128, 128], F32)
            nc.vector.tensor_copy(iota_f[:], iota_i[:])
            ident_bf = pers.tile([128, 128], F16)
            identf = pers.tile([128, 128], F32)
            make_identity(nc, identf[:])
            nc.vector.tensor_copy(ident_bf[:], identf[:])
            ones1 = pers.tile([1, 128], F32)
            nc.gpsimd.memset(ones1[:], 1.0)
            onesc = pers.tile([128, 16], F16)
            nc.gpsimd.memset(onesc[:], 1.0)

            # residents
            dstw_t = pers.tile([128, ntile], F32)
            nc.sync.dma_start(dstw_t[:], dstw[:, :])
            batchw_t = pers.tile([128, NWIN], F32)
            nc.sync.dma_start(batchw_t[:], batchw[:, :])
            src_t = pers.tile([128, epc // 16], I16)
            nc.sync.dma_start(src_t[:], src16[:, :])
            vmod_t = pers.tile([128, 1], F16)
            nc.sync.dma_start(vmod_t[:], vmod[:, :])
            Vxa_t = pers.tile([128, NF], F16)
            nc.sync.dma_start(Vxa_t[:], Vxa[:, :])
            Vxb_t = pers.tile([128, NF], F16)
            nc.sync.dma_start(Vxb_t[:], Vxb[:, :])
            fxtb_t = pers.tile([128, 1], F32)
            nc.sync.dma_start(fxtb_t[:], fxtb_col[:, :])

            dinv_all = pers.tile([128, NWIN], F32)
            adw_all = pers.tile([128, NWIN, 10], F16)
            cT = pers.tile([128, NBLK, NF, BPC], F16)
            xtT_sb = pers.tile([128, BPC], F16)

            # broadcast biases (row-replicated tiles)
            bias_tiles = {}
            with tc.tile_pool(name="psB", bufs=1, space="PSUM") as psB:

                def bcast_bias(dram, width, name):
                    t = pers.tile([128, width], F32, tag=f"bc_{name}")
                    row = pers.tile([1, width], F32, tag=f"br_{name}")
                    nc.sync.dma_start(row[:], dram[0:1, :])
                    for n0 in range(0, width, 512):
                        nn = min(512, width - n0)
                        ps = psB.tile([128, 512], F32, space="PSUM", tag="bcps")
                        nc.tensor.matmul(
                            ps[:, :nn], lhsT=ones1[:], rhs=row[:, n0 : n0 + nn],
                            start=True, stop=True,
                        )
                        nc.any.tensor_copy(t[:, n0 : n0 + nn], ps[:, :nn])
                    return t

                gatb_bc = bcast_bias(gat_b, HID, "gatb")
                gcnb_bc = bcast_bias(gcn_b, HID, "gcnb")

            # ---- phase 1: own h rows ----
            with (
                tc.tile_pool(name="p1", bufs=1) as p1,
                tc.tile_pool(name="p1h", bufs=3) as p1h,
                tc.tile_pool(name="ps1", bufs=2, space="PSUM") as ps1,
            ):
                xT_sb = p1.tile([78, NPAD], F16)
                nc.sync.dma_start(xT_sb[:], xTc[:, :])
                wp_sb = p1.tile([78, 1024], F16)
                nc.sync.dma_start(wp_sb[:], wpack[:, :])
                for t in range(NWIN):
                    hp = ps1.tile([128, 1024], F32, space="PSUM", tag="hp")
                    for n0 in (0, 512):
                        nc.tensor.matmul(
                            hp[:, n0 : n0 + 512],
                            lhsT=xT_sb[:, t * 128 : (t + 1) * 128],
                            rhs=wp_sb[:, n0 : n0 + 512],
                            start=True,
                            stop=True,
                        )
                    hrow = p1h.tile([128, 800], F16, tag="hrow")
                    nc.vector.tensor_copy(hrow[:, 0:HID], hp[:, 0:HID])
                    nc.vector.tensor_copy(
                        hrow[:, 780:800].bitcast(F32), hp[:, 780:790]
                    )
                    nc.vector.tensor_copy(adw_all[:, t, :], hp[:, 790:800])
                    nc.sync.dma_start(
                        hin.ap()[t * 128 : (t + 1) * 128, 0:800], hrow[:]
                    )
                nc.gpsimd.collective_compute(
                    "AllGather",
                    OP.bypass,
                    replica_groups=RG,
                    ins=[hin.ap().opt()],
                    outs=[htabG.ap().opt()],
                )

            # ---- protein conv (runs during AllGather #1; no graph deps) ----
            ppo = ctx.enter_context(tc.tile_pool(name="ppo", bufs=2))
            ppt = ctx.enter_context(tc.tile_pool(name="ppt", bufs=3))
            with tc.tile_pool(name="psCq", bufs=2, space="PSUM") as psCq:
                for g in range(BPC):
                    tokrA = ppt.tile([128, TOKP], F16, tag="tokrA")
                    nc.sync.dma_start(tokrA[:], tokba.ap()[:, g, :])
                    tokrB = ppt.tile([128, TOKP], F16, tag="tokrB")
                    nc.sync.dma_start(tokrB[:], tokbb.ap()[:, g, :])
                    OHa = ppo.tile([128, NBLK, 128], F16, tag="OHa")
                    OHb = ppo.tile([128, NBLK, 128], F16, tag="OHb")
                    for tok, OH in ((tokrA, OHa), (tokrB, OHb)):
                        nc.vector.tensor_tensor(
                            OH[:],
                            tok.rearrange("p (b q) -> p b q", q=TOKB)[:, :, 0:128],
                            vmod_t[:, :, None].to_broadcast([128, NBLK, 128]),
                            op=OP.is_equal,
                        )
                    Cq = psCq.tile([128, NBLK, NF], F32, space="PSUM", tag="Cq")
                    for blk in range(NBLK):
                        nc.tensor.matmul(
                            Cq[:, blk, :], lhsT=OHa[:, blk, :], rhs=Vxa_t[:],
                            start=True, stop=False,
                        )
                        nc.tensor.matmul(
                            Cq[:, blk, :], lhsT=OHb[:, blk, :], rhs=Vxb_t[:],
                            start=False, stop=True,
                        )
                    nc.scalar.copy(cT[:, :, :, g], Cq[:, :, :])

            # ---- fxt matmul (fills the AllGather #1 window) ----
            with (
                tc.tile_pool(name="fxp", bufs=2) as fxp,
                tc.tile_pool(name="fxw", bufs=2) as fxw,
                tc.tile_pool(name="psX", bufs=1, space="PSUM") as psX,
                tc.tile_pool(name="psXT", bufs=1, space="PSUM") as psXT,
            ):
                xt_ps = psX.tile([BPC, 128], F32, space="PSUM", tag="xtps")
                NR = NF * NBLK  # 288
                for sc in range(NR // 16):
                    wpt = fxw.tile([128, 16, 128], F16, tag="wpt")
                    nc.sync.dma_start(
                        wpt[:],
                        fxtW.ap()[sc * 2048 : (sc + 1) * 2048, :].rearrange(
                            "(c p) j -> p c j", p=128
                        ),
                    )
                    for sub in range(16):
                        r = sc * 16 + sub
                        ch, blk = r // NBLK, r % NBLK
                        nc.tensor.matmul(
                            xt_ps[:, :],
                            lhsT=cT[:, blk, ch, :],
                            rhs=wpt[:, sub, :],
                            start=(r == 0),
                            stop=(r == NR - 1),
                        )
                xt_sb = fxp.tile([BPC, 128], F16, tag="xtsb")
                nc.vector.tensor_copy(xt_sb[:], xt_ps[:])
                xtT_ps = psXT.tile([128, BPC], F16, space="PSUM", tag="xtT")
                nc.tensor.transpose(xtT_ps[:, :], xt_sb[:, :], ident_bf[0:BPC, 0:BPC])
                nc.scalar.activation(
                    xtT_sb[:], xtT_ps[:], AF.Identity, bias=fxtb_t[:, 0:1]
                )
                # preload gcn weights while AG2 is still in flight
                gcnw_sb = pers.tile([128, 7, 784], F16)
                nc.sync.dma_start(
                    gcnw_sb[:], gcnW.ap().rearrange("(c p) f -> p c f", p=128)
                )

            # ---- phase 2: GAT edge phase ----
            def edge_phase(table, gat, epilogue):
                with (
                    tc.tile_pool(name="msgp", bufs=2) as msgp,
                    tc.tile_pool(name="maskp", bufs=3) as maskp,
                    tc.tile_pool(name="mtp", bufs=2) as mtp,
                    tc.tile_pool(name="smallp", bufs=2) as smallp,
                    tc.tile_pool(name="epip", bufs=2) as epip,
                    tc.tile_pool(name="psA", bufs=2, space="PSUM") as psA,
                    tc.tile_pool(name="psS", bufs=2, space="PSUM") as psS,
                    tc.tile_pool(name="psD", bufs=2, space="PSUM") as psD,
                ):
                    aggp = None
                    for c in range(nchunk):
                        T = min(16, ntile - c * 16)
                        msg = msgp.tile([128, 16, RBF], F16, tag="msg")
                        nc.gpsimd.dma_gather(
                            msg[:, 0:T, :],
                            table.ap()[:, 0:RBF],
                            src_t[:, c * 128 : c * 128 + T * 8],
                            num_idxs=T * 128,
                            num_idxs_reg=T * 128,
                            elem_size=RBF,
                            elem_step=RBF,
                            single_packet=False,
                            queue_num=c % NQ,
                        )
                        maskall = maskp.tile([128, 16, 128], F16, tag="maskall")
                        if gat:
                            nc.scalar.copy(msg[:, 0:T, 810:811], onesc[:, 0:T, None])
                            sall = smallp.tile([128, 16, 10], F32, tag="sall")
                        for q4 in range(-(-T // 4)):
                            q4n = min(4, T - q4 * 4)
                            jsl = slice(q4 * 4, q4 * 4 + q4n)
                            g4 = c * 16 + q4 * 4
                            nc.vector.tensor_tensor(
                                maskall[:, jsl, :],
                                dstw_t[:, g4 : g4 + q4n, None].to_broadcast(
                                    [128, q4n, 128]
                                ),
                                iota_f[:, None, :].to_broadcast([128, q4n, 128]),
                                op=OP.is_equal,
                            )
                            if not gat:
                                continue
                            trT = psD.tile([128, 512], F16, space="PSUM", tag="trT")
                            for i in range(q4n):
                                nc.tensor.transpose(
                                    trT[:, i * 128 : (i + 1) * 128],
                                    maskall[:, q4 * 4 + i, :],
                                    ident_bf[:],
                                )
                            maskT = mtp.tile([128, 4, 128], F16, tag="maskT")
                            nc.scalar.copy(
                                maskT[:, 0:q4n, :],
                                trT[:, 0 : q4n * 128].rearrange(
                                    "p (a b) -> p a b", b=128
                                ),
                            )
                            adx = psS.tile([128, 4, 16], F32, space="PSUM", tag="adx")
                            for i in range(q4n):
                                nc.tensor.matmul(
                                    adx[:, i, 0:10],
                                    lhsT=maskT[:, i, :],
                                    rhs=adw_all[:, (g4 + i) // tpw, :],
                                    start=True,
                                    stop=True,
                                )
                            adxs = smallp.tile([128, 4, 16], F32, tag="adxs")
                            nc.scalar.copy(
                                adxs[:, 0:q4n, 0:10], adx[:, 0:q4n, 0:10]
                            )
                            nc.vector.tensor_tensor(
                                sall[:, jsl, :],
                                msg[:, jsl, 780:800].bitcast(F32),
                                adxs[:, 0:q4n, 0:10],
                                op=OP.add,
                            )
                            s2 = smallp.tile([128, 4, 10], F32, tag="s2")
                            nc.vector.tensor_scalar_mul(
                                s2[:, 0:q4n, :], sall[:, jsl, :], 0.2
                            )
                            nc.vector.tensor_tensor(
                                sall[:, jsl, :], sall[:, jsl, :], s2[:, 0:q4n, :],
                                op=OP.max,
                            )
                            nc.scalar.activation(
                                msg[:, jsl, 800:810], sall[:, jsl, :], AF.Exp
                            )
                            nc.vector.tensor_tensor(
                                msg[:, jsl, 0:HID].rearrange(
                                    "p c (f h) -> p c f h", h=H
                                ),
                                msg[:, jsl, 0:HID].rearrange(
                                    "p c (f h) -> p c f h", h=H
                                ),
                                msg[:, jsl, None, 800:810].to_broadcast(
                                    [128, q4n, F, H]
                                ),
                                op=OP.mult,
                            )
                        n_hi = 811 if gat else HID
                        for j in range(T):
                            g = c * 16 + j
                            w = g // tpw
                            first = g % tpw == 0
                            last = g % tpw == tpw - 1
                            if first:
                                aggp = psA.tile(
                                    [128, 1024], F32, space="PSUM", tag="aggp"
                                )
                            for n0, nn in ((0, 512), (512, n_hi - 512)):
                                nc.tensor.matmul(
                                    aggp[:, n0 : n0 + nn],
                                    lhsT=maskall[:, j, :],
                                    rhs=msg[:, j, n0 : n0 + nn],
                                    start=first,
                                    stop=last,
                                )
                            if last:
                                epilogue(w, aggp, epip)

            def gat_epilogue(w, aggp, epip):
                aggsb = epip.tile([128, 816], F32, tag="aggsb")
                nc.scalar.copy(aggsb[:, 0:811], aggp[:, 0:811])
                rec = epip.tile([128, 12], F32, tag="rec")
                nc.vector.tensor_scalar_add(rec[:, 0:11], aggsb[:, 800:811], 1e-20)
                rcp = epip.tile([128, 12], F32, tag="rcp")
                nc.vector.reciprocal(rcp[:, 0:10], rec[:, 0:10])
                nc.scalar.activation(rcp[:, 10:11], rec[:, 10:11], AF.Sqrt)
                nc.vector.reciprocal(dinv_all[:, w : w + 1], rcp[:, 10:11])
                x1w = epip.tile([128, HID], F32, tag="x1w")
                nc.vector.tensor_tensor(
                    x1w[:].rearrange("p (f h) -> p f h", h=H),
                    aggsb[:, 0:HID].rearrange("p (f h) -> p f h", h=H),
                    rcp[:, None, 0:10].to_broadcast([128, F, H]),
                    op=OP.mult,
                )
                nc.vector.tensor_tensor(x1w[:], x1w[:], gatb_bc[:], op=OP.add)
                agrow = epip.tile([128, HID], F16, tag="agrow")
                nc.scalar.activation(
                    agrow[:], x1w[:], AF.Relu, scale=dinv_all[:, w : w + 1]
                )
                nc.sync.dma_start(agin.ap()[w * 128 : (w + 1) * 128, 0:HID], agrow[:])

            edge_phase(htabG, True, gat_epilogue)

            nc.gpsimd.collective_compute(
                "AllGather",
                OP.bypass,
                replica_groups=RG,
                ins=[agin.ap().opt()],
                outs=[htab2G.ap().opt()],
            )

            # ---- phase 4: GCN edge phase (aggregate x1*dinv, project, pool) ----
            with (
                tc.tile_pool(name="psP", bufs=1, space="PSUM") as psP,
                tc.tile_pool(name="psTr", bufs=1, space="PSUM") as psTr,
                tc.tile_pool(name="psH", bufs=1, space="PSUM") as psH,
            ):
                poolps = psP.tile([GSLOT, 784], F32, space="PSUM", tag="poolps")

                def gcn_epilogue(w, aggp, epip):
                    aggs = epip.tile([128, HID], F16, tag="aggs")
                    nc.scalar.copy(aggs[:], aggp[:, 0:HID])
                    aT = epip.tile([128, 7, 128], F16, tag="aT")
                    for kc in range(7):
                        sz = 128 if kc < 6 else 12
                        trp = psTr.tile([128, 128], F16, space="PSUM", tag="trp")
                        nc.tensor.transpose(
                            trp[0:sz, :], aggs[:, kc * 128 : kc * 128 + sz],
                            ident_bf[:],
                        )
                        nc.scalar.copy(aT[0:sz, kc, :], trp[0:sz, :])
                    x2w = epip.tile([128, HID], F16, tag="x2w")
                    for n0, nn in ((0, 512), (512, 268)):
                        h2ps = psH.tile([128, 512], F32, space="PSUM", tag="h2ps")
                        for kc in range(7):
                            sz = 128 if kc < 6 else 12
                            nc.tensor.matmul(
                                h2ps[:, 0:nn],
                                lhsT=aT[0:sz, kc, :],
                                rhs=gcnw_sb[0:sz, kc, n0 : n0 + nn],
                                start=(kc == 0),
                                stop=(kc == 6),
                            )
                        x2f = epip.tile([128, 512], F32, tag="x2f")
                        nc.scalar.activation(
                            x2f[:, 0:nn], h2ps[:, 0:nn], AF.Identity,
                            scale=dinv_all[:, w : w + 1],
                        )
                        nc.vector.tensor_tensor(
                            x2f[:, 0:nn], x2f[:, 0:nn], gcnb_bc[:, n0 : n0 + nn],
                            op=OP.add,
                        )
                        nc.scalar.activation(
                            x2w[:, n0 : n0 + nn], x2f[:, 0:nn], AF.Relu
                        )
                    ph = epip.tile([128, GSLOT], F16, tag="poolhot")
                    nc.vector.tensor_tensor(
                        ph[:],
                        batchw_t[:, w : w + 1].to_broadcast([128, GSLOT]),
                        iota_f[:, 0:GSLOT],
                        op=OP.is_equal,
                    )
                    for n0, nn in ((0, 512), (512, 268)):
                        nc.tensor.matmul(
                            poolps[:, n0 : n0 + nn],
                            lhsT=ph[:],
                            rhs=x2w[:, n0 : n0 + nn],
                            start=(w == 0),
                            stop=(w == NWIN - 1),
                        )

                edge_phase(htab2G, False, gcn_epilogue)
                poolsb = pers.tile([GSLOT, 784], F16)
                nc.any.tensor_copy(poolsb[:, 0:HID], poolps[:, 0:HID])
                nc.gpsimd.memset(poolsb[:, HID:784], 0.0)

            # ---- pool AllGather + transposed dense tail ----
            with (
                tc.tile_pool(name="p5", bufs=1) as p5,
                tc.tile_pool(name="p5w", bufs=2) as p5w,
                tc.tile_pool(name="ps5", bufs=2, space="PSUM") as ps5,
            ):
                nc.sync.dma_start(poolin.ap()[:, :], poolsb[:])
                nc.gpsimd.collective_compute(
                    "AllGather",
                    OP.bypass,
                    replica_groups=RG,
                    ins=[poolin.ap().opt()],
                    outs=[poolall.ap().opt()],
                )
                Cc_sb = p5.tile([128, 4, BPC], F16)
                nc.sync.dma_start(
                    Cc_sb[:], Cc.ap().rearrange("(c p) g -> p c g", p=128)
                )
                # preload head weights (overlaps AG3)
                w1 = p5.tile([128, 7, 1536], F16)
                nc.sync.dma_start(
                    w1[:], fcg1W.ap().rearrange("(c p) f -> p c f", p=128)
                )
                w2 = p5.tile([128, 12, 128], F16)
                nc.sync.dma_start(
                    w2[:], fcg2W.ap().rearrange("(c p) f -> p c f", p=128)
                )
                wf1 = p5.tile([128, 2, 1024], F16)
                nc.sync.dma_start(
                    wf1[:], f1W.ap().rearrange("(c p) f -> p c f", p=128)
                )
                wf2 = p5.tile([128, 8, 512], F16)
                nc.sync.dma_start(
                    wf2[:], f2W.ap().rearrange("(c p) f -> p c f", p=128)
                )
                wf3 = p5.tile([128, 4, 256], F16)
                nc.sync.dma_start(
                    wf3[:], f3W.ap().rearrange("(c p) f -> p c f", p=128)
                )
                wf4 = p5.tile([128, 2, 128], F16)
                nc.sync.dma_start(
                    wf4[:], f4W.ap().rearrange("(c p) f -> p c f", p=128)
                )
                wo = p5.tile([128, 1], F16)
                nc.sync.dma_start(wo[:], oW.ap()[:, :])
                ob_sb = p5.tile([1, 1], F32)
                nc.sync.dma_start(ob_sb[:], o_b.ap()[:, :])
                bcols = {}
                for nm, drm, w_ in (
                    ("fcg1", fcg1b_col, 12), ("fcg2", fcg2b_col, 1),
                    ("f1", f1b_col, 8), ("f2", f2b_col, 4), ("f3", f3b_col, 2),
                    ("f4", f4b_col, 1),
                ):
                    bt = p5.tile([128, w_], F32, tag=f"bc_{nm}")
                    nc.sync.dma_start(bt[:], drm.ap()[:, :])
                    bcols[nm] = bt

                pall = p5.tile([128, 4, 784], F16)
                nc.sync.dma_start(
                    pall[:], poolall.ap().rearrange("(c p) f -> p c f", p=128)
                )
                # xgT[f, g] = sum_slots pall[slot, f] * Cc[slot, g]
                xgT = p5.tile([128, 7, BPC], F16)
                for fc in range(7):
                    sz = 128 if fc < 6 else 12
                    xg_ps = ps5.tile([128, BPC], F32, space="PSUM", tag="mmps")
                    for sc in range(4):
                        nc.tensor.matmul(
                            xg_ps[0:sz, :],
                            lhsT=pall[:, sc, fc * 128 : fc * 128 + sz],
                            rhs=Cc_sb[:, sc, :],
                            start=(sc == 0),
                            stop=(sc == 3),
                        )
                    nc.scalar.copy(xgT[0:sz, fc, :], xg_ps[0:sz, :])

                def dense_T(xT_t, kcs, szs, w_sb, ncs, bname, relu, tag):
                    """yT[n, g] = act(W.T @ x + b): returns [128, ncs, BPC] bf16."""
                    yT = p5.tile([128, ncs, BPC], F16, tag=tag)
                    for n_c in range(ncs):
                        yps = ps5.tile([128, BPC], F32, space="PSUM", tag="mmps")
                        for kc in range(kcs):
                            sz = szs[kc]
                            nc.tensor.matmul(
                                yps[:, :],
                                lhsT=w_sb[0:sz, kc, n_c * 128 : (n_c + 1) * 128],
                                rhs=xT_t[0:sz, kc, :],
                                start=(kc == 0),
                                stop=(kc == kcs - 1),
                            )
                        nc.scalar.activation(
                            yT[:, n_c, :],
                            yps[:, :],
                            AF.Relu if relu else AF.Identity,
                            bias=bcols[bname][:, n_c : n_c + 1],
                        )
                    return yT

                y1 = dense_T(xgT, 7, [128] * 6 + [12], w1, 12, "fcg1", True, "y1")
                xgo = dense_T(y1, 12, [128] * 12, w2, 1, "fcg2", False, "xgo")
                xc = p5.tile([128, 2, BPC], F16, tag="xc")
                nc.any.tensor_copy(xc[:, 0, :], xgo[:, 0, :])
                nc.any.tensor_copy(xc[:, 1, :], xtT_sb[:])
                a1 = dense_T(xc, 2, [128, 128], wf1, 8, "f1", True, "a1")
                a2 = dense_T(a1, 8, [128] * 8, wf2, 4, "f2", True, "a2")
                a3 = dense_T(a2, 4, [128] * 4, wf3, 2, "f3", True, "a3")
                a4 = dense_T(a3, 2, [128, 128], wf4, 1, "f4", True, "a4")
                yo_ps = ps5.tile([1, BPC], F32, space="PSUM", tag="yops")
                nc.tensor.matmul(
                    yo_ps[:, :], lhsT=wo[:, 0:1], rhs=a4[:, 0, :],
                    start=True, stop=True,
                )
                yo = p5.tile([1, BPC], F32, tag="yo")
                nc.scalar.activation(
                    yo[:], yo_ps[:], AF.Identity, bias=ob_sb[:, 0:1]
                )
                nc.sync.dma_start(out_d.ap()[:, :], yo[:])
                if KDEBUG:
                    dbg = p5.tile([128, NWIN, RBF], BF16, tag="dbg")
                    nc.sync.dma_start(
                        dbg[:], agin.ap().rearrange("(c p) f -> p c f", p=128)
                    )
                    nc.sync.dma_start(
                        out_ag.ap().rearrange("(c p) f -> p c f", p=128), dbg[:]
                    )
                    psb2 = p5.tile([GSLOT, 784], F32, tag="psb2")
                    nc.any.tensor_copy(psb2[:], poolsb[:])
                    nc.sync.dma_start(out_pool.ap()[:, :], psb2[:])
                    xt2 = p5.tile([128, BPC], F32, tag="xt2")
                    nc.any.tensor_copy(xt2[:], xtT_sb[:])
                    nc.sync.dma_start(out_xt.ap()[:, :], xt2[:])
                    xg2 = p5.tile([128, 7, BPC], F32, tag="xg2")
                    nc.any.tensor_copy(xg2[:], xgT[:])
                    nc.sync.dma_start(
                        out_xg.ap().rearrange("p (a b) -> p a b", b=BPC), xg2[:]
                    )

    nc.compile()
    _CACHE[key] = nc
    return nc


# ---------------------------------------------------------------- entry point


def _ensure_ntff_hook():
    """Install antenv.axon_hooks + register the ctypes NTFF hook if the image
    lacks them (profiling only; failures are non-fatal)."""
    import types

    try:
        import antenv.axon_hooks  # noqa: F401

        if antenv.axon_hooks.get_axon_ntff_profile_hook() is not None:
            return
    except ImportError:
        import antenv

        mod = types.ModuleType("antenv.axon_hooks")
        mod._hook = None

        def set_axon_ntff_profile_hook(h, _m=mod):
            _m._hook = h

        def get_axon_ntff_profile_hook(_m=mod):
            return _m._hook

        mod.set_axon_ntff_profile_hook = set_axon_ntff_profile_hook
        mod.get_axon_ntff_profile_hook = get_axon_ntff_profile_hook
        sys.modules["antenv.axon_hooks"] = mod
        antenv.axon_hooks = mod
    try:
        from antenv.axon_hooks import set_axon_ntff_profile_hook as _set
        from trn_agent_boot.trn_boot import _ntff_profile_via_ctypes

        hook = _ntff_profile_via_ctypes("/opt/axon/libaxon_pjrt.so")
        if hook is not None:
            _set(hook)
    except Exception:
        pass


def kernel(**inputs) -> np.ndarray:
    per_core, meta = host_prep(inputs)
    nc = build_bass(meta)
    in_maps = [{k: np.ascontiguousarray(v) for k, v in d.items()} for d in per_core]
    trace = bool(int(os.environ.get("KERNEL_TRACE", "0")))
    if trace:
        _ensure_ntff_hook()
    res = run_bass_kernel_spmd(nc, in_maps, core_ids=list(range(NCORES)), trace=trace)
    if trace and res.exec_time_ns is not None:
        print(f"HW exec time: {res.exec_time_ns} ns")
        kernel.last_exec_ns = res.exec_time_ns
    out = np.concatenate(
        [res.results[c]["out"][0, :BPC, None] for c in range(NCORES)], 0
    )
    return out.astype(np.float32)
